# revision 1
# baseline (speedup 1.0000x reference)
"""Trainium2 Bass kernel for segment-causal GQA attention (nn_Attention_31722628448794).

Sharding: 8 cores = batch (2) x kv-head (4). Each core computes its batch's
4 q-heads / 1 kv-head slice end-to-end (QKV proj + RoPE + RMS-norm + block-
sparse attention + partial output projection over its 512 rows of Wo).
Host transposes x, permutes Wq columns, tiles weights, precomputes RoPE
tables / segment masks, and sums the 4 partial outputs per batch (row-
parallel Wo unshard).

All matmuls run as float32r (FP22 multiply, fp32 accumulate) which is 4x
faster than true fp32 on the PE at moving-dim >= 256, ~1e-4 relative error.

Device layouts (per core, T=1024, D=2048, H=128, G=4 q-heads):
  xT      [D, T]   x[b] transposed (host)
  qT/kT   [h, t]   projections computed transposed: lhsT=W-tile, rhs=xT-tile
  V       [s, h]   via VT [h,t] projection + 8 PE transposes
  logits  [s, t]   lhsT=kT s-tile, rhs=qT t-chunk; softmax over s = partition
                   dim; no max-subtraction (|logit| <= sqrt(H) after RMS
                   norm); denominators via ones-matmuls; SCALE*rstd_k folded
                   into the exp() per-partition scale operand; rstd_q applied
                   to qT via K=4 broadcast matmuls.
  out     [t, d]   lhsT=qkvT t-tile, rhs=Wo tile, accumulated over 4 heads.
"""

import sys

sys.path.insert(0, "/opt/trn_rl_repo")

import numpy as np

import concourse.bacc as bacc
import concourse.bass as bass  # noqa: F401
import concourse.tile as tile
from concourse import mybir
from concourse.bass_utils import run_bass_kernel_spmd

B, T, D = 2, 1024, 2048
N, K, H = 16, 4, 128
G = N // K
EPS = 1e-6
SCALE = H ** -0.5
ROPE_BASE = 10000.0
NCHUNK = 2          # t chunks of 512
CW = T // NCHUNK    # 512
NS = T // 128       # 8 s-tiles
ND = D // 128       # 16 d-tiles
F32 = mybir.dt.float32
F32R = mybir.dt.float32r
MULT = mybir.AluOpType.mult

LAST_RESULTS = None  # test harness reads exec_time_ns from here


def _positions(seg):
    t = seg.shape[0]
    idx = np.arange(t, dtype=np.int64)
    is_start = np.concatenate([[True], seg[1:] != seg[:-1]])
    seg_start = np.maximum.accumulate(np.where(is_start, idx, 0))
    return (idx - seg_start).astype(np.float32)


def _classify(seg_rows):
    """Union tile classification over batches.

    Returns (plan, masks_per_batch): plan[c] = list of (si, kind, mask_idx);
    masks_per_batch[b] = float32 [max(n_masks,1), 128, CW] of 0/1.
    """
    idx = np.arange(T)
    valids = []
    for b in range(B):
        seg = seg_rows[b]
        valids.append((seg[:, None] == seg[None, :]) & (idx[:, None] <= idx[None, :]))
    plan = []
    mask_list = [[] for _ in range(B)]
    n_masks = 0
    for c in range(NCHUNK):
        t0 = c * CW
        entries = []
        for si in range(NS):
            s0 = si * 128
            subs = [v[s0:s0 + 128, t0:t0 + CW] for v in valids]
            if not any(s.any() for s in subs):
                continue
            if all(s.all() for s in subs):
                entries.append((si, "full", -1))
            else:
                for b in range(B):
                    mask_list[b].append(subs[b].astype(np.float32))
                entries.append((si, "partial", n_masks))
                n_masks += 1
        plan.append(entries)
    masks = []
    for b in range(B):
        if n_masks:
            masks.append(np.ascontiguousarray(np.stack(mask_list[b]), np.float32))
        else:
            masks.append(np.zeros((1, 128, CW), np.float32))
    return plan, masks


def _build_nc(plan, n_masks):
    from contextlib import ExitStack

    nc = bacc.Bacc(None, target_bir_lowering=False, debug=False)
    dt = F32
    xT_d = nc.dram_tensor("xT", [D, T], F32R, kind="ExternalInput")
    wq_d = nc.dram_tensor("wq", [G, ND, 128, 128], F32R, kind="ExternalInput")
    wk_d = nc.dram_tensor("wk", [ND, 128, 128], F32R, kind="ExternalInput")
    wv_d = nc.dram_tensor("wv", [ND, 128, 128], F32R, kind="ExternalInput")
    wo_d = nc.dram_tensor("wo", [G, 128, D], F32R, kind="ExternalInput")
    cos2_d = nc.dram_tensor("cos2", [128, T], dt, kind="ExternalInput")
    sin2_d = nc.dram_tensor("sin2", [128, T], dt, kind="ExternalInput")
    qsc_d = nc.dram_tensor("qsc", [128, 2], dt, kind="ExternalInput")
    ksc_d = nc.dram_tensor("ksc", [128, 2], dt, kind="ExternalInput")
    sel_d = nc.dram_tensor("sel", [128, 2], F32R, kind="ExternalInput")
    bc_d = nc.dram_tensor("bc", [2, 128], F32R, kind="ExternalInput")
    ones_d = nc.dram_tensor("ones", [128, 2], F32R, kind="ExternalInput")
    onesr_d = nc.dram_tensor("onesr", [1, 128], F32R, kind="ExternalInput")
    iden_d = nc.dram_tensor("iden", [128, 128], dt, kind="ExternalInput")
    biasc_d = nc.dram_tensor("biasc", [128, 2], dt, kind="ExternalInput")
    msk_d = nc.dram_tensor("masks", [n_masks, 128, CW], mybir.dt.bfloat16, kind="ExternalInput")
    out_d = nc.dram_tensor("out", [T, D], dt, kind="ExternalOutput")

    def r(ap):
        return ap.bitcast(F32R)

    es = ExitStack()
    with es:
        es.enter_context(nc.allow_low_precision("fp32r matmul operands"))
        tc = es.enter_context(tile.TileContext(nc))
        pool = lambda *a, **k: es.enter_context(tc.tile_pool(*a, **k))
        pp = pool(name="persist", bufs=1)

        es1 = ExitStack()
        pool1 = lambda *a, **k: es1.enter_context(tc.tile_pool(*a, **k))
        xtp = pool1(name="xt", bufs=1)
        wtp = pool1(name="wts", bufs=4)
        sbs = pool1(name="sb_stream", bufs=2)
        sbo = pool1(name="sb_once", bufs=1)
        rsp = pool1(name="ropes", bufs=3)
        ps1 = ExitStack()
        psproj = ps1.enter_context(tc.tile_pool(name="ps_proj", bufs=4, space="PSUM"))
        ps_ss = ps1.enter_context(tc.tile_pool(name="ps_ss", bufs=1, space="PSUM"))
        ps_bc = ps1.enter_context(tc.tile_pool(name="ps_bc", bufs=1, space="PSUM"))

        def load_w(dram_ap):
            w = wtp.tile([128, ND * 128], F32R, tag="w", name="w")
            nc.sync.dma_start(
                w[:].rearrange("p (a b) -> p a b", a=ND),
                dram_ap.transpose([1, 0, 2]))
            return w

        xt = []
        wA0 = wtp.tile([128, ND * 128], F32R, tag="w", name="w")
        wA2 = wtp.tile([128, ND * 128], F32R, tag="w", name="w")
        for d_i in range(ND):
            nc.sync.dma_start(wA0[:, d_i * 128:(d_i + 1) * 128], wq_d[0][d_i])
            nc.sync.dma_start(wA2[:, d_i * 128:(d_i + 1) * 128], wq_d[2][d_i])
            x_tile = xtp.tile([128, T], F32R, tag=f"xt{d_i}", name=f"xt{d_i}")
            nc.sync.dma_start(x_tile[:], xT_d[d_i * 128:(d_i + 1) * 128, :])
            xt.append(x_tile)
        wA = [wA0, wA2]

        cos2 = sbo.tile([128, T], dt, tag="cos2", name="cos2")
        sin2 = sbo.tile([128, T], dt, tag="sin2", name="sin2")
        iden = sbo.tile([128, 128], dt, tag="iden", name="iden")
        qsc = pp.tile([128, 2], dt, tag="qsc", name="qsc")
        ksc = pp.tile([128, 2], dt, tag="ksc", name="ksc")
        sel = pp.tile([128, 2], F32R, tag="sel", name="sel")
        bc = pp.tile([2, 128], F32R, tag="bc", name="bc")
        ones = pp.tile([128, 2], F32R, tag="ones", name="ones")
        onesr = pp.tile([1, 128], F32R, tag="onesr", name="onesr")
        biasc = pp.tile([128, 2], dt, tag="biasc", name="biasc")
        for t_, d_ in [(cos2, cos2_d), (sin2, sin2_d), (qsc, qsc_d),
                       (ksc, ksc_d), (sel, sel_d), (bc, bc_d),
                       (ones, ones_d), (onesr, onesr_d), (iden, iden_d),
                       (biasc, biasc_d)]:
            nc.sync.dma_start(t_[:], d_[:])

        qh = [pp.tile([128, T], F32R, tag=f"qh{g}", name=f"qh{g}") for g in range(G)]
        kTn = pp.tile([128, T], F32R, tag="kTn", name="kTn")
        V = pp.tile([128, T], F32R, tag="V", name="V")
        sexp = pp.tile([128, 2 * NS], dt, tag="sexp", name="sexp")
        qkvh = [pp.tile([128, T], F32R, tag=f"qkvh{g}", name=f"qkvh{g}")
                for g in range(G)]

        def project4(w2):
            """d-outer accumulation of 4 psum tiles ([f in w2] x [chunk]) so the
            PE follows the xT stream instead of stalling on the full 8MB load."""
            pss = {(fi, c): psproj.tile([128, CW], dt, tag="proj", name="proj")
                   for fi in range(2) for c in range(NCHUNK)}
            for d_i in range(ND):
                for fi in range(2):
                    for c in range(NCHUNK):
                        nc.tensor.matmul(
                            pss[(fi, c)][:],
                            r(w2[fi][:, d_i * 128:(d_i + 1) * 128]),
                            r(xt[d_i][:, c * CW:(c + 1) * CW]),
                            start=(d_i == 0), stop=(d_i == ND - 1))
            return pss

        def rope(psa, psb, out_a, out_b, sc, cs, np_):
            """psa/psb are SBUF copies; split the 4 multiplies DVE/GpSimd."""
            m1 = sbs.tile([128, CW], dt, tag="m1", name="m1")
            m2 = sbs.tile([128, CW], dt, tag="m2", name="m2")
            cc, ss = cos2[0:np_, cs], sin2[0:np_, cs]
            m1a, m2a = m1[0:np_, :], m2[0:np_, :]
            nc.vector.scalar_tensor_tensor(m1a, psa, sc[:, 0:1], cc, MULT, MULT)
            nc.vector.scalar_tensor_tensor(m2a, psb, sc[:, 1:2], ss, MULT, MULT)
            nc.vector.tensor_sub(out_a, m1a, m2a)
            nc.vector.scalar_tensor_tensor(m1a, psb, sc[:, 1:2], cc, MULT, MULT)
            nc.vector.scalar_tensor_tensor(m2a, psa, sc[:, 0:1], ss, MULT, MULT)
            nc.vector.tensor_add(out_b, m1a, m2a)

        # ---------- q pairs: project (d-outer), rope, per-pair rstd ----------
        for pi, (fa, fb) in enumerate([(0, 2), (1, 3)]):
            w2 = wA if pi == 0 else [load_w(wq_d[1]), load_w(wq_d[3])]
            pss = project4(w2)
            ga, gb = (0, 1) if pi == 0 else (2, 3)
            pcs = {}
            for c in range(NCHUNK):
                pca = sbs.tile([128, CW], dt, tag="pca", name="pca")
                pcb = sbs.tile([128, CW], dt, tag="pcb", name="pcb")
                nc.scalar.copy(pca[:], pss[(0, c)][:])
                nc.vector.tensor_copy(pcb[:], pss[(1, c)][:])
                pcs[c] = (pca, pcb)
            for c in range(NCHUNK):
                cs = slice(c * CW, (c + 1) * CW)
                pca, pcb = pcs[c]
                ssq = ps_ss.tile([2, CW], dt, tag="ss", name="ss")
                for i, pc in enumerate([pca, pcb]):
                    sq = sbs.tile([128, CW], F32R, tag="sq", name="sq")
                    nc.scalar.square(sq[:], pc[:])
                    nc.tensor.matmul(ssq[:], r(sel[:]), r(sq[:]),
                                     start=(i == 0), stop=(i == 1))
                ra = rsp.tile([128, CW], dt, tag="ra", name="ra")
                rb = rsp.tile([128, CW], dt, tag="rb", name="rb")
                rope(pca[:], pcb[:], ra[:], rb[:], qsc, cs, 128)
                stmp = sbs.tile([2, CW], dt, tag="stmp", name="stmp")
                nc.scalar.activation(stmp[:], ssq[:],
                                     mybir.ActivationFunctionType.Sqrt,
                                     bias=biasc[0:2, 1:2], scale=float(1.0 / H))
                rstd = sbs.tile([2, CW], F32R, tag="rstd", name="rstd")
                nc.vector.reciprocal(rstd[:], stmp[:])
                bps = ps_bc.tile([128, CW], dt, tag="bcp", name="bcp")
                nc.tensor.matmul(bps[:], r(bc[:]), r(rstd[:]), start=True, stop=True)
                nc.vector.tensor_mul(qh[ga][0:64, cs], ra[0:64, :], bps[0:64, :])
                nc.vector.tensor_mul(qh[gb][0:64, cs], ra[64:128, :], bps[64:128, :])
                nc.vector.tensor_mul(qh[ga][64:128, cs], rb[0:64, :], bps[0:64, :])
                nc.vector.tensor_mul(qh[gb][64:128, cs], rb[64:128, :], bps[64:128, :])

        # ---------- k and v (one project4 group) ----------
        wkv = [load_w(wk_d[:]), load_w(wv_d[:])]
        pss = project4(wkv)
        sqk = sbo.tile([128, T], F32R, tag="sqk", name="sqk")
        vt_sb = sbo.tile([128, T], dt, tag="vt", name="vt")
        for c in range(NCHUNK):
            cs = slice(c * CW, (c + 1) * CW)
            psk, psv = pss[(0, c)], pss[(1, c)]
            pck = sbs.tile([128, CW], dt, tag="pca", name="pck")
            nc.scalar.copy(pck[:], psk[:])
            nc.scalar.copy(vt_sb[:, cs], psv[:])
            nc.scalar.square(sqk[:, cs], pck[:])
            # base-aligned k rope: second-half inputs all sliced at [64:128]
            # (cos2/sin2 rows 64..127 duplicate rows 0..63 by construction)
            m1 = sbs.tile([128, CW], dt, tag="m1", name="m1")
            m2 = sbs.tile([128, CW], dt, tag="m2", name="m2")
            ca, sa_ = cos2[0:64, cs], sin2[0:64, cs]
            cb, sb_ = cos2[64:128, cs], sin2[64:128, cs]
            k0, k1 = pck[0:64, :], pck[64:128, :]
            sc0, sc1 = ksc[0:64, 0:1], ksc[64:128, 1:2]
            nc.vector.scalar_tensor_tensor(m1[0:64, :], k0, sc0, ca, MULT, MULT)
            nc.vector.scalar_tensor_tensor(m2[0:64, :], k1, sc1, sb_, MULT, MULT)
            nc.vector.tensor_sub(kTn[0:64, cs], m1[0:64, :], m2[0:64, :])
            nc.vector.scalar_tensor_tensor(m1[0:64, :], k1, sc1, cb, MULT, MULT)
            nc.vector.scalar_tensor_tensor(m2[0:64, :], k0, sc0, sa_, MULT, MULT)
            nc.vector.tensor_add(kTn[64:128, cs], m1[0:64, :], m2[0:64, :])
        # SCALE*rstd_k = 1/sqrt(sumsq + H*eps)   (SCALE^2 = 1/H)
        kss = ps_ss.tile([128, 2 * NS], dt, tag="kss", name="kss", bufs=1)
        for j in range(NS):
            nc.tensor.matmul(kss[:, 2 * j:2 * j + 2],
                             r(sqk[:, j * 128:(j + 1) * 128]), r(ones[:]),
                             start=True, stop=True)
        ktmp = sbo.tile([128, 2 * NS], dt, tag="ktmp", name="ktmp")
        nc.scalar.activation(ktmp[:], kss[:], mybir.ActivationFunctionType.Sqrt,
                             bias=biasc[:, 0:1], scale=1.0)
        nc.vector.reciprocal(sexp[:], ktmp[:])
        for j in range(NS):
            vp = ps_bc.tile([128, 128], dt, tag="vtp", name="vtp")
            nc.tensor.transpose(vp[:], vt_sb[:, j * 128:(j + 1) * 128], iden[:])
            nc.scalar.copy(V[:, j * 128:(j + 1) * 128], vp[:])

        es1.close()   # free phase-1 SBUF
        ps1.close()   # free phase-1 PSUM banks

        # ---------- attention + out-projection ----------
        wop = pool(name="wo", bufs=1)
        sbP = pool(name="sbP", bufs=8)
        sbx = pool(name="sbx", bufs=2)
        mkp = pool(name="mks", bufs=1)
        osp = pool(name="outs", bufs=3)
        ps_lg = pool(name="ps_lg", bufs=3, space="PSUM")
        ps_qkv = pool(name="ps_qkv", bufs=2, space="PSUM")
        ps_den = pool(name="ps_den", bufs=1, space="PSUM")
        ps_op = pool(name="ps_op", bufs=2, space="PSUM")

        all_mk = {}
        for c in range(NCHUNK):
            for si, kind, mi in plan[c]:
                if kind == "partial":
                    mk = mkp.tile([128, CW], mybir.dt.bfloat16, tag=f"mk{mi}", name=f"mk{mi}")
                    nc.sync.dma_start(mk[:], msk_d[mi])
                    all_mk[mi] = mk
        wo_sb = []
        for g in range(G):
            w = wop.tile([128, D], F32R, tag=f"wo{g}", name=f"wo{g}")
            nc.sync.dma_start(w[:], wo_d[g])
            wo_sb.append(w)

        for c in range(NCHUNK):
            cs = slice(c * CW, (c + 1) * CW)
            entries = plan[c]
            n_e = len(entries)
            mk_tiles = all_mk
            for g in range(G):
                qkv_ps = ps_qkv.tile([128, CW], dt, tag="qkv", name="qkv")
                den_ps = ps_den.tile([1, CW], dt, tag="den", name="den")
                for ei, (si, kind, mi) in enumerate(entries):
                    lg = ps_lg.tile([128, CW], dt, tag="lg", name="lg")
                    nc.tensor.matmul(
                        lg[:], r(kTn[:, si * 128:(si + 1) * 128]),
                        r(qh[g][:, cs]), start=True, stop=True)
                    P = sbP.tile([128, CW], F32R, tag="P", name="P")
                    nc.scalar.activation(P[:], lg[:],
                                         mybir.ActivationFunctionType.Exp,
                                         scale=sexp[:, 2 * si:2 * si + 1])
                    if kind == "partial":
                        eng = nc.gpsimd if (ei % 3 == 2) else nc.vector
                        eng.tensor_mul(P[:], P[:].bitcast(F32), mk_tiles[mi][:])
                    nc.tensor.matmul(den_ps[:], r(ones[:, 0:1]), r(P[:]),
                                     start=(ei == 0), stop=(ei == n_e - 1))
                    nc.tensor.matmul(qkv_ps[:], r(V[:, si * 128:(si + 1) * 128]),
                                     r(P[:]), start=(ei == 0), stop=(ei == n_e - 1))
                rec = sbx.tile([1, CW], F32R, tag="rec", name="rec")
                nc.vector.reciprocal(rec[:], den_ps[:])
                bcs = sbx.tile([128, CW], dt, tag="bcs", name="bcs")
                nc.gpsimd.partition_broadcast(bcs[:], rec[:].bitcast(F32))
                nc.vector.tensor_mul(qkvh[g][:, cs], qkv_ps[:], bcs[:])

            for tt in range(4):
                t0 = c * CW + tt * 128
                for dc in range(4):
                    op = ps_op.tile([128, CW], dt, tag="op", name="op")
                    for g in range(G):
                        nc.tensor.matmul(
                            op[:], r(qkvh[g][:, t0:t0 + 128]),
                            r(wo_sb[g][:, dc * CW:(dc + 1) * CW]),
                            start=(g == 0), stop=(g == G - 1))
                    ob = osp.tile([128, CW], dt, tag="ob", name="ob")
                    if (tt * 4 + dc) % 2 == 0:
                        nc.vector.tensor_copy(ob[:], op[:])
                    else:
                        nc.scalar.copy(ob[:], op[:])
                    nc.sync.dma_start(
                        out_d[t0:t0 + 128, dc * CW:(dc + 1) * CW], ob[:])

    nc.finalize()
    return nc


_CACHE = {}


def kernel(x, segment_ids, Wq, Wk, Wv, Wo, q_scale, k_scale):
    global LAST_RESULTS
    import os
    import ml_dtypes

    x = np.asarray(x, np.float32)
    seg = np.asarray(segment_ids)
    Wq = np.asarray(Wq, np.float32)
    Wk = np.asarray(Wk, np.float32)
    Wv = np.asarray(Wv, np.float32)
    Wo = np.asarray(Wo, np.float32)
    q_scale = np.asarray(q_scale, np.float32)
    k_scale = np.asarray(k_scale, np.float32)

    plan, masks = _classify([seg[b] for b in range(B)])
    key = repr(plan)
    if key not in _CACHE:
        _CACHE[key] = _build_nc(plan, masks[0].shape[0])
    nc = _CACHE[key]

    half = H // 2
    timescale = ROPE_BASE ** (2.0 * np.arange(half, dtype=np.float32) / H)
    cos2b, sin2b = [], []
    for b in range(B):
        pos = _positions(seg[b])
        sinus = pos[:, None].astype(np.float64) / timescale[None, :]
        sT = np.sin(sinus).T.astype(np.float32)
        cT = np.cos(sinus).T.astype(np.float32)
        cos2b.append(np.ascontiguousarray(np.vstack([cT, cT])))
        sin2b.append(np.ascontiguousarray(np.vstack([sT, sT])))

    p64 = np.arange(128) < 64
    sel = np.zeros((128, 2), np.float32)
    sel[p64, 0] = 1.0
    sel[~p64, 1] = 1.0
    bc = np.zeros((2, 128), np.float32)
    bc[0, 0:64] = 1.0
    bc[1, 64:128] = 1.0
    ones = np.ones((128, 2), np.float32)
    onesr = np.ones((1, 128), np.float32)
    iden = np.eye(128, dtype=np.float32)
    biasc = np.zeros((128, 2), np.float32)
    biasc[:, 0] = H * EPS
    biasc[:, 1] = EPS
    qsc = np.stack([np.tile(q_scale[:64], 2), np.tile(q_scale[64:], 2)], 1)
    qsc = np.ascontiguousarray(qsc, np.float32)
    ksc = np.zeros((128, 2), np.float32)
    ksc[0:64, 0] = k_scale[:64]
    ksc[64:128, 1] = k_scale[64:]

    in_maps = []
    for core in range(8):
        b, kv = core // K, core % K
        qcols = []
        for hv in range(2):
            for g4 in range(G):
                base = kv * 512 + g4 * 128 + hv * 64
                qcols.extend(range(base, base + 64))
        qp = np.array(qcols)
        wq_t = np.ascontiguousarray(
            Wq[:, qp].reshape(ND, 128, G, 128).transpose(2, 0, 1, 3))
        wk_t = np.ascontiguousarray(
            Wk[:, kv * 128:(kv + 1) * 128].reshape(ND, 128, 128))
        wv_t = np.ascontiguousarray(
            Wv[:, kv * 128:(kv + 1) * 128].reshape(ND, 128, 128))
        wo_t = np.ascontiguousarray(Wo[kv * 512:(kv + 1) * 512].reshape(G, 128, D))
        in_maps.append({
            "xT": np.ascontiguousarray(x[b].T),
            "wq": wq_t, "wk": wk_t, "wv": wv_t, "wo": wo_t,
            "cos2": cos2b[b], "sin2": sin2b[b],
            "qsc": qsc, "ksc": ksc, "sel": sel, "bc": bc,
            "ones": ones, "onesr": onesr, "iden": iden, "biasc": biasc,
            "masks": masks[b].astype(ml_dtypes.bfloat16),
        })

    do_trace = os.environ.get("BASS_TRACE") == "1"
    res = run_bass_kernel_spmd(
        nc, in_maps, core_ids=list(range(8)), trace=do_trace)
    LAST_RESULTS = res

    out = np.zeros((B, T, D), np.float32)
    for core in range(8):
        out[core // K] += res.results[core]["out"]
    return out



# revision 7
# speedup vs baseline: 1.4307x; 1.4307x over previous
"""Trainium2 Bass kernel v2 for segment-causal GQA attention.

Sharding: 8 cores = batch (2) x kv-head (4); host sums the 4 row-parallel
Wo partial outputs per batch.  All device compute in fp16 (1 PE cycle/row
at any moving width, 2-byte DVE fast modes, half the DMA bytes of fp32).

Layout per core (T=1024, D=2048, H=128, G=4 q-heads):
  xt      [128, 16*1024]  x[b]^T d-tiles side by side (4 DMA'd groups)
  qh[g]   [128, T]   rope'd, rstd-scaled q per head (transposed)
  kTn     [128, T]   rope'd k, with SCALE*rstd_k folded in per-column
  V       [128, 8*128]  v in [s,h] layout per 128-s-block (direct proj)
  attention: per 128-wide t-block tb, the <=4 valid s-blocks' logits are
  packed into one PSUM bank [128, nv*128]; one exp (bias=-4 keeps P in
  fp16 range without max-subtraction), one packed mask multiply, per-
  block qkv/den accumulation; the out-projection of each tb (4x4
  matmuls into [128t, 512d] psums) interleaves with the next tb's
  softmax work to keep the PE saturated.
"""

import sys

sys.path.insert(0, "/opt/trn_rl_repo")

import numpy as np

import concourse.bacc as bacc
import concourse.bass as bass  # noqa: F401
import concourse.tile as tile
from concourse import mybir
from concourse.bass_utils import run_bass_kernel_spmd

B, T, D = 2, 1024, 2048
N, K, H = 16, 4, 128
G = N // K
EPS = 1e-6
SCALE = H ** -0.5
ROPE_BASE = 10000.0
NCHUNK = 2
CW = T // NCHUNK        # 512
NTB = T // 128          # 8 t-blocks (and s-blocks)
ND = D // 128           # 16
F32 = mybir.dt.float32
F16 = mybir.dt.float16
MULT = mybir.AluOpType.mult
EXPB = -4.0             # exp bias: keeps P in fp16 range without max-sub

LAST_RESULTS = None


def _positions(seg):
    t = seg.shape[0]
    idx = np.arange(t, dtype=np.int64)
    is_start = np.concatenate([[True], seg[1:] != seg[:-1]])
    seg_start = np.maximum.accumulate(np.where(is_start, idx, 0))
    return (idx - seg_start).astype(np.float64)


def _classify(seg_rows):
    """Union-over-batches 128x128 block plan.

    Returns (plan, masks): plan[tb] = list of valid s-block indices;
    masks[b] = fp16 [128, n_blocks*128] 0/1 pack in plan order.
    """
    idx = np.arange(T)
    valids = []
    for b in range(B):
        seg = seg_rows[b]
        valids.append((seg[:, None] == seg[None, :]) & (idx[:, None] <= idx[None, :]))
    plan = []
    packs = [[] for _ in range(B)]
    for tb in range(NTB):
        t0 = tb * 128
        ent = []
        for si in range(NTB):
            s0 = si * 128
            subs = [v[s0:s0 + 128, t0:t0 + 128] for v in valids]
            if any(s.any() for s in subs):
                ent.append(si)
                for b in range(B):
                    packs[b].append(subs[b])
        plan.append(ent)
    masks = []
    for b in range(B):
        m = np.concatenate(packs[b], axis=1) if packs[b] else np.zeros((128, 128), bool)
        masks.append(np.ascontiguousarray(m.astype(np.float16)))
    return plan, masks


def _build_nc(plan, n_mask_cols):
    from contextlib import ExitStack

    nc = bacc.Bacc(None, target_bir_lowering=False, debug=False)
    xT_d = nc.dram_tensor("xT", [ND, 128, T], F16, kind="ExternalInput")
    wq_d = nc.dram_tensor("wq", [G, 128, ND * 128], F16, kind="ExternalInput")
    wk_d = nc.dram_tensor("wk", [128, ND * 128], F16, kind="ExternalInput")
    wv_d = nc.dram_tensor("wv", [128, ND * 128], F16, kind="ExternalInput")
    wo_d = nc.dram_tensor("wo", [G, 128, D], F16, kind="ExternalInput")
    # prescaled rope tables: cos/sin x per-partition rms-scale columns
    cqa_d = nc.dram_tensor("cqa", [128, T], F16, kind="ExternalInput")
    sqa_d = nc.dram_tensor("sqa", [128, T], F16, kind="ExternalInput")
    cqb_d = nc.dram_tensor("cqb", [128, T], F16, kind="ExternalInput")
    sqb_d = nc.dram_tensor("sqb", [128, T], F16, kind="ExternalInput")
    ckt_d = nc.dram_tensor("ckt", [128, T], F16, kind="ExternalInput")
    skt_d = nc.dram_tensor("skt", [128, T], F16, kind="ExternalInput")
    tblf_d = nc.dram_tensor("tblf", [128, 7], F32, kind="ExternalInput")
    tblh_d = nc.dram_tensor("tblh", [128, 194], F16, kind="ExternalInput")
    msk_d = nc.dram_tensor("masks", [128, n_mask_cols], F16, kind="ExternalInput")
    out_d = nc.dram_tensor("out", [T, D], F16, kind="ExternalOutput")

    es = ExitStack()
    with es:
        es.enter_context(nc.allow_low_precision("fp16 kernel"))
        tc = es.enter_context(tile.TileContext(nc))
        pool = lambda *a, **k: es.enter_context(tc.tile_pool(*a, **k))
        pp = pool(name="persist", bufs=1)

        # ---------------- persistent tiles ----------------
        xt = pp.tile([128, ND * T], F16, tag="xt", name="xt")  # 4MB
        qh = [pp.tile([128, T], F16, tag=f"qh{g}", name=f"qh{g}") for g in range(G)]
        kTn = pp.tile([128, T], F16, tag="kTn", name="kTn")
        V = pp.tile([128, NTB * 128], F16, tag="V", name="V")
        wqs = [pp.tile([128, ND * 128], F16, tag=f"wq{g}", name=f"wq{g}")
               for g in range(G)]
        wk_sb = pp.tile([128, ND * 128], F16, tag="wk", name="wk")
        wv_sb = pp.tile([128, ND * 128], F16, tag="wv", name="wv")
        wo_sb = [pp.tile([128, D], F16, tag=f"wo{g}", name=f"wo{g}")
                 for g in range(G)]
        cqa = pp.tile([128, T], F16, tag="cqa", name="cqa")
        sqa = pp.tile([128, T], F16, tag="sqa", name="sqa")
        cqb = pp.tile([128, T], F16, tag="cqb", name="cqb")
        sqb = pp.tile([128, T], F16, tag="sqb", name="sqb")
        ckt = pp.tile([128, T], F16, tag="ckt", name="ckt")
        skt = pp.tile([128, T], F16, tag="skt", name="skt")
        tblf = pp.tile([128, 7], F32, tag="tblf", name="tblf")
        tblh = pp.tile([128, 194], F16, tag="tblh", name="tblh")
        msk = pp.tile([128, n_mask_cols], F16, tag="msk", name="msk")

        qsc = tblf[:, 0:2]       # f32 per-partition scalars
        ksc = tblf[:, 2:4]
        biasc = tblf[:, 4:6]     # [:,0]=H*EPS  [:,1]=EPS
        expb = tblf[:, 6:7]      # exp bias column (EXPB)
        sel65 = tblh[:, 0:65]    # half-selector cols at 0 and 64
        ones1 = tblh[:, 65:66]
        iden = tblh[:, 66:194]   # fp16 identity

        # ---------------- DMA issue (consume order) ----------------
        def xt_ap(d):
            return xt[:, d * T:(d + 1) * T]

        xtv = xt[:].rearrange("p (a t) -> p a t", a=ND)
        # startup splits: first 2 d-tiles of x and first 2 d-cols of wqA
        nc.sync.dma_start(wqs[0][:, 0:256], wq_d[0][:, 0:256])
        nc.sync.dma_start(wqs[2][:, 0:256], wq_d[2][:, 0:256])
        nc.sync.dma_start(xtv[:, 0:2, :], xT_d[0:2].transpose([1, 0, 2]))
        nc.sync.dma_start(tblf[:], tblf_d[:])
        nc.sync.dma_start(tblh[:], tblh_d[:])
        nc.sync.dma_start(wqs[0][:, 256:2048], wq_d[0][:, 256:2048])
        nc.sync.dma_start(wqs[2][:, 256:2048], wq_d[2][:, 256:2048])
        nc.sync.dma_start(xtv[:, 2:4, :], xT_d[2:4].transpose([1, 0, 2]))
        nc.sync.dma_start(xtv[:, 4:8, :], xT_d[4:8].transpose([1, 0, 2]))
        nc.sync.dma_start(xtv[:, 8:12, :], xT_d[8:12].transpose([1, 0, 2]))
        nc.sync.dma_start(xtv[:, 12:16, :], xT_d[12:16].transpose([1, 0, 2]))
        nc.sync.dma_start(wqs[1][:], wq_d[1])
        nc.sync.dma_start(wqs[3][:], wq_d[3])
        nc.sync.dma_start(cqa[:], cqa_d[:])
        nc.sync.dma_start(sqa[:], sqa_d[:])
        nc.sync.dma_start(cqb[:], cqb_d[:])
        nc.sync.dma_start(sqb[:], sqb_d[:])
        nc.sync.dma_start(wk_sb[:], wk_d[:])
        nc.sync.dma_start(ckt[:], ckt_d[:])
        nc.sync.dma_start(skt[:], skt_d[:])
        nc.sync.dma_start(wv_sb[:], wv_d[:])
        nc.sync.dma_start(msk[:], msk_d[:])
        for g in range(G):
            nc.sync.dma_start(wo_sb[g][:], wo_d[g])

        # ---------------- phase-1 pools ----------------
        es1 = ExitStack()
        pool1 = lambda *a, **k: es1.enter_context(tc.tile_pool(*a, **k))
        sbs = pool1(name="sb_stream", bufs=3)
        rsp = pool1(name="ropes", bufs=3)
        vtp = pool1(name="vtp", bufs=1)
        ps1 = ExitStack()
        psproj = ps1.enter_context(tc.tile_pool(name="ps_proj", bufs=4, space="PSUM"))
        ps_ss = ps1.enter_context(tc.tile_pool(name="ps_ss", bufs=1, space="PSUM"))
        ps_v = ps1.enter_context(tc.tile_pool(name="ps_v", bufs=2, space="PSUM"))

        def project4(wa, wb):
            """d-outer accumulation: psums[(fi, c)] = [128, CW] f32."""
            pss = {(fi, c): psproj.tile([128, CW], F32, tag="proj", name="proj")
                   for fi in range(2) for c in range(NCHUNK)}
            for d_i in range(ND):
                for fi, w in enumerate((wa, wb)):
                    for c in range(NCHUNK):
                        nc.tensor.matmul(
                            pss[(fi, c)][:],
                            w[:, d_i * 128:(d_i + 1) * 128],
                            xt_ap(d_i)[:, c * CW:(c + 1) * CW],
                            start=(d_i == 0), stop=(d_i == ND - 1))
            return pss

        def rope(psa, psb, out_a, out_b, cs):
            m1 = sbs.tile([128, CW], F16, tag="m1", name="m1")
            m2 = sbs.tile([128, CW], F16, tag="m2", name="m2")
            nc.vector.tensor_mul(m1[:], psa, cqa[:, cs])
            nc.vector.tensor_mul(m2[:], psb, sqb[:, cs])
            nc.vector.tensor_sub(out_a, m1[:], m2[:])
            nc.vector.tensor_mul(m1[:], psb, cqb[:, cs])
            nc.vector.tensor_mul(m2[:], psa, sqa[:, cs])
            nc.vector.tensor_add(out_b, m1[:], m2[:])

        # ---------------- q pairs ----------------
        for pi in range(2):
            wa, wb = (wqs[0], wqs[2]) if pi == 0 else (wqs[1], wqs[3])
            ga, gb = (0, 1) if pi == 0 else (2, 3)
            pss = project4(wa, wb)
            pcs = {}
            for c in range(NCHUNK):
                pca = sbs.tile([128, CW], F16, tag="pca", name="pca")
                pcb = sbs.tile([128, CW], F16, tag="pcb", name="pcb")
                nc.scalar.copy(pca[:], pss[(0, c)][:])
                nc.vector.tensor_copy(pcb[:], pss[(1, c)][:])
                pcs[c] = (pca, pcb)
            for c in range(NCHUNK):
                cs = slice(c * CW, (c + 1) * CW)
                pca, pcb = pcs[c]
                ssq = ps_ss.tile([65, CW], F32, tag="ss", name="ss")
                for i, pc in enumerate([pca, pcb]):
                    sq = sbs.tile([128, CW], F16, tag="sq", name="sq")
                    nc.vector.tensor_mul(sq[:], pc[:], pc[:])
                    nc.tensor.matmul(ssq[:], sel65, sq[:], start=(i == 0), stop=(i == 1))
                ra = rsp.tile([128, CW], F16, tag="ra", name="ra")
                rb = rsp.tile([128, CW], F16, tag="rb", name="rb")
                rope(pca[:], pcb[:], ra[:], rb[:], cs)
                stmp0 = sbs.tile([1, CW], F32, tag="stmp0", name="stmp0")
                stmp1 = sbs.tile([1, CW], F32, tag="stmp1", name="stmp1")
                nc.scalar.activation(stmp0[:], ssq[0:1, :],
                                     mybir.ActivationFunctionType.Sqrt,
                                     bias=biasc[0:1, 1:2], scale=float(1.0 / H))
                nc.scalar.activation(stmp1[:], ssq[64:65, :],
                                     mybir.ActivationFunctionType.Sqrt,
                                     bias=biasc[0:1, 1:2], scale=float(1.0 / H))
                rstd0 = sbs.tile([1, CW], F16, tag="rstd0", name="rstd0")
                rstd1 = sbs.tile([1, CW], F16, tag="rstd1", name="rstd1")
                nc.vector.reciprocal(rstd0[:], stmp0[:])
                nc.vector.reciprocal(rstd1[:], stmp1[:])
                bca = sbs.tile([128, CW], F16, tag="bca", name="bca")
                bcb = sbs.tile([128, CW], F16, tag="bcb", name="bcb")
                nc.gpsimd.partition_broadcast(bca[:], rstd0[:], channels=128)
                nc.gpsimd.partition_broadcast(bcb[:], rstd1[:], channels=128)
                nc.vector.tensor_mul(qh[ga][0:64, cs], ra[0:64, :], bca[0:64, :])
                nc.vector.tensor_mul(qh[gb][0:64, cs], ra[64:128, :], bcb[64:128, :])
                nc.vector.tensor_mul(qh[ga][64:128, cs], rb[0:64, :], bca[0:64, :])
                nc.vector.tensor_mul(qh[gb][64:128, cs], rb[64:128, :], bcb[64:128, :])

        # ---------------- k ----------------
        for c in range(NCHUNK):
            cs = slice(c * CW, (c + 1) * CW)
            psk = psproj.tile([128, CW], F32, tag="proj", name="proj")
            for d_i in range(ND):
                nc.tensor.matmul(psk[:], wk_sb[:, d_i * 128:(d_i + 1) * 128],
                                 xt_ap(d_i)[:, cs],
                                 start=(d_i == 0), stop=(d_i == ND - 1))
            pck = sbs.tile([128, CW], F16, tag="pck", name="pck")
            nc.scalar.copy(pck[:], psk[:])
            sqk = sbs.tile([128, CW], F16, tag="sqk", name="sqk")
            nc.vector.tensor_mul(sqk[:], pck[:], pck[:])
            # row-form sumsq -> sexp = 1/sqrt(sumsq + H*eps) = SCALE*rstd_k
            kssr = ps_ss.tile([1, CW], F32, tag="kssr", name="kssr")
            nc.tensor.matmul(kssr[:], ones1, sqk[:], start=True, stop=True)
            ktmp = sbs.tile([1, CW], F32, tag="ktmp", name="ktmp")
            nc.scalar.activation(ktmp[:], kssr[:],
                                 mybir.ActivationFunctionType.Sqrt,
                                 bias=biasc[0:1, 0:1], scale=1.0)
            krst = sbs.tile([1, CW], F16, tag="krst", name="krst")
            nc.vector.reciprocal(krst[:], ktmp[:])
            m1 = sbs.tile([128, CW], F16, tag="m1", name="m1")
            m2 = sbs.tile([128, CW], F16, tag="m2", name="m2")
            k0, k1 = pck[0:64, :], pck[64:128, :]
            nc.vector.tensor_mul(m1[0:64, :], k0, ckt[0:64, cs])
            nc.vector.tensor_mul(m2[0:64, :], k1, skt[64:128, cs])
            nc.vector.tensor_sub(kTn[0:64, cs], m1[0:64, :], m2[0:64, :])
            nc.vector.tensor_mul(m1[0:64, :], k1, ckt[64:128, cs])
            nc.vector.tensor_mul(m2[0:64, :], k0, skt[0:64, cs])
            nc.vector.tensor_add(kTn[64:128, cs], m1[0:64, :], m2[0:64, :])
            # fold SCALE*rstd_k into this chunk of kTn
            kbcc = sbs.tile([128, CW], F16, tag="kbcc", name="kbcc")
            nc.gpsimd.partition_broadcast(kbcc[:], krst[:], channels=128)
            nc.vector.tensor_mul(kTn[:, cs], kTn[:, cs], kbcc[:])

        # ---------------- v: VT projection + PE transposes ----------------
        vt_sb = vtp.tile([128, T], F16, tag="vt", name="vt")
        for c in range(NCHUNK):
            cs = slice(c * CW, (c + 1) * CW)
            psv = psproj.tile([128, CW], F32, tag="proj", name="proj")
            for d_i in range(ND):
                nc.tensor.matmul(psv[:], wv_sb[:, d_i * 128:(d_i + 1) * 128],
                                 xt_ap(d_i)[:, cs],
                                 start=(d_i == 0), stop=(d_i == ND - 1))
            nc.scalar.copy(vt_sb[:, cs], psv[:])
        for j in range(NTB):
            vp = ps_v.tile([128, 128], F16, tag="pv", name="pv")
            nc.tensor.transpose(vp[:], vt_sb[:, j * 128:(j + 1) * 128], iden)
            if j % 2 == 0:
                nc.scalar.copy(V[:, j * 128:(j + 1) * 128], vp[:])
            else:
                nc.vector.tensor_copy(V[:, j * 128:(j + 1) * 128], vp[:])

        es1.close()
        ps1.close()

        # ---------------- attention + out-projection per t-block ----------------
        sbP = pool(name="sbP", bufs=4)
        sbD = pool(name="sbD", bufs=3)
        sbx = pool(name="sbx", bufs=3)
        osp = pool(name="outs", bufs=2)
        ps_lg = pool(name="ps_lg", bufs=4, space="PSUM")
        ps_qkv = pool(name="ps_qkv", bufs=2, space="PSUM")
        ps_op = pool(name="ps_op", bufs=2, space="PSUM")

        moff = []
        off = 0
        for tb in range(NTB):
            moff.append(off)
            off += len(plan[tb]) * 128

        from concourse import bass_isa

        def softmax_head(tb, g, lg, nv):
            """exp + packed mask + Pool denominator for (g, tb)."""
            w = nv * 128
            P = sbP.tile([128, 512], F16, tag="P", name="P")
            nc.scalar.activation(P[:, 0:w], lg[:, 0:w],
                                 mybir.ActivationFunctionType.Exp,
                                 bias=expb[:, 0:1], scale=1.0)
            mk = msk[:, moff[tb]:moff[tb] + w]
            nc.vector.tensor_mul(P[:, 0:w], P[:, 0:w], mk)
            dbc = sbD.tile([128, 512], F16, tag="dbc", name="dbc")
            nc.gpsimd.partition_all_reduce(dbc[:, 0:w], P[:, 0:w], channels=128,
                                           reduce_op=bass_isa.ReduceOp.add)
            if nv == 1:
                dsum = dbc[:, 0:128]
            else:
                acc = sbD.tile([128, 128], F16, tag="dfold", name="dfold")
                nc.vector.tensor_add(acc[:], dbc[:, 0:128], dbc[:, 128:256])
                for bi in range(2, nv):
                    nc.vector.tensor_add(acc[:], acc[:],
                                         dbc[:, bi * 128:(bi + 1) * 128])
                dsum = acc[:]
            rec = sbD.tile([128, 128], F16, tag="recg", name="recg")
            nc.vector.reciprocal(rec[:], dsum)
            return P, rec

        def qkv_mm(g, ent, P, qkv_ps):
            nv = len(ent)
            gs = slice(g * 128, (g + 1) * 128)
            for bi, si in enumerate(ent):
                nc.tensor.matmul(qkv_ps[:, gs], V[:, si * 128:(si + 1) * 128],
                                 P[:, bi * 128:(bi + 1) * 128],
                                 start=(bi == 0), stop=(bi == nv - 1))

        def outproj(tb, qkvh):
            t0 = tb * 128
            ob = osp.tile([128, D], F16, tag="ob", name="ob")
            for dc in range(4):
                op = ps_op.tile([128, CW], F32, tag="op", name="op")
                for g in range(G):
                    nc.tensor.matmul(op[:],
                                     qkvh[:, g * 128:(g + 1) * 128],
                                     wo_sb[g][:, dc * CW:(dc + 1) * CW],
                                     start=(g == 0), stop=(g == G - 1))
                if dc % 2 == 0:
                    nc.vector.tensor_copy(ob[:, dc * CW:(dc + 1) * CW], op[:])
                else:
                    nc.scalar.copy(ob[:, dc * CW:(dc + 1) * CW], op[:])
                if tb == NTB - 1:
                    nc.sync.dma_start(
                        out_d[t0:t0 + 128, dc * CW:(dc + 1) * CW],
                        ob[:, dc * CW:(dc + 1) * CW])
            if tb != NTB - 1:
                nc.sync.dma_start(out_d[t0:t0 + 128, :], ob[:])

        prev = None
        for tb in range(NTB):
            ent = plan[tb]
            nv = len(ent)
            t0 = tb * 128
            qkv_ps = ps_qkv.tile([128, 512], F32, tag="qkv", name="qkv")
            qkvh = sbx.tile([128, 512], F16, tag="qkvh", name="qkvh")

            Ps = {}
            for g in range(G):
                lg = ps_lg.tile([128, 512], F32, tag="lg", name="lg")
                for bi, si in enumerate(ent):
                    nc.tensor.matmul(lg[:, bi * 128:(bi + 1) * 128],
                                     kTn[:, si * 128:(si + 1) * 128],
                                     qh[g][:, t0:t0 + 128],
                                     start=True, stop=True)
                Ps[g] = softmax_head(tb, g, lg, nv)
                if g > 0:
                    P, rec = Ps.pop(g - 1)
                    qkv_mm(g - 1, ent, P, qkv_ps)
                    gs = slice((g - 1) * 128, g * 128)
                    nc.vector.tensor_mul(qkvh[:, gs], qkv_ps[:, gs], rec[:])
            P, rec = Ps.pop(G - 1)
            qkv_mm(G - 1, ent, P, qkv_ps)
            gs = slice((G - 1) * 128, G * 128)
            nc.vector.tensor_mul(qkvh[:, gs], qkv_ps[:, gs], rec[:])

            if prev is not None:
                outproj(prev[0], prev[1])
            prev = (tb, qkvh)
        outproj(prev[0], prev[1])

    nc.finalize()
    return nc


_CACHE = {}


def kernel(x, segment_ids, Wq, Wk, Wv, Wo, q_scale, k_scale):
    global LAST_RESULTS
    import os

    x = np.asarray(x, np.float32)
    seg = np.asarray(segment_ids)
    Wq = np.asarray(Wq, np.float32)
    Wk = np.asarray(Wk, np.float32)
    Wv = np.asarray(Wv, np.float32)
    Wo = np.asarray(Wo, np.float32)
    q_scale = np.asarray(q_scale, np.float32)
    k_scale = np.asarray(k_scale, np.float32)

    plan, masks = _classify([seg[b] for b in range(B)])
    key = repr(plan)
    if key not in _CACHE:
        _CACHE[key] = _build_nc(plan, masks[0].shape[1])
    nc = _CACHE[key]

    half = H // 2
    timescale = ROPE_BASE ** (2.0 * np.arange(half, dtype=np.float64) / H)
    qscA = np.tile(q_scale[:64], 2).astype(np.float64)[:, None]
    qscB = np.tile(q_scale[64:], 2).astype(np.float64)[:, None]
    kvec = k_scale.astype(np.float64)[:, None]
    tabs = []  # per batch: (cqa, sqa, cqb, sqb, ckt, skt)
    for b in range(B):
        pos = _positions(seg[b])
        sinus = pos[:, None] / timescale[None, :]
        sT = np.sin(sinus).T
        cT = np.cos(sinus).T
        c2 = np.vstack([cT, cT])
        s2 = np.vstack([sT, sT])
        tabs.append(tuple(
            np.ascontiguousarray(a, np.float16)
            for a in (c2 * qscA, s2 * qscA, c2 * qscB, s2 * qscB,
                      c2 * kvec, s2 * kvec)))

    tblf = np.zeros((128, 7), np.float32)
    tblf[:, 0] = np.tile(q_scale[:64], 2)
    tblf[:, 1] = np.tile(q_scale[64:], 2)
    tblf[0:64, 2] = k_scale[:64]
    tblf[64:128, 3] = k_scale[64:]
    tblf[:, 4] = H * EPS
    tblf[:, 5] = EPS
    tblf[:, 6] = EXPB
    tblh = np.zeros((128, 194), np.float16)
    tblh[0:64, 0] = 1.0
    tblh[64:128, 64] = 1.0
    tblh[:, 65] = 1.0
    tblh[:, 66:194] = np.eye(128, dtype=np.float16)

    in_maps = []
    for core in range(8):
        b, kv = core // K, core % K
        qcols = []
        for hv in range(2):
            for g4 in range(G):
                base = kv * 512 + g4 * 128 + hv * 64
                qcols.extend(range(base, base + 64))
        qp = np.array(qcols)
        wq_t = np.ascontiguousarray(
            Wq[:, qp].reshape(ND, 128, G, 128).transpose(2, 1, 0, 3)
            .reshape(G, 128, ND * 128), np.float16)
        wk_t = np.ascontiguousarray(
            Wk[:, kv * 128:(kv + 1) * 128].reshape(ND, 128, 128)
            .transpose(1, 0, 2).reshape(128, ND * 128), np.float16)
        wv_t = np.ascontiguousarray(
            Wv[:, kv * 128:(kv + 1) * 128].reshape(ND, 128, 128)
            .transpose(1, 0, 2).reshape(128, ND * 128), np.float16)
        wo_t = np.ascontiguousarray(
            Wo[kv * 512:(kv + 1) * 512].reshape(G, 128, D), np.float16)
        xt_t = np.ascontiguousarray(
            x[b].T.reshape(ND, 128, T), np.float16)
        cqa, sqa, cqb, sqb, ckt, skt = tabs[b]
        in_maps.append({
            "xT": xt_t, "wq": wq_t, "wk": wk_t, "wv": wv_t, "wo": wo_t,
            "cqa": cqa, "sqa": sqa, "cqb": cqb, "sqb": sqb,
            "ckt": ckt, "skt": skt,
            "tblf": tblf, "tblh": tblh, "masks": masks[b],
        })

    do_trace = os.environ.get("BASS_TRACE") == "1"
    res = run_bass_kernel_spmd(
        nc, in_maps, core_ids=list(range(8)), trace=do_trace)
    LAST_RESULTS = res

    out = np.zeros((B, T, D), np.float32)
    for core in range(8):
        out[core // K] += res.results[core]["out"].astype(np.float32)
    return out


# revision 8
# speedup vs baseline: 1.4643x; 1.0235x over previous
"""Trainium2 Bass kernel v2 for segment-causal GQA attention.

Sharding: 8 cores = batch (2) x kv-head (4); host sums the 4 row-parallel
Wo partial outputs per batch.  All device compute in fp16 (1 PE cycle/row
at any moving width, 2-byte DVE fast modes, half the DMA bytes of fp32).

Layout per core (T=1024, D=2048, H=128, G=4 q-heads):
  xt      [128, 16*1024]  x[b]^T d-tiles side by side (4 DMA'd groups)
  qh[g]   [128, T]   rope'd, rstd-scaled q per head (transposed)
  kTn     [128, T]   rope'd k, with SCALE*rstd_k folded in per-column
  V       [128, 8*128]  v in [s,h] layout per 128-s-block (direct proj)
  attention: per 128-wide t-block tb, the <=4 valid s-blocks' logits are
  packed into one PSUM bank [128, nv*128]; one exp (bias=-4 keeps P in
  fp16 range without max-subtraction), one packed mask multiply, per-
  block qkv/den accumulation; the out-projection of each tb (4x4
  matmuls into [128t, 512d] psums) interleaves with the next tb's
  softmax work to keep the PE saturated.
"""

import sys

sys.path.insert(0, "/opt/trn_rl_repo")

import numpy as np

import concourse.bacc as bacc
import concourse.bass as bass  # noqa: F401
import concourse.tile as tile
from concourse import mybir
from concourse.bass_utils import run_bass_kernel_spmd

B, T, D = 2, 1024, 2048
N, K, H = 16, 4, 128
G = N // K
EPS = 1e-6
SCALE = H ** -0.5
ROPE_BASE = 10000.0
NCHUNK = 2
CW = T // NCHUNK        # 512
NTB = T // 128          # 8 t-blocks (and s-blocks)
ND = D // 128           # 16
F32 = mybir.dt.float32
F16 = mybir.dt.float16
MULT = mybir.AluOpType.mult
EXPB = -4.0             # exp bias: keeps P in fp16 range without max-sub

LAST_RESULTS = None


def _positions(seg):
    t = seg.shape[0]
    idx = np.arange(t, dtype=np.int64)
    is_start = np.concatenate([[True], seg[1:] != seg[:-1]])
    seg_start = np.maximum.accumulate(np.where(is_start, idx, 0))
    return (idx - seg_start).astype(np.float64)


def _classify(seg_rows):
    """Union-over-batches 128x128 block plan.

    Returns (plan, masks): plan[tb] = list of valid s-block indices;
    masks[b] = fp16 [128, n_blocks*128] 0/1 pack in plan order.
    """
    idx = np.arange(T)
    valids = []
    for b in range(B):
        seg = seg_rows[b]
        valids.append((seg[:, None] == seg[None, :]) & (idx[:, None] <= idx[None, :]))
    plan = []
    packs = [[] for _ in range(B)]
    for tb in range(NTB):
        t0 = tb * 128
        ent = []
        for si in range(NTB):
            s0 = si * 128
            subs = [v[s0:s0 + 128, t0:t0 + 128] for v in valids]
            if any(s.any() for s in subs):
                ent.append(si)
                for b in range(B):
                    packs[b].append(subs[b])
        plan.append(ent)
    masks = []
    for b in range(B):
        m = np.concatenate(packs[b], axis=1) if packs[b] else np.zeros((128, 128), bool)
        masks.append(np.ascontiguousarray(m.astype(np.float16)))
    return plan, masks


def _build_nc(plan, n_mask_cols):
    from contextlib import ExitStack

    nc = bacc.Bacc(None, target_bir_lowering=False, debug=False)
    xT_d = nc.dram_tensor("xT", [ND, 128, T], F16, kind="ExternalInput")
    wq_d = nc.dram_tensor("wq", [G, 128, ND * 128], F16, kind="ExternalInput")
    wk_d = nc.dram_tensor("wk", [128, ND * 128], F16, kind="ExternalInput")
    wv_d = nc.dram_tensor("wv", [128, ND * 128], F16, kind="ExternalInput")
    wo_d = nc.dram_tensor("wo", [G, 128, D], F16, kind="ExternalInput")
    # prescaled rope tables: cos/sin x per-partition rms-scale columns
    cqa_d = nc.dram_tensor("cqa", [128, T], F16, kind="ExternalInput")
    sqa_d = nc.dram_tensor("sqa", [128, T], F16, kind="ExternalInput")
    cqb_d = nc.dram_tensor("cqb", [128, T], F16, kind="ExternalInput")
    sqb_d = nc.dram_tensor("sqb", [128, T], F16, kind="ExternalInput")
    ckt_d = nc.dram_tensor("ckt", [128, T], F16, kind="ExternalInput")
    skt_d = nc.dram_tensor("skt", [128, T], F16, kind="ExternalInput")
    tblf_d = nc.dram_tensor("tblf", [128, 7], F32, kind="ExternalInput")
    tblh_d = nc.dram_tensor("tblh", [128, 194], F16, kind="ExternalInput")
    msk_d = nc.dram_tensor("masks", [128, n_mask_cols], F16, kind="ExternalInput")
    out_d = nc.dram_tensor("out", [T, D], F16, kind="ExternalOutput")

    es = ExitStack()
    with es:
        es.enter_context(nc.allow_low_precision("fp16 kernel"))
        tc = es.enter_context(tile.TileContext(nc))
        pool = lambda *a, **k: es.enter_context(tc.tile_pool(*a, **k))
        pp = pool(name="persist", bufs=1)

        # ---------------- persistent tiles ----------------
        xt = pp.tile([128, ND * T], F16, tag="xt", name="xt")  # 4MB
        qh = [pp.tile([128, T], F16, tag=f"qh{g}", name=f"qh{g}") for g in range(G)]
        kTn = pp.tile([128, T], F16, tag="kTn", name="kTn")
        V = pp.tile([128, NTB * 128], F16, tag="V", name="V")
        wqs = [pp.tile([128, ND * 128], F16, tag=f"wq{g}", name=f"wq{g}")
               for g in range(G)]
        wk_sb = pp.tile([128, ND * 128], F16, tag="wk", name="wk")
        wv_sb = pp.tile([128, ND * 128], F16, tag="wv", name="wv")
        wo_sb = [pp.tile([128, D], F16, tag=f"wo{g}", name=f"wo{g}")
                 for g in range(G)]
        cqa = pp.tile([128, T], F16, tag="cqa", name="cqa")
        sqa = pp.tile([128, T], F16, tag="sqa", name="sqa")
        cqb = pp.tile([128, T], F16, tag="cqb", name="cqb")
        sqb = pp.tile([128, T], F16, tag="sqb", name="sqb")
        ckt = pp.tile([128, T], F16, tag="ckt", name="ckt")
        skt = pp.tile([128, T], F16, tag="skt", name="skt")
        tblf = pp.tile([128, 7], F32, tag="tblf", name="tblf")
        tblh = pp.tile([128, 194], F16, tag="tblh", name="tblh")
        msk = pp.tile([128, n_mask_cols], F16, tag="msk", name="msk")

        qsc = tblf[:, 0:2]       # f32 per-partition scalars
        ksc = tblf[:, 2:4]
        biasc = tblf[:, 4:6]     # [:,0]=H*EPS  [:,1]=EPS
        expb = tblf[:, 6:7]      # exp bias column (EXPB)
        sel65 = tblh[:, 0:65]    # half-selector cols at 0 and 64
        ones1 = tblh[:, 65:66]
        iden = tblh[:, 66:194]   # fp16 identity

        # ---------------- DMA issue (consume order) ----------------
        def xt_ap(d):
            return xt[:, d * T:(d + 1) * T]

        xtv = xt[:].rearrange("p (a t) -> p a t", a=ND)
        # startup splits: first 2 d-tiles of x and first 2 d-cols of wqA
        nc.sync.dma_start(wqs[0][:, 0:256], wq_d[0][:, 0:256])
        nc.sync.dma_start(wqs[2][:, 0:256], wq_d[2][:, 0:256])
        nc.sync.dma_start(xtv[:, 0:2, :], xT_d[0:2].transpose([1, 0, 2]))
        nc.sync.dma_start(tblf[:], tblf_d[:])
        nc.sync.dma_start(tblh[:], tblh_d[:])
        nc.sync.dma_start(wqs[0][:, 256:2048], wq_d[0][:, 256:2048])
        nc.sync.dma_start(wqs[2][:, 256:2048], wq_d[2][:, 256:2048])
        for i in range(1, 8):
            nc.sync.dma_start(xtv[:, 2 * i:2 * i + 2, :],
                              xT_d[2 * i:2 * i + 2].transpose([1, 0, 2]))
        nc.sync.dma_start(wqs[1][:], wq_d[1])
        nc.sync.dma_start(wqs[3][:], wq_d[3])
        nc.sync.dma_start(cqa[:], cqa_d[:])
        nc.sync.dma_start(sqa[:], sqa_d[:])
        nc.sync.dma_start(cqb[:], cqb_d[:])
        nc.sync.dma_start(sqb[:], sqb_d[:])
        nc.sync.dma_start(wk_sb[:], wk_d[:])
        nc.sync.dma_start(ckt[:], ckt_d[:])
        nc.sync.dma_start(skt[:], skt_d[:])
        nc.sync.dma_start(wv_sb[:], wv_d[:])
        nc.sync.dma_start(msk[:], msk_d[:])
        for g in range(G):
            nc.sync.dma_start(wo_sb[g][:], wo_d[g])

        # ---------------- phase-1 pools ----------------
        es1 = ExitStack()
        pool1 = lambda *a, **k: es1.enter_context(tc.tile_pool(*a, **k))
        sbs = pool1(name="sb_stream", bufs=3)
        rsp = pool1(name="ropes", bufs=3)
        vtp = pool1(name="vtp", bufs=1)
        ps1 = ExitStack()
        psproj = ps1.enter_context(tc.tile_pool(name="ps_proj", bufs=4, space="PSUM"))
        ps_ss = ps1.enter_context(tc.tile_pool(name="ps_ss", bufs=1, space="PSUM"))
        ps_v = ps1.enter_context(tc.tile_pool(name="ps_v", bufs=2, space="PSUM"))

        def project4(wa, wb):
            """d-outer accumulation: psums[(fi, c)] = [128, CW] f32."""
            pss = {(fi, c): psproj.tile([128, CW], F32, tag="proj", name="proj")
                   for fi in range(2) for c in range(NCHUNK)}
            for d_i in range(ND):
                for fi, w in enumerate((wa, wb)):
                    for c in range(NCHUNK):
                        nc.tensor.matmul(
                            pss[(fi, c)][:],
                            w[:, d_i * 128:(d_i + 1) * 128],
                            xt_ap(d_i)[:, c * CW:(c + 1) * CW],
                            start=(d_i == 0), stop=(d_i == ND - 1))
            return pss

        def rope(psa, psb, out_a, out_b, cs):
            m1 = sbs.tile([128, CW], F16, tag="m1", name="m1")
            m2 = sbs.tile([128, CW], F16, tag="m2", name="m2")
            nc.vector.tensor_mul(m1[:], psa, cqa[:, cs])
            nc.vector.tensor_mul(m2[:], psb, sqb[:, cs])
            nc.vector.tensor_sub(out_a, m1[:], m2[:])
            nc.vector.tensor_mul(m1[:], psb, cqb[:, cs])
            nc.vector.tensor_mul(m2[:], psa, sqa[:, cs])
            nc.vector.tensor_add(out_b, m1[:], m2[:])

        # warm the Exp activation table early so the load is off the
        # attention critical path
        warm = sbs.tile([1, 2], F16, tag="warm", name="warm")

        # ---------------- q pairs ----------------
        for pi in range(2):
            wa, wb = (wqs[0], wqs[2]) if pi == 0 else (wqs[1], wqs[3])
            ga, gb = (0, 1) if pi == 0 else (2, 3)
            pss = project4(wa, wb)
            pcs = {}
            for c in range(NCHUNK):
                pca = sbs.tile([128, CW], F16, tag="pca", name="pca")
                pcb = sbs.tile([128, CW], F16, tag="pcb", name="pcb")
                nc.scalar.copy(pca[:], pss[(0, c)][:])
                nc.vector.tensor_copy(pcb[:], pss[(1, c)][:])
                pcs[c] = (pca, pcb)
            if pi == 0:
                nc.scalar.activation(warm[:], tblf[0:1, 0:2],
                                     mybir.ActivationFunctionType.Exp,
                                     bias=expb[0:1, 0:1], scale=1.0)
            for c in range(NCHUNK):
                cs = slice(c * CW, (c + 1) * CW)
                pca, pcb = pcs[c]
                ssq = ps_ss.tile([65, CW], F32, tag="ss", name="ss")
                for i, pc in enumerate([pca, pcb]):
                    sq = sbs.tile([128, CW], F16, tag="sq", name="sq")
                    nc.vector.tensor_mul(sq[:], pc[:], pc[:])
                    nc.tensor.matmul(ssq[:], sel65, sq[:], start=(i == 0), stop=(i == 1))
                ra = rsp.tile([128, CW], F16, tag="ra", name="ra")
                rb = rsp.tile([128, CW], F16, tag="rb", name="rb")
                rope(pca[:], pcb[:], ra[:], rb[:], cs)
                stmp0 = sbs.tile([1, CW], F32, tag="stmp0", name="stmp0")
                stmp1 = sbs.tile([1, CW], F32, tag="stmp1", name="stmp1")
                nc.scalar.activation(stmp0[:], ssq[0:1, :],
                                     mybir.ActivationFunctionType.Sqrt,
                                     bias=biasc[0:1, 1:2], scale=float(1.0 / H))
                nc.scalar.activation(stmp1[:], ssq[64:65, :],
                                     mybir.ActivationFunctionType.Sqrt,
                                     bias=biasc[0:1, 1:2], scale=float(1.0 / H))
                rstd0 = sbs.tile([1, CW], F16, tag="rstd0", name="rstd0")
                rstd1 = sbs.tile([1, CW], F16, tag="rstd1", name="rstd1")
                nc.vector.reciprocal(rstd0[:], stmp0[:])
                nc.vector.reciprocal(rstd1[:], stmp1[:])
                bca = sbs.tile([128, CW], F16, tag="bca", name="bca")
                bcb = sbs.tile([128, CW], F16, tag="bcb", name="bcb")
                nc.gpsimd.partition_broadcast(bca[:], rstd0[:], channels=128)
                nc.gpsimd.partition_broadcast(bcb[:], rstd1[:], channels=128)
                nc.vector.tensor_mul(qh[ga][0:64, cs], ra[0:64, :], bca[0:64, :])
                nc.vector.tensor_mul(qh[gb][0:64, cs], ra[64:128, :], bcb[64:128, :])
                nc.vector.tensor_mul(qh[ga][64:128, cs], rb[0:64, :], bca[0:64, :])
                nc.vector.tensor_mul(qh[gb][64:128, cs], rb[64:128, :], bcb[64:128, :])

        # ---------------- k ----------------
        for c in range(NCHUNK):
            cs = slice(c * CW, (c + 1) * CW)
            psk = psproj.tile([128, CW], F32, tag="proj", name="proj")
            for d_i in range(ND):
                nc.tensor.matmul(psk[:], wk_sb[:, d_i * 128:(d_i + 1) * 128],
                                 xt_ap(d_i)[:, cs],
                                 start=(d_i == 0), stop=(d_i == ND - 1))
            pck = sbs.tile([128, CW], F16, tag="pck", name="pck")
            nc.scalar.copy(pck[:], psk[:])
            sqk = sbs.tile([128, CW], F16, tag="sqk", name="sqk")
            nc.vector.tensor_mul(sqk[:], pck[:], pck[:])
            # row-form sumsq -> sexp = 1/sqrt(sumsq + H*eps) = SCALE*rstd_k
            kssr = ps_ss.tile([1, CW], F32, tag="kssr", name="kssr")
            nc.tensor.matmul(kssr[:], ones1, sqk[:], start=True, stop=True)
            ktmp = sbs.tile([1, CW], F32, tag="ktmp", name="ktmp")
            nc.scalar.activation(ktmp[:], kssr[:],
                                 mybir.ActivationFunctionType.Sqrt,
                                 bias=biasc[0:1, 0:1], scale=1.0)
            krst = sbs.tile([1, CW], F16, tag="krst", name="krst")
            nc.vector.reciprocal(krst[:], ktmp[:])
            m1 = sbs.tile([128, CW], F16, tag="m1", name="m1")
            m2 = sbs.tile([128, CW], F16, tag="m2", name="m2")
            k0, k1 = pck[0:64, :], pck[64:128, :]
            nc.vector.tensor_mul(m1[0:64, :], k0, ckt[0:64, cs])
            nc.vector.tensor_mul(m2[0:64, :], k1, skt[64:128, cs])
            nc.vector.tensor_sub(kTn[0:64, cs], m1[0:64, :], m2[0:64, :])
            nc.vector.tensor_mul(m1[0:64, :], k1, ckt[64:128, cs])
            nc.vector.tensor_mul(m2[0:64, :], k0, skt[0:64, cs])
            nc.vector.tensor_add(kTn[64:128, cs], m1[0:64, :], m2[0:64, :])
            # fold SCALE*rstd_k into this chunk of kTn
            kbcc = sbs.tile([128, CW], F16, tag="kbcc", name="kbcc")
            nc.gpsimd.partition_broadcast(kbcc[:], krst[:], channels=128)
            nc.vector.tensor_mul(kTn[:, cs], kTn[:, cs], kbcc[:])

        # ---------------- v: VT projection + PE transposes ----------------
        vt_sb = vtp.tile([128, T], F16, tag="vt", name="vt")
        for c in range(NCHUNK):
            cs = slice(c * CW, (c + 1) * CW)
            psv = psproj.tile([128, CW], F32, tag="proj", name="proj")
            for d_i in range(ND):
                nc.tensor.matmul(psv[:], wv_sb[:, d_i * 128:(d_i + 1) * 128],
                                 xt_ap(d_i)[:, cs],
                                 start=(d_i == 0), stop=(d_i == ND - 1))
            nc.scalar.copy(vt_sb[:, cs], psv[:])
        for j in range(NTB):
            vp = ps_v.tile([128, 128], F16, tag="pv", name="pv")
            nc.tensor.transpose(vp[:], vt_sb[:, j * 128:(j + 1) * 128], iden)
            if j % 2 == 0:
                nc.scalar.copy(V[:, j * 128:(j + 1) * 128], vp[:])
            else:
                nc.vector.tensor_copy(V[:, j * 128:(j + 1) * 128], vp[:])

        es1.close()
        ps1.close()

        # ---------------- attention + out-projection per t-block ----------------
        sbP = pool(name="sbP", bufs=4)
        sbD = pool(name="sbD", bufs=3)
        sbx = pool(name="sbx", bufs=3)
        osp = pool(name="outs", bufs=2)
        ps_lg = pool(name="ps_lg", bufs=4, space="PSUM")
        ps_qkv = pool(name="ps_qkv", bufs=2, space="PSUM")
        ps_op = pool(name="ps_op", bufs=2, space="PSUM")

        moff = []
        off = 0
        for tb in range(NTB):
            moff.append(off)
            off += len(plan[tb]) * 128

        from concourse import bass_isa

        def softmax_head(tb, g, lg, nv):
            """exp + packed mask + Pool denominator for (g, tb)."""
            w = nv * 128
            P = sbP.tile([128, 512], F16, tag="P", name="P")
            nc.scalar.activation(P[:, 0:w], lg[:, 0:w],
                                 mybir.ActivationFunctionType.Exp,
                                 bias=expb[:, 0:1], scale=1.0)
            mk = msk[:, moff[tb]:moff[tb] + w]
            nc.vector.tensor_mul(P[:, 0:w], P[:, 0:w], mk)
            dbc = sbD.tile([128, 512], F16, tag="dbc", name="dbc")
            nc.gpsimd.partition_all_reduce(dbc[:, 0:w], P[:, 0:w], channels=128,
                                           reduce_op=bass_isa.ReduceOp.add)
            if nv == 1:
                dsum = dbc[:, 0:128]
            else:
                acc = sbD.tile([128, 128], F16, tag="dfold", name="dfold")
                nc.vector.tensor_add(acc[:], dbc[:, 0:128], dbc[:, 128:256])
                for bi in range(2, nv):
                    nc.vector.tensor_add(acc[:], acc[:],
                                         dbc[:, bi * 128:(bi + 1) * 128])
                dsum = acc[:]
            rec = sbD.tile([128, 128], F16, tag="recg", name="recg")
            nc.vector.reciprocal(rec[:], dsum)
            return P, rec

        def qkv_mm(g, ent, P, qkv_ps):
            nv = len(ent)
            gs = slice(g * 128, (g + 1) * 128)
            for bi, si in enumerate(ent):
                nc.tensor.matmul(qkv_ps[:, gs], V[:, si * 128:(si + 1) * 128],
                                 P[:, bi * 128:(bi + 1) * 128],
                                 start=(bi == 0), stop=(bi == nv - 1))

        def outproj_dc(tb, qkvh, ob, dc, flush):
            t0 = tb * 128
            op = ps_op.tile([128, CW], F32, tag="op", name="op")
            for g in range(G):
                nc.tensor.matmul(op[:],
                                 qkvh[:, g * 128:(g + 1) * 128],
                                 wo_sb[g][:, dc * CW:(dc + 1) * CW],
                                 start=(g == 0), stop=(g == G - 1))
            if dc % 2 == 0:
                nc.vector.tensor_copy(ob[:, dc * CW:(dc + 1) * CW], op[:])
            else:
                nc.scalar.copy(ob[:, dc * CW:(dc + 1) * CW], op[:])
            if flush:
                nc.sync.dma_start(
                    out_d[t0:t0 + 128, dc * CW:(dc + 1) * CW],
                    ob[:, dc * CW:(dc + 1) * CW])
            elif dc == 3:
                nc.sync.dma_start(out_d[t0:t0 + 128, :], ob[:])

        tb_order = sorted(range(NTB), key=lambda t: -len(plan[t]))
        prev = None
        for tb in tb_order:
            ent = plan[tb]
            nv = len(ent)
            t0 = tb * 128
            qkv_ps = ps_qkv.tile([128, 512], F32, tag="qkv", name="qkv")
            qkvh = sbx.tile([128, 512], F16, tag="qkvh", name="qkvh")
            ob = osp.tile([128, D], F16, tag="ob", name="ob")

            Ps = {}
            for g in range(G):
                lg = ps_lg.tile([128, 512], F32, tag="lg", name="lg")
                for bi, si in enumerate(ent):
                    nc.tensor.matmul(lg[:, bi * 128:(bi + 1) * 128],
                                     kTn[:, si * 128:(si + 1) * 128],
                                     qh[g][:, t0:t0 + 128],
                                     start=True, stop=True)
                Ps[g] = softmax_head(tb, g, lg, nv)
                if prev is not None:
                    outproj_dc(prev[0], prev[1], prev[2], g, False)
                if g > 0:
                    P, rec = Ps.pop(g - 1)
                    qkv_mm(g - 1, ent, P, qkv_ps)
                    gs = slice((g - 1) * 128, g * 128)
                    nc.vector.tensor_mul(qkvh[:, gs], qkv_ps[:, gs], rec[:])
            P, rec = Ps.pop(G - 1)
            qkv_mm(G - 1, ent, P, qkv_ps)
            gs = slice((G - 1) * 128, G * 128)
            nc.vector.tensor_mul(qkvh[:, gs], qkv_ps[:, gs], rec[:])
            prev = (tb, qkvh, ob)
        for dc in range(4):
            outproj_dc(prev[0], prev[1], prev[2], dc, True)

    nc.finalize()
    return nc


_CACHE = {}


def kernel(x, segment_ids, Wq, Wk, Wv, Wo, q_scale, k_scale):
    global LAST_RESULTS
    import os

    x = np.asarray(x, np.float32)
    seg = np.asarray(segment_ids)
    Wq = np.asarray(Wq, np.float32)
    Wk = np.asarray(Wk, np.float32)
    Wv = np.asarray(Wv, np.float32)
    Wo = np.asarray(Wo, np.float32)
    q_scale = np.asarray(q_scale, np.float32)
    k_scale = np.asarray(k_scale, np.float32)

    plan, masks = _classify([seg[b] for b in range(B)])
    key = repr(plan)
    if key not in _CACHE:
        _CACHE[key] = _build_nc(plan, masks[0].shape[1])
    nc = _CACHE[key]

    half = H // 2
    timescale = ROPE_BASE ** (2.0 * np.arange(half, dtype=np.float64) / H)
    qscA = np.tile(q_scale[:64], 2).astype(np.float64)[:, None]
    qscB = np.tile(q_scale[64:], 2).astype(np.float64)[:, None]
    kvec = k_scale.astype(np.float64)[:, None]
    tabs = []  # per batch: (cqa, sqa, cqb, sqb, ckt, skt)
    for b in range(B):
        pos = _positions(seg[b])
        sinus = pos[:, None] / timescale[None, :]
        sT = np.sin(sinus).T
        cT = np.cos(sinus).T
        c2 = np.vstack([cT, cT])
        s2 = np.vstack([sT, sT])
        tabs.append(tuple(
            np.ascontiguousarray(a, np.float16)
            for a in (c2 * qscA, s2 * qscA, c2 * qscB, s2 * qscB,
                      c2 * kvec, s2 * kvec)))

    tblf = np.zeros((128, 7), np.float32)
    tblf[:, 0] = np.tile(q_scale[:64], 2)
    tblf[:, 1] = np.tile(q_scale[64:], 2)
    tblf[0:64, 2] = k_scale[:64]
    tblf[64:128, 3] = k_scale[64:]
    tblf[:, 4] = H * EPS
    tblf[:, 5] = EPS
    tblf[:, 6] = EXPB
    tblh = np.zeros((128, 194), np.float16)
    tblh[0:64, 0] = 1.0
    tblh[64:128, 64] = 1.0
    tblh[:, 65] = 1.0
    tblh[:, 66:194] = np.eye(128, dtype=np.float16)

    in_maps = []
    for core in range(8):
        b, kv = core // K, core % K
        qcols = []
        for hv in range(2):
            for g4 in range(G):
                base = kv * 512 + g4 * 128 + hv * 64
                qcols.extend(range(base, base + 64))
        qp = np.array(qcols)
        wq_t = np.ascontiguousarray(
            Wq[:, qp].reshape(ND, 128, G, 128).transpose(2, 1, 0, 3)
            .reshape(G, 128, ND * 128), np.float16)
        wk_t = np.ascontiguousarray(
            Wk[:, kv * 128:(kv + 1) * 128].reshape(ND, 128, 128)
            .transpose(1, 0, 2).reshape(128, ND * 128), np.float16)
        wv_t = np.ascontiguousarray(
            Wv[:, kv * 128:(kv + 1) * 128].reshape(ND, 128, 128)
            .transpose(1, 0, 2).reshape(128, ND * 128), np.float16)
        wo_t = np.ascontiguousarray(
            Wo[kv * 512:(kv + 1) * 512].reshape(G, 128, D), np.float16)
        xt_t = np.ascontiguousarray(
            x[b].T.reshape(ND, 128, T), np.float16)
        cqa, sqa, cqb, sqb, ckt, skt = tabs[b]
        in_maps.append({
            "xT": xt_t, "wq": wq_t, "wk": wk_t, "wv": wv_t, "wo": wo_t,
            "cqa": cqa, "sqa": sqa, "cqb": cqb, "sqb": sqb,
            "ckt": ckt, "skt": skt,
            "tblf": tblf, "tblh": tblh, "masks": masks[b],
        })

    do_trace = os.environ.get("BASS_TRACE") == "1"
    res = run_bass_kernel_spmd(
        nc, in_maps, core_ids=list(range(8)), trace=do_trace)
    LAST_RESULTS = res

    out = np.zeros((B, T, D), np.float32)
    for core in range(8):
        out[core // K] += res.results[core]["out"].astype(np.float32)
    return out


# revision 9
# speedup vs baseline: 1.4871x; 1.0156x over previous
"""Trainium2 Bass kernel v2 for segment-causal GQA attention.

Sharding: 8 cores = batch (2) x kv-head (4); host sums the 4 row-parallel
Wo partial outputs per batch.  All device compute in fp16 (1 PE cycle/row
at any moving width, 2-byte DVE fast modes, half the DMA bytes of fp32).

Layout per core (T=1024, D=2048, H=128, G=4 q-heads):
  xt      [128, 16*1024]  x[b]^T d-tiles side by side (4 DMA'd groups)
  qh[g]   [128, T]   rope'd, rstd-scaled q per head (transposed)
  kTn     [128, T]   rope'd k, with SCALE*rstd_k folded in per-column
  V       [128, 8*128]  v in [s,h] layout per 128-s-block (direct proj)
  attention: per 128-wide t-block tb, the <=4 valid s-blocks' logits are
  packed into one PSUM bank [128, nv*128]; one exp (bias=-4 keeps P in
  fp16 range without max-subtraction), one packed mask multiply, per-
  block qkv/den accumulation; the out-projection of each tb (4x4
  matmuls into [128t, 512d] psums) interleaves with the next tb's
  softmax work to keep the PE saturated.
"""

import sys

sys.path.insert(0, "/opt/trn_rl_repo")

import numpy as np

import concourse.bacc as bacc
import concourse.bass as bass  # noqa: F401
import concourse.tile as tile
from concourse import mybir
from concourse.bass_utils import run_bass_kernel_spmd

B, T, D = 2, 1024, 2048
N, K, H = 16, 4, 128
G = N // K
EPS = 1e-6
SCALE = H ** -0.5
ROPE_BASE = 10000.0
NCHUNK = 2
CW = T // NCHUNK        # 512
NTB = T // 128          # 8 t-blocks (and s-blocks)
ND = D // 128           # 16
F32 = mybir.dt.float32
F16 = mybir.dt.float16
MULT = mybir.AluOpType.mult
EXPB = -4.0             # exp bias: keeps P in fp16 range without max-sub

LAST_RESULTS = None


def _positions(seg):
    t = seg.shape[0]
    idx = np.arange(t, dtype=np.int64)
    is_start = np.concatenate([[True], seg[1:] != seg[:-1]])
    seg_start = np.maximum.accumulate(np.where(is_start, idx, 0))
    return (idx - seg_start).astype(np.float64)


def _classify(seg_rows):
    """Union-over-batches 128x128 block plan.

    Returns (plan, masks): plan[tb] = list of valid s-block indices;
    masks[b] = fp16 [128, n_blocks*128] 0/1 pack in plan order.
    """
    idx = np.arange(T)
    valids = []
    for b in range(B):
        seg = seg_rows[b]
        valids.append((seg[:, None] == seg[None, :]) & (idx[:, None] <= idx[None, :]))
    plan = []
    packs = [[] for _ in range(B)]
    for tb in range(NTB):
        t0 = tb * 128
        ent = []
        for si in range(NTB):
            s0 = si * 128
            subs = [v[s0:s0 + 128, t0:t0 + 128] for v in valids]
            if any(s.any() for s in subs):
                ent.append(si)
                for b in range(B):
                    packs[b].append(subs[b])
        plan.append(ent)
    masks = []
    for b in range(B):
        m = np.concatenate(packs[b], axis=1) if packs[b] else np.zeros((128, 128), bool)
        masks.append(np.ascontiguousarray(m.astype(np.float16)))
    return plan, masks


def _build_nc(plan, n_mask_cols):
    from contextlib import ExitStack

    nc = bacc.Bacc(None, target_bir_lowering=False, debug=False)
    xT_d = nc.dram_tensor("xT", [ND, 128, T], F16, kind="ExternalInput")
    wq_d = nc.dram_tensor("wq", [G, 128, ND * 128], F16, kind="ExternalInput")
    wk_d = nc.dram_tensor("wk", [128, ND * 128], F16, kind="ExternalInput")
    wv_d = nc.dram_tensor("wv", [128, ND * 128], F16, kind="ExternalInput")
    wo_d = nc.dram_tensor("wo", [G, 128, D], F16, kind="ExternalInput")
    # prescaled rope tables: cos/sin x per-partition rms-scale columns
    cqa_d = nc.dram_tensor("cqa", [128, T], F16, kind="ExternalInput")
    sqa_d = nc.dram_tensor("sqa", [128, T], F16, kind="ExternalInput")
    cqb_d = nc.dram_tensor("cqb", [128, T], F16, kind="ExternalInput")
    sqb_d = nc.dram_tensor("sqb", [128, T], F16, kind="ExternalInput")
    ckt_d = nc.dram_tensor("ckt", [128, T], F16, kind="ExternalInput")
    skt_d = nc.dram_tensor("skt", [128, T], F16, kind="ExternalInput")
    tblf_d = nc.dram_tensor("tblf", [128, 7], F32, kind="ExternalInput")
    tblh_d = nc.dram_tensor("tblh", [128, 194], F16, kind="ExternalInput")
    msk_d = nc.dram_tensor("masks", [128, n_mask_cols], F16, kind="ExternalInput")
    out_d = nc.dram_tensor("out", [T, D], F16, kind="ExternalOutput")

    es = ExitStack()
    with es:
        es.enter_context(nc.allow_low_precision("fp16 kernel"))
        tc = es.enter_context(tile.TileContext(nc))
        pool = lambda *a, **k: es.enter_context(tc.tile_pool(*a, **k))
        pp = pool(name="persist", bufs=1)

        # ---------------- persistent tiles ----------------
        xt = pp.tile([128, ND * T], F16, tag="xt", name="xt")  # 4MB
        qh = [pp.tile([128, T], F16, tag=f"qh{g}", name=f"qh{g}") for g in range(G)]
        kTn = pp.tile([128, T], F16, tag="kTn", name="kTn")
        V = pp.tile([128, NTB * 128], F16, tag="V", name="V")
        wqs = [pp.tile([128, ND * 128], F16, tag=f"wq{g}", name=f"wq{g}")
               for g in range(G)]
        wk_sb = pp.tile([128, ND * 128], F16, tag="wk", name="wk")
        wv_sb = pp.tile([128, ND * 128], F16, tag="wv", name="wv")
        wo_sb = [pp.tile([128, D], F16, tag=f"wo{g}", name=f"wo{g}")
                 for g in range(G)]
        cqa = pp.tile([128, T], F16, tag="cqa", name="cqa")
        sqa = pp.tile([128, T], F16, tag="sqa", name="sqa")
        cqb = pp.tile([128, T], F16, tag="cqb", name="cqb")
        sqb = pp.tile([128, T], F16, tag="sqb", name="sqb")
        ckt = pp.tile([128, T], F16, tag="ckt", name="ckt")
        skt = pp.tile([128, T], F16, tag="skt", name="skt")
        tblf = pp.tile([128, 7], F32, tag="tblf", name="tblf")
        tblh = pp.tile([128, 194], F16, tag="tblh", name="tblh")
        msk = pp.tile([128, n_mask_cols], F16, tag="msk", name="msk")

        qsc = tblf[:, 0:2]       # f32 per-partition scalars
        ksc = tblf[:, 2:4]
        biasc = tblf[:, 4:6]     # [:,0]=H*EPS  [:,1]=EPS
        expb = tblf[:, 6:7]      # exp bias column (EXPB)
        sel65 = tblh[:, 0:65]    # half-selector cols at 0 and 64
        ones1 = tblh[:, 65:66]
        iden = tblh[:, 66:194]   # fp16 identity

        # ---------------- DMA issue (consume order) ----------------
        def xt_ap(d):
            return xt[:, d * T:(d + 1) * T]

        xtv = xt[:].rearrange("p (a t) -> p a t", a=ND)
        # startup splits: first 2 d-tiles of x and first 2 d-cols of wqA
        nc.sync.dma_start(wqs[0][:, 0:512], wq_d[0][:, 0:512])
        nc.sync.dma_start(wqs[2][:, 0:512], wq_d[2][:, 0:512])
        nc.sync.dma_start(xtv[:, 0:2, :], xT_d[0:2].transpose([1, 0, 2]))
        nc.sync.dma_start(tblf[:], tblf_d[:])
        nc.sync.dma_start(tblh[:], tblh_d[:])
        nc.sync.dma_start(xtv[:, 2:4, :], xT_d[2:4].transpose([1, 0, 2]))
        nc.sync.dma_start(wqs[0][:, 512:2048], wq_d[0][:, 512:2048])
        nc.sync.dma_start(wqs[2][:, 512:2048], wq_d[2][:, 512:2048])
        for i in range(2, 8):
            nc.sync.dma_start(xtv[:, 2 * i:2 * i + 2, :],
                              xT_d[2 * i:2 * i + 2].transpose([1, 0, 2]))
        nc.sync.dma_start(wqs[1][:], wq_d[1])
        nc.sync.dma_start(wqs[3][:], wq_d[3])
        nc.sync.dma_start(cqa[:], cqa_d[:])
        nc.sync.dma_start(sqa[:], sqa_d[:])
        nc.sync.dma_start(cqb[:], cqb_d[:])
        nc.sync.dma_start(sqb[:], sqb_d[:])
        nc.sync.dma_start(wk_sb[:], wk_d[:])
        nc.sync.dma_start(ckt[:], ckt_d[:])
        nc.sync.dma_start(skt[:], skt_d[:])
        nc.sync.dma_start(wv_sb[:], wv_d[:])
        nc.sync.dma_start(msk[:], msk_d[:])
        for g in range(G):
            nc.sync.dma_start(wo_sb[g][:], wo_d[g])

        # ---------------- phase-1 pools ----------------
        es1 = ExitStack()
        pool1 = lambda *a, **k: es1.enter_context(tc.tile_pool(*a, **k))
        sbs = pool1(name="sb_stream", bufs=3)
        rsp = pool1(name="ropes", bufs=3)
        vtp = pool1(name="vtp", bufs=1)
        ps1 = ExitStack()
        psproj = ps1.enter_context(tc.tile_pool(name="ps_proj", bufs=4, space="PSUM"))
        ps_ss = ps1.enter_context(tc.tile_pool(name="ps_ss", bufs=1, space="PSUM"))
        ps_v = ps1.enter_context(tc.tile_pool(name="ps_v", bufs=2, space="PSUM"))

        def project4(wa, wb):
            """d-outer accumulation: psums[(fi, c)] = [128, CW] f32."""
            pss = {(fi, c): psproj.tile([128, CW], F32, tag="proj", name="proj")
                   for fi in range(2) for c in range(NCHUNK)}
            for d_i in range(ND):
                for fi, w in enumerate((wa, wb)):
                    for c in range(NCHUNK):
                        nc.tensor.matmul(
                            pss[(fi, c)][:],
                            w[:, d_i * 128:(d_i + 1) * 128],
                            xt_ap(d_i)[:, c * CW:(c + 1) * CW],
                            start=(d_i == 0), stop=(d_i == ND - 1))
            return pss

        def rope(psa, psb, out_a, out_b, cs):
            m1 = sbs.tile([128, CW], F16, tag="m1", name="m1")
            m2 = sbs.tile([128, CW], F16, tag="m2", name="m2")
            nc.vector.tensor_mul(m1[:], psa, cqa[:, cs])
            nc.vector.tensor_mul(m2[:], psb, sqb[:, cs])
            nc.vector.tensor_sub(out_a, m1[:], m2[:])
            nc.vector.tensor_mul(m1[:], psb, cqb[:, cs])
            nc.vector.tensor_mul(m2[:], psa, sqa[:, cs])
            nc.vector.tensor_add(out_b, m1[:], m2[:])

        # warm the Exp activation table early so the load is off the
        # attention critical path
        warm = sbs.tile([1, 2], F16, tag="warm", name="warm")

        # ---------------- q pairs ----------------
        for pi in range(2):
            wa, wb = (wqs[0], wqs[2]) if pi == 0 else (wqs[1], wqs[3])
            ga, gb = (0, 1) if pi == 0 else (2, 3)
            pss = project4(wa, wb)
            pcs = {}
            for c in range(NCHUNK):
                pca = sbs.tile([128, CW], F16, tag="pca", name="pca")
                pcb = sbs.tile([128, CW], F16, tag="pcb", name="pcb")
                nc.scalar.copy(pca[:], pss[(0, c)][:])
                nc.vector.tensor_copy(pcb[:], pss[(1, c)][:])
                pcs[c] = (pca, pcb)
            for c in range(NCHUNK):
                cs = slice(c * CW, (c + 1) * CW)
                pca, pcb = pcs[c]
                ssq = ps_ss.tile([65, CW], F32, tag="ss", name="ss")
                for i, pc in enumerate([pca, pcb]):
                    sq = sbs.tile([128, CW], F16, tag="sq", name="sq")
                    nc.vector.tensor_mul(sq[:], pc[:], pc[:])
                    nc.tensor.matmul(ssq[:], sel65, sq[:], start=(i == 0), stop=(i == 1))
                ra = rsp.tile([128, CW], F16, tag="ra", name="ra")
                rb = rsp.tile([128, CW], F16, tag="rb", name="rb")
                rope(pca[:], pcb[:], ra[:], rb[:], cs)
                stmp0 = sbs.tile([1, CW], F32, tag="stmp0", name="stmp0")
                stmp1 = sbs.tile([1, CW], F32, tag="stmp1", name="stmp1")
                nc.scalar.activation(stmp0[:], ssq[0:1, :],
                                     mybir.ActivationFunctionType.Sqrt,
                                     bias=biasc[0:1, 1:2], scale=float(1.0 / H))
                nc.scalar.activation(stmp1[:], ssq[64:65, :],
                                     mybir.ActivationFunctionType.Sqrt,
                                     bias=biasc[0:1, 1:2], scale=float(1.0 / H))
                rstd0 = sbs.tile([1, CW], F16, tag="rstd0", name="rstd0")
                rstd1 = sbs.tile([1, CW], F16, tag="rstd1", name="rstd1")
                nc.vector.reciprocal(rstd0[:], stmp0[:])
                nc.vector.reciprocal(rstd1[:], stmp1[:])
                bca = sbs.tile([128, CW], F16, tag="bca", name="bca")
                bcb = sbs.tile([128, CW], F16, tag="bcb", name="bcb")
                nc.gpsimd.partition_broadcast(bca[:], rstd0[:], channels=128)
                nc.gpsimd.partition_broadcast(bcb[:], rstd1[:], channels=128)
                nc.vector.tensor_mul(qh[ga][0:64, cs], ra[0:64, :], bca[0:64, :])
                nc.vector.tensor_mul(qh[gb][0:64, cs], ra[64:128, :], bcb[64:128, :])
                nc.vector.tensor_mul(qh[ga][64:128, cs], rb[0:64, :], bca[0:64, :])
                nc.vector.tensor_mul(qh[gb][64:128, cs], rb[64:128, :], bcb[64:128, :])

        # ---------------- k ----------------
        for c in range(NCHUNK):
            cs = slice(c * CW, (c + 1) * CW)
            psk = psproj.tile([128, CW], F32, tag="proj", name="proj")
            for d_i in range(ND):
                nc.tensor.matmul(psk[:], wk_sb[:, d_i * 128:(d_i + 1) * 128],
                                 xt_ap(d_i)[:, cs],
                                 start=(d_i == 0), stop=(d_i == ND - 1))
            pck = sbs.tile([128, CW], F16, tag="pck", name="pck")
            nc.scalar.copy(pck[:], psk[:])
            sqk = sbs.tile([128, CW], F16, tag="sqk", name="sqk")
            nc.vector.tensor_mul(sqk[:], pck[:], pck[:])
            # row-form sumsq -> sexp = 1/sqrt(sumsq + H*eps) = SCALE*rstd_k
            kssr = ps_ss.tile([1, CW], F32, tag="kssr", name="kssr")
            nc.tensor.matmul(kssr[:], ones1, sqk[:], start=True, stop=True)
            ktmp = sbs.tile([1, CW], F32, tag="ktmp", name="ktmp")
            nc.scalar.activation(ktmp[:], kssr[:],
                                 mybir.ActivationFunctionType.Sqrt,
                                 bias=biasc[0:1, 0:1], scale=1.0)
            krst = sbs.tile([1, CW], F16, tag="krst", name="krst")
            nc.vector.reciprocal(krst[:], ktmp[:])
            last_ktmp = ktmp
            m1 = sbs.tile([128, CW], F16, tag="m1", name="m1")
            m2 = sbs.tile([128, CW], F16, tag="m2", name="m2")
            k0, k1 = pck[0:64, :], pck[64:128, :]
            nc.vector.tensor_mul(m1[0:64, :], k0, ckt[0:64, cs])
            nc.vector.tensor_mul(m2[0:64, :], k1, skt[64:128, cs])
            nc.vector.tensor_sub(kTn[0:64, cs], m1[0:64, :], m2[0:64, :])
            nc.vector.tensor_mul(m1[0:64, :], k1, ckt[64:128, cs])
            nc.vector.tensor_mul(m2[0:64, :], k0, skt[0:64, cs])
            nc.vector.tensor_add(kTn[64:128, cs], m1[0:64, :], m2[0:64, :])
            # fold SCALE*rstd_k into this chunk of kTn
            kbcc = sbs.tile([128, CW], F16, tag="kbcc", name="kbcc")
            nc.gpsimd.partition_broadcast(kbcc[:], krst[:], channels=128)
            nc.vector.tensor_mul(kTn[:, cs], kTn[:, cs], kbcc[:])

        # switch the Act table to the exp set now, off the attention
        # critical path; reading the last sqrt output forces the scheduler
        # to place this after every sqrt-set activation
        nc.scalar.activation(warm[:], last_ktmp[0:1, 0:2],
                             mybir.ActivationFunctionType.Exp,
                             bias=expb[0:1, 0:1], scale=-1.0)

        # ---------------- v: VT projection + PE transposes ----------------
        vts = {}
        for c in range(NCHUNK):
            cs = slice(c * CW, (c + 1) * CW)
            psv = psproj.tile([128, CW], F32, tag="proj", name="proj")
            for d_i in range(ND):
                nc.tensor.matmul(psv[:], wv_sb[:, d_i * 128:(d_i + 1) * 128],
                                 xt_ap(d_i)[:, cs],
                                 start=(d_i == 0), stop=(d_i == ND - 1))
            vt_c = vtp.tile([128, CW], F16, tag=f"vt{c}", name=f"vt{c}")
            nc.scalar.copy(vt_c[:], psv[:])
            vts[c] = vt_c
            for jj in range(4):
                j = 4 * c + jj
                vp = ps_v.tile([128, 128], F16, tag="pv", name="pv")
                nc.tensor.transpose(vp[:], vt_c[:, jj * 128:(jj + 1) * 128], iden)
                if j % 2 == 0:
                    nc.scalar.copy(V[:, j * 128:(j + 1) * 128], vp[:])
                else:
                    nc.vector.tensor_copy(V[:, j * 128:(j + 1) * 128], vp[:])

        es1.close()
        ps1.close()

        # ---------------- attention + out-projection per t-block ----------------
        sbP = pool(name="sbP", bufs=5)
        sbD = pool(name="sbD", bufs=4)
        sbx = pool(name="sbx", bufs=3)
        osp = pool(name="outs", bufs=2)
        ps_lg = pool(name="ps_lg", bufs=4, space="PSUM")
        ps_qkv = pool(name="ps_qkv", bufs=2, space="PSUM")
        ps_op = pool(name="ps_op", bufs=2, space="PSUM")

        moff = []
        off = 0
        for tb in range(NTB):
            moff.append(off)
            off += len(plan[tb]) * 128

        from concourse import bass_isa

        def softmax_head(tb, g, lg, nv):
            """exp + packed mask + Pool denominator for (g, tb)."""
            w = nv * 128
            P = sbP.tile([128, 512], F16, tag="P", name="P")
            nc.scalar.activation(P[:, 0:w], lg[:, 0:w],
                                 mybir.ActivationFunctionType.Exp,
                                 bias=expb[:, 0:1], scale=1.0)
            mk = msk[:, moff[tb]:moff[tb] + w]
            nc.vector.tensor_mul(P[:, 0:w], P[:, 0:w], mk)
            dbc = sbD.tile([128, 512], F16, tag="dbc", name="dbc")
            nc.gpsimd.partition_all_reduce(dbc[:, 0:w], P[:, 0:w], channels=128,
                                           reduce_op=bass_isa.ReduceOp.add)
            if nv == 1:
                dsum = dbc[:, 0:128]
            else:
                acc = sbD.tile([128, 128], F16, tag="dfold", name="dfold")
                nc.vector.tensor_add(acc[:], dbc[:, 0:128], dbc[:, 128:256])
                for bi in range(2, nv):
                    nc.vector.tensor_add(acc[:], acc[:],
                                         dbc[:, bi * 128:(bi + 1) * 128])
                dsum = acc[:]
            rec = sbD.tile([128, 128], F16, tag="recg", name="recg")
            nc.vector.reciprocal(rec[:], dsum)
            return P, rec

        def qkv_mm(g, ent, P, qkv_ps):
            nv = len(ent)
            gs = slice(g * 128, (g + 1) * 128)
            for bi, si in enumerate(ent):
                nc.tensor.matmul(qkv_ps[:, gs], V[:, si * 128:(si + 1) * 128],
                                 P[:, bi * 128:(bi + 1) * 128],
                                 start=(bi == 0), stop=(bi == nv - 1))

        def outproj_dc(tb, qkvh, ob, dc, flush):
            t0 = tb * 128
            op = ps_op.tile([128, CW], F32, tag="op", name="op")
            for g in range(G):
                nc.tensor.matmul(op[:],
                                 qkvh[:, g * 128:(g + 1) * 128],
                                 wo_sb[g][:, dc * CW:(dc + 1) * CW],
                                 start=(g == 0), stop=(g == G - 1))
            if dc % 2 == 0:
                nc.vector.tensor_copy(ob[:, dc * CW:(dc + 1) * CW], op[:])
            else:
                nc.scalar.copy(ob[:, dc * CW:(dc + 1) * CW], op[:])
            if flush:
                nc.sync.dma_start(
                    out_d[t0:t0 + 128, dc * CW:(dc + 1) * CW],
                    ob[:, dc * CW:(dc + 1) * CW])
            elif dc == 3:
                nc.sync.dma_start(out_d[t0:t0 + 128, :], ob[:])

        tb_order = sorted(range(NTB), key=lambda t: -len(plan[t]))
        prev = None
        for tb in tb_order:
            ent = plan[tb]
            nv = len(ent)
            t0 = tb * 128
            qkv_ps = ps_qkv.tile([128, 512], F32, tag="qkv", name="qkv")
            qkvh = sbx.tile([128, 512], F16, tag="qkvh", name="qkvh")
            ob = osp.tile([128, D], F16, tag="ob", name="ob")

            Ps = {}

            def flush_qkv(gq):
                P, rec = Ps.pop(gq)
                qkv_mm(gq, ent, P, qkv_ps)
                gs = slice(gq * 128, (gq + 1) * 128)
                nc.vector.tensor_mul(qkvh[:, gs], qkv_ps[:, gs], rec[:])

            for g in range(G):
                lg = ps_lg.tile([128, 512], F32, tag="lg", name="lg")
                for bi, si in enumerate(ent):
                    nc.tensor.matmul(lg[:, bi * 128:(bi + 1) * 128],
                                     kTn[:, si * 128:(si + 1) * 128],
                                     qh[g][:, t0:t0 + 128],
                                     start=True, stop=True)
                Ps[g] = softmax_head(tb, g, lg, nv)
                if prev is not None:
                    outproj_dc(prev[0], prev[1], prev[2], g, False)
                if g > 1:
                    flush_qkv(g - 2)
            flush_qkv(G - 2)
            flush_qkv(G - 1)
            prev = (tb, qkvh, ob)
        for dc in range(4):
            outproj_dc(prev[0], prev[1], prev[2], dc, True)

    nc.finalize()
    return nc


_CACHE = {}


def kernel(x, segment_ids, Wq, Wk, Wv, Wo, q_scale, k_scale):
    global LAST_RESULTS
    import os

    x = np.asarray(x, np.float32)
    seg = np.asarray(segment_ids)
    Wq = np.asarray(Wq, np.float32)
    Wk = np.asarray(Wk, np.float32)
    Wv = np.asarray(Wv, np.float32)
    Wo = np.asarray(Wo, np.float32)
    q_scale = np.asarray(q_scale, np.float32)
    k_scale = np.asarray(k_scale, np.float32)

    plan, masks = _classify([seg[b] for b in range(B)])
    key = repr(plan)
    if key not in _CACHE:
        _CACHE[key] = _build_nc(plan, masks[0].shape[1])
    nc = _CACHE[key]

    half = H // 2
    timescale = ROPE_BASE ** (2.0 * np.arange(half, dtype=np.float64) / H)
    qscA = np.tile(q_scale[:64], 2).astype(np.float64)[:, None]
    qscB = np.tile(q_scale[64:], 2).astype(np.float64)[:, None]
    kvec = k_scale.astype(np.float64)[:, None]
    tabs = []  # per batch: (cqa, sqa, cqb, sqb, ckt, skt)
    for b in range(B):
        pos = _positions(seg[b])
        sinus = pos[:, None] / timescale[None, :]
        sT = np.sin(sinus).T
        cT = np.cos(sinus).T
        c2 = np.vstack([cT, cT])
        s2 = np.vstack([sT, sT])
        tabs.append(tuple(
            np.ascontiguousarray(a, np.float16)
            for a in (c2 * qscA, s2 * qscA, c2 * qscB, s2 * qscB,
                      c2 * kvec, s2 * kvec)))

    tblf = np.zeros((128, 7), np.float32)
    tblf[:, 0] = np.tile(q_scale[:64], 2)
    tblf[:, 1] = np.tile(q_scale[64:], 2)
    tblf[0:64, 2] = k_scale[:64]
    tblf[64:128, 3] = k_scale[64:]
    tblf[:, 4] = H * EPS
    tblf[:, 5] = EPS
    tblf[:, 6] = EXPB
    tblh = np.zeros((128, 194), np.float16)
    tblh[0:64, 0] = 1.0
    tblh[64:128, 64] = 1.0
    tblh[:, 65] = 1.0
    tblh[:, 66:194] = np.eye(128, dtype=np.float16)

    in_maps = []
    for core in range(8):
        b, kv = core // K, core % K
        qcols = []
        for hv in range(2):
            for g4 in range(G):
                base = kv * 512 + g4 * 128 + hv * 64
                qcols.extend(range(base, base + 64))
        qp = np.array(qcols)
        wq_t = np.ascontiguousarray(
            Wq[:, qp].reshape(ND, 128, G, 128).transpose(2, 1, 0, 3)
            .reshape(G, 128, ND * 128), np.float16)
        wk_t = np.ascontiguousarray(
            Wk[:, kv * 128:(kv + 1) * 128].reshape(ND, 128, 128)
            .transpose(1, 0, 2).reshape(128, ND * 128), np.float16)
        wv_t = np.ascontiguousarray(
            Wv[:, kv * 128:(kv + 1) * 128].reshape(ND, 128, 128)
            .transpose(1, 0, 2).reshape(128, ND * 128), np.float16)
        wo_t = np.ascontiguousarray(
            Wo[kv * 512:(kv + 1) * 512].reshape(G, 128, D), np.float16)
        xt_t = np.ascontiguousarray(
            x[b].T.reshape(ND, 128, T), np.float16)
        cqa, sqa, cqb, sqb, ckt, skt = tabs[b]
        in_maps.append({
            "xT": xt_t, "wq": wq_t, "wk": wk_t, "wv": wv_t, "wo": wo_t,
            "cqa": cqa, "sqa": sqa, "cqb": cqb, "sqb": sqb,
            "ckt": ckt, "skt": skt,
            "tblf": tblf, "tblh": tblh, "masks": masks[b],
        })

    do_trace = os.environ.get("BASS_TRACE") == "1"
    res = run_bass_kernel_spmd(
        nc, in_maps, core_ids=list(range(8)), trace=do_trace)
    LAST_RESULTS = res

    out = np.zeros((B, T, D), np.float32)
    for core in range(8):
        out[core // K] += res.results[core]["out"].astype(np.float32)
    return out


# revision 10
# speedup vs baseline: 1.4893x; 1.0015x over previous
"""Trainium2 Bass kernel v2 for segment-causal GQA attention.

Sharding: 8 cores = batch (2) x kv-head (4); host sums the 4 row-parallel
Wo partial outputs per batch.  All device compute in fp16 (1 PE cycle/row
at any moving width, 2-byte DVE fast modes, half the DMA bytes of fp32).

Layout per core (T=1024, D=2048, H=128, G=4 q-heads):
  xt      [128, 16*1024]  x[b]^T d-tiles side by side (4 DMA'd groups)
  qh[g]   [128, T]   rope'd, rstd-scaled q per head (transposed)
  kTn     [128, T]   rope'd k, with SCALE*rstd_k folded in per-column
  V       [128, 8*128]  v in [s,h] layout per 128-s-block (direct proj)
  attention: per 128-wide t-block tb, the <=4 valid s-blocks' logits are
  packed into one PSUM bank [128, nv*128]; one exp (bias=-4 keeps P in
  fp16 range without max-subtraction), one packed mask multiply, per-
  block qkv/den accumulation; the out-projection of each tb (4x4
  matmuls into [128t, 512d] psums) interleaves with the next tb's
  softmax work to keep the PE saturated.
"""

import sys

sys.path.insert(0, "/opt/trn_rl_repo")

import numpy as np

import concourse.bacc as bacc
import concourse.bass as bass  # noqa: F401
import concourse.tile as tile
from concourse import mybir
from concourse.bass_utils import run_bass_kernel_spmd

B, T, D = 2, 1024, 2048
N, K, H = 16, 4, 128
G = N // K
EPS = 1e-6
SCALE = H ** -0.5
ROPE_BASE = 10000.0
NCHUNK = 2
CW = T // NCHUNK        # 512
NTB = T // 128          # 8 t-blocks (and s-blocks)
ND = D // 128           # 16
F32 = mybir.dt.float32
F16 = mybir.dt.float16
MULT = mybir.AluOpType.mult
EXPB = -4.0             # exp bias: keeps P in fp16 range without max-sub

LAST_RESULTS = None


def _positions(seg):
    t = seg.shape[0]
    idx = np.arange(t, dtype=np.int64)
    is_start = np.concatenate([[True], seg[1:] != seg[:-1]])
    seg_start = np.maximum.accumulate(np.where(is_start, idx, 0))
    return (idx - seg_start).astype(np.float64)


def _classify(seg_rows):
    """Union-over-batches 128x128 block plan.

    Returns (plan, masks): plan[tb] = list of valid s-block indices;
    masks[b] = fp16 [128, n_blocks*128] 0/1 pack in plan order.
    """
    idx = np.arange(T)
    valids = []
    for b in range(B):
        seg = seg_rows[b]
        valids.append((seg[:, None] == seg[None, :]) & (idx[:, None] <= idx[None, :]))
    plan = []
    packs = [[] for _ in range(B)]
    for tb in range(NTB):
        t0 = tb * 128
        ent = []
        for si in range(NTB):
            s0 = si * 128
            subs = [v[s0:s0 + 128, t0:t0 + 128] for v in valids]
            if any(s.any() for s in subs):
                ent.append(si)
                for b in range(B):
                    packs[b].append(subs[b])
        plan.append(ent)
    masks = []
    for b in range(B):
        m = np.concatenate(packs[b], axis=1) if packs[b] else np.zeros((128, 128), bool)
        masks.append(np.ascontiguousarray(m.astype(np.float16)))
    return plan, masks


def _build_nc(plan, n_mask_cols):
    from contextlib import ExitStack

    nc = bacc.Bacc(None, target_bir_lowering=False, debug=False)
    xT_d = nc.dram_tensor("xT", [ND, 128, T], F16, kind="ExternalInput")
    wq_d = nc.dram_tensor("wq", [G, 128, ND * 128], F16, kind="ExternalInput")
    wk_d = nc.dram_tensor("wk", [128, ND * 128], F16, kind="ExternalInput")
    wv_d = nc.dram_tensor("wv", [128, ND * 128], F16, kind="ExternalInput")
    wo_d = nc.dram_tensor("wo", [G, 128, D], F16, kind="ExternalInput")
    # prescaled rope tables: cos/sin x per-partition rms-scale columns
    cqa_d = nc.dram_tensor("cqa", [128, T], F16, kind="ExternalInput")
    sqa_d = nc.dram_tensor("sqa", [128, T], F16, kind="ExternalInput")
    cqb_d = nc.dram_tensor("cqb", [128, T], F16, kind="ExternalInput")
    sqb_d = nc.dram_tensor("sqb", [128, T], F16, kind="ExternalInput")
    ckt_d = nc.dram_tensor("ckt", [128, T], F16, kind="ExternalInput")
    skt_d = nc.dram_tensor("skt", [128, T], F16, kind="ExternalInput")
    tblf_d = nc.dram_tensor("tblf", [128, 7], F32, kind="ExternalInput")
    tblh_d = nc.dram_tensor("tblh", [128, 194], F16, kind="ExternalInput")
    msk_d = nc.dram_tensor("masks", [128, n_mask_cols], F16, kind="ExternalInput")
    out_d = nc.dram_tensor("out", [T, D], F16, kind="ExternalOutput")

    es = ExitStack()
    with es:
        es.enter_context(nc.allow_low_precision("fp16 kernel"))
        tc = es.enter_context(tile.TileContext(nc))
        pool = lambda *a, **k: es.enter_context(tc.tile_pool(*a, **k))
        pp = pool(name="persist", bufs=1)

        # ---------------- persistent tiles ----------------
        xt = pp.tile([128, ND * T], F16, tag="xt", name="xt")  # 4MB
        qh = [pp.tile([128, T], F16, tag=f"qh{g}", name=f"qh{g}") for g in range(G)]
        kTn = pp.tile([128, T], F16, tag="kTn", name="kTn")
        V = pp.tile([128, NTB * 128], F16, tag="V", name="V")
        wqs = [pp.tile([128, ND * 128], F16, tag=f"wq{g}", name=f"wq{g}")
               for g in range(G)]
        wk_sb = pp.tile([128, ND * 128], F16, tag="wk", name="wk")
        wv_sb = pp.tile([128, ND * 128], F16, tag="wv", name="wv")
        wo_sb = [pp.tile([128, D], F16, tag=f"wo{g}", name=f"wo{g}")
                 for g in range(G)]
        cqa = pp.tile([128, T], F16, tag="cqa", name="cqa")
        sqa = pp.tile([128, T], F16, tag="sqa", name="sqa")
        cqb = pp.tile([128, T], F16, tag="cqb", name="cqb")
        sqb = pp.tile([128, T], F16, tag="sqb", name="sqb")
        ckt = pp.tile([128, T], F16, tag="ckt", name="ckt")
        skt = pp.tile([128, T], F16, tag="skt", name="skt")
        tblf = pp.tile([128, 7], F32, tag="tblf", name="tblf")
        tblh = pp.tile([128, 194], F16, tag="tblh", name="tblh")
        msk = pp.tile([128, n_mask_cols], F16, tag="msk", name="msk")

        qsc = tblf[:, 0:2]       # f32 per-partition scalars
        ksc = tblf[:, 2:4]
        biasc = tblf[:, 4:6]     # [:,0]=H*EPS  [:,1]=EPS
        expb = tblf[:, 6:7]      # exp bias column (EXPB)
        sel65 = tblh[:, 0:65]    # half-selector cols at 0 and 64
        ones1 = tblh[:, 65:66]
        iden = tblh[:, 66:194]   # fp16 identity

        # ---------------- DMA issue (consume order) ----------------
        def xt_ap(d):
            return xt[:, d * T:(d + 1) * T]

        xtv = xt[:].rearrange("p (a t) -> p a t", a=ND)
        # startup splits: first 2 d-tiles of x and first 2 d-cols of wqA
        nc.sync.dma_start(wqs[0][:, 0:512], wq_d[0][:, 0:512])
        nc.sync.dma_start(wqs[2][:, 0:512], wq_d[2][:, 0:512])
        nc.sync.dma_start(xtv[:, 0:2, :], xT_d[0:2].transpose([1, 0, 2]))
        nc.sync.dma_start(tblf[:], tblf_d[:])
        nc.sync.dma_start(tblh[:], tblh_d[:])
        nc.sync.dma_start(xtv[:, 2:4, :], xT_d[2:4].transpose([1, 0, 2]))
        nc.sync.dma_start(wqs[0][:, 512:2048], wq_d[0][:, 512:2048])
        nc.sync.dma_start(wqs[2][:, 512:2048], wq_d[2][:, 512:2048])
        for i in range(2, 8):
            nc.sync.dma_start(xtv[:, 2 * i:2 * i + 2, :],
                              xT_d[2 * i:2 * i + 2].transpose([1, 0, 2]))
        nc.sync.dma_start(wqs[1][:], wq_d[1])
        nc.sync.dma_start(wqs[3][:], wq_d[3])
        nc.sync.dma_start(cqa[:], cqa_d[:])
        nc.sync.dma_start(sqa[:], sqa_d[:])
        nc.sync.dma_start(cqb[:], cqb_d[:])
        nc.sync.dma_start(sqb[:], sqb_d[:])
        nc.sync.dma_start(wk_sb[:], wk_d[:])
        nc.sync.dma_start(ckt[:], ckt_d[:])
        nc.sync.dma_start(skt[:], skt_d[:])
        nc.sync.dma_start(wv_sb[:], wv_d[:])
        nc.sync.dma_start(msk[:], msk_d[:])
        for g in range(G):
            nc.sync.dma_start(wo_sb[g][:], wo_d[g])

        # ---------------- phase-1 pools ----------------
        es1 = ExitStack()
        pool1 = lambda *a, **k: es1.enter_context(tc.tile_pool(*a, **k))
        sbs = pool1(name="sb_stream", bufs=4)
        rsp = pool1(name="ropes", bufs=3)
        vtp = pool1(name="vtp", bufs=1)
        ps1 = ExitStack()
        psproj = ps1.enter_context(tc.tile_pool(name="ps_proj", bufs=4, space="PSUM"))
        ps_ss = ps1.enter_context(tc.tile_pool(name="ps_ss", bufs=1, space="PSUM"))
        ps_v = ps1.enter_context(tc.tile_pool(name="ps_v", bufs=2, space="PSUM"))

        def project4(wa, wb):
            """d-outer accumulation: psums[(fi, c)] = [128, CW] f32."""
            pss = {(fi, c): psproj.tile([128, CW], F32, tag="proj", name="proj")
                   for fi in range(2) for c in range(NCHUNK)}
            for d_i in range(ND):
                for fi, w in enumerate((wa, wb)):
                    for c in range(NCHUNK):
                        nc.tensor.matmul(
                            pss[(fi, c)][:],
                            w[:, d_i * 128:(d_i + 1) * 128],
                            xt_ap(d_i)[:, c * CW:(c + 1) * CW],
                            start=(d_i == 0), stop=(d_i == ND - 1))
            return pss

        def rope(psa, psb, out_a, out_b, cs):
            m1 = sbs.tile([128, CW], F16, tag="m1", name="m1")
            m2 = sbs.tile([128, CW], F16, tag="m2", name="m2")
            nc.vector.tensor_mul(m1[:], psa, cqa[:, cs])
            nc.vector.tensor_mul(m2[:], psb, sqb[:, cs])
            nc.vector.tensor_sub(out_a, m1[:], m2[:])
            nc.vector.tensor_mul(m1[:], psb, cqb[:, cs])
            nc.vector.tensor_mul(m2[:], psa, sqa[:, cs])
            nc.vector.tensor_add(out_b, m1[:], m2[:])

        # warm the Exp activation table early so the load is off the
        # attention critical path
        warm = sbs.tile([1, 2], F16, tag="warm", name="warm")

        # ---------------- q pairs ----------------
        for pi in range(2):
            wa, wb = (wqs[0], wqs[2]) if pi == 0 else (wqs[1], wqs[3])
            ga, gb = (0, 1) if pi == 0 else (2, 3)
            pss = project4(wa, wb)
            pcs = {}
            for c in range(NCHUNK):
                pca = sbs.tile([128, CW], F16, tag="pca", name="pca")
                pcb = sbs.tile([128, CW], F16, tag="pcb", name="pcb")
                nc.scalar.copy(pca[:], pss[(0, c)][:])
                nc.vector.tensor_copy(pcb[:], pss[(1, c)][:])
                pcs[c] = (pca, pcb)
            for c in range(NCHUNK):
                cs = slice(c * CW, (c + 1) * CW)
                pca, pcb = pcs[c]
                ssq = ps_ss.tile([65, CW], F32, tag="ss", name="ss")
                for i, pc in enumerate([pca, pcb]):
                    sq = sbs.tile([128, CW], F16, tag="sq", name="sq")
                    nc.vector.tensor_mul(sq[:], pc[:], pc[:])
                    nc.tensor.matmul(ssq[:], sel65, sq[:], start=(i == 0), stop=(i == 1))
                ra = rsp.tile([128, CW], F16, tag="ra", name="ra")
                rb = rsp.tile([128, CW], F16, tag="rb", name="rb")
                rope(pca[:], pcb[:], ra[:], rb[:], cs)
                stmp0 = sbs.tile([1, CW], F32, tag="stmp0", name="stmp0")
                stmp1 = sbs.tile([1, CW], F32, tag="stmp1", name="stmp1")
                nc.scalar.activation(stmp0[:], ssq[0:1, :],
                                     mybir.ActivationFunctionType.Sqrt,
                                     bias=biasc[0:1, 1:2], scale=float(1.0 / H))
                nc.scalar.activation(stmp1[:], ssq[64:65, :],
                                     mybir.ActivationFunctionType.Sqrt,
                                     bias=biasc[0:1, 1:2], scale=float(1.0 / H))
                rstd0 = sbs.tile([1, CW], F16, tag="rstd0", name="rstd0")
                rstd1 = sbs.tile([1, CW], F16, tag="rstd1", name="rstd1")
                nc.vector.reciprocal(rstd0[:], stmp0[:])
                nc.vector.reciprocal(rstd1[:], stmp1[:])
                bca = sbs.tile([128, CW], F16, tag="bca", name="bca")
                bcb = sbs.tile([128, CW], F16, tag="bcb", name="bcb")
                nc.gpsimd.partition_broadcast(bca[:], rstd0[:], channels=128)
                nc.gpsimd.partition_broadcast(bcb[:], rstd1[:], channels=128)
                nc.vector.tensor_mul(qh[ga][0:64, cs], ra[0:64, :], bca[0:64, :])
                nc.vector.tensor_mul(qh[gb][0:64, cs], ra[64:128, :], bcb[64:128, :])
                nc.vector.tensor_mul(qh[ga][64:128, cs], rb[0:64, :], bca[0:64, :])
                nc.vector.tensor_mul(qh[gb][64:128, cs], rb[64:128, :], bcb[64:128, :])

        # ---------------- k ----------------
        for c in range(NCHUNK):
            cs = slice(c * CW, (c + 1) * CW)
            psk = psproj.tile([128, CW], F32, tag="proj", name="proj")
            for d_i in range(ND):
                nc.tensor.matmul(psk[:], wk_sb[:, d_i * 128:(d_i + 1) * 128],
                                 xt_ap(d_i)[:, cs],
                                 start=(d_i == 0), stop=(d_i == ND - 1))
            pck = sbs.tile([128, CW], F16, tag="pck", name="pck")
            nc.scalar.copy(pck[:], psk[:])
            sqk = sbs.tile([128, CW], F16, tag="sqk", name="sqk")
            nc.vector.tensor_mul(sqk[:], pck[:], pck[:])
            # row-form sumsq -> sexp = 1/sqrt(sumsq + H*eps) = SCALE*rstd_k
            kssr = ps_ss.tile([1, CW], F32, tag="kssr", name="kssr")
            nc.tensor.matmul(kssr[:], ones1, sqk[:], start=True, stop=True)
            ktmp = sbs.tile([1, CW], F32, tag="ktmp", name="ktmp")
            nc.scalar.activation(ktmp[:], kssr[:],
                                 mybir.ActivationFunctionType.Sqrt,
                                 bias=biasc[0:1, 0:1], scale=1.0)
            krst = sbs.tile([1, CW], F16, tag="krst", name="krst")
            nc.vector.reciprocal(krst[:], ktmp[:])
            last_ktmp = ktmp
            m1 = sbs.tile([128, CW], F16, tag="m1", name="m1")
            m2 = sbs.tile([128, CW], F16, tag="m2", name="m2")
            k0, k1 = pck[0:64, :], pck[64:128, :]
            nc.vector.tensor_mul(m1[0:64, :], k0, ckt[0:64, cs])
            nc.vector.tensor_mul(m2[0:64, :], k1, skt[64:128, cs])
            nc.vector.tensor_sub(kTn[0:64, cs], m1[0:64, :], m2[0:64, :])
            nc.vector.tensor_mul(m1[0:64, :], k1, ckt[64:128, cs])
            nc.vector.tensor_mul(m2[0:64, :], k0, skt[0:64, cs])
            nc.vector.tensor_add(kTn[64:128, cs], m1[0:64, :], m2[0:64, :])
            # fold SCALE*rstd_k into this chunk of kTn
            kbcc = sbs.tile([128, CW], F16, tag="kbcc", name="kbcc")
            nc.gpsimd.partition_broadcast(kbcc[:], krst[:], channels=128)
            nc.vector.tensor_mul(kTn[:, cs], kTn[:, cs], kbcc[:])

        # switch the Act table to the exp set now, off the attention
        # critical path; reading the last sqrt output forces the scheduler
        # to place this after every sqrt-set activation
        nc.scalar.activation(warm[:], last_ktmp[0:1, 0:2],
                             mybir.ActivationFunctionType.Exp,
                             bias=expb[0:1, 0:1], scale=-1.0)

        # ---------------- v: VT projection + PE transposes ----------------
        vts = {}
        for c in range(NCHUNK):
            cs = slice(c * CW, (c + 1) * CW)
            psv = psproj.tile([128, CW], F32, tag="proj", name="proj")
            for d_i in range(ND):
                nc.tensor.matmul(psv[:], wv_sb[:, d_i * 128:(d_i + 1) * 128],
                                 xt_ap(d_i)[:, cs],
                                 start=(d_i == 0), stop=(d_i == ND - 1))
            vt_c = vtp.tile([128, CW], F16, tag=f"vt{c}", name=f"vt{c}")
            nc.scalar.copy(vt_c[:], psv[:])
            vts[c] = vt_c
            for jj in range(4):
                j = 4 * c + jj
                vp = ps_v.tile([128, 128], F16, tag="pv", name="pv")
                nc.tensor.transpose(vp[:], vt_c[:, jj * 128:(jj + 1) * 128], iden)
                if j % 2 == 0:
                    nc.scalar.copy(V[:, j * 128:(j + 1) * 128], vp[:])
                else:
                    nc.vector.tensor_copy(V[:, j * 128:(j + 1) * 128], vp[:])

        es1.close()
        ps1.close()

        # ---------------- attention + out-projection per t-block ----------------
        sbP = pool(name="sbP", bufs=5)
        sbD = pool(name="sbD", bufs=4)
        sbx = pool(name="sbx", bufs=3)
        osp = pool(name="outs", bufs=3)
        ps_lg = pool(name="ps_lg", bufs=4, space="PSUM")
        ps_qkv = pool(name="ps_qkv", bufs=2, space="PSUM")
        ps_op = pool(name="ps_op", bufs=2, space="PSUM")

        moff = []
        off = 0
        for tb in range(NTB):
            moff.append(off)
            off += len(plan[tb]) * 128

        from concourse import bass_isa

        def softmax_head(tb, g, lg, nv):
            """exp + packed mask + Pool denominator for (g, tb)."""
            w = nv * 128
            P = sbP.tile([128, 512], F16, tag="P", name="P")
            nc.scalar.activation(P[:, 0:w], lg[:, 0:w],
                                 mybir.ActivationFunctionType.Exp,
                                 bias=expb[:, 0:1], scale=1.0)
            mk = msk[:, moff[tb]:moff[tb] + w]
            nc.vector.tensor_mul(P[:, 0:w], P[:, 0:w], mk)
            dbc = sbD.tile([128, 512], F16, tag="dbc", name="dbc")
            nc.gpsimd.partition_all_reduce(dbc[:, 0:w], P[:, 0:w], channels=128,
                                           reduce_op=bass_isa.ReduceOp.add)
            if nv == 1:
                dsum = dbc[:, 0:128]
            else:
                acc = sbD.tile([128, 128], F16, tag="dfold", name="dfold")
                nc.vector.tensor_add(acc[:], dbc[:, 0:128], dbc[:, 128:256])
                for bi in range(2, nv):
                    nc.vector.tensor_add(acc[:], acc[:],
                                         dbc[:, bi * 128:(bi + 1) * 128])
                dsum = acc[:]
            rec = sbD.tile([128, 128], F16, tag="recg", name="recg")
            nc.vector.reciprocal(rec[:], dsum)
            return P, rec

        def qkv_mm(g, ent, P, qkv_ps):
            nv = len(ent)
            gs = slice(g * 128, (g + 1) * 128)
            for bi, si in enumerate(ent):
                nc.tensor.matmul(qkv_ps[:, gs], V[:, si * 128:(si + 1) * 128],
                                 P[:, bi * 128:(bi + 1) * 128],
                                 start=(bi == 0), stop=(bi == nv - 1))

        def outproj_dc(tb, qkvh, ob, dc, flush):
            t0 = tb * 128
            op = ps_op.tile([128, CW], F32, tag="op", name="op")
            for g in range(G):
                nc.tensor.matmul(op[:],
                                 qkvh[:, g * 128:(g + 1) * 128],
                                 wo_sb[g][:, dc * CW:(dc + 1) * CW],
                                 start=(g == 0), stop=(g == G - 1))
            if dc % 2 == 0:
                nc.vector.tensor_copy(ob[:, dc * CW:(dc + 1) * CW], op[:])
            else:
                nc.scalar.copy(ob[:, dc * CW:(dc + 1) * CW], op[:])
            if flush:
                nc.sync.dma_start(
                    out_d[t0:t0 + 128, dc * CW:(dc + 1) * CW],
                    ob[:, dc * CW:(dc + 1) * CW])
            elif dc == 3:
                nc.sync.dma_start(out_d[t0:t0 + 128, :], ob[:])

        tb_order = sorted(range(NTB), key=lambda t: -len(plan[t]))
        prev = None
        for tb in tb_order:
            ent = plan[tb]
            nv = len(ent)
            t0 = tb * 128
            qkv_ps = ps_qkv.tile([128, 512], F32, tag="qkv", name="qkv")
            qkvh = sbx.tile([128, 512], F16, tag="qkvh", name="qkvh")
            ob = osp.tile([128, D], F16, tag="ob", name="ob")

            Ps = {}

            def flush_qkv(gq):
                P, rec = Ps.pop(gq)
                qkv_mm(gq, ent, P, qkv_ps)
                gs = slice(gq * 128, (gq + 1) * 128)
                nc.vector.tensor_mul(qkvh[:, gs], qkv_ps[:, gs], rec[:])

            for g in range(G):
                lg = ps_lg.tile([128, 512], F32, tag="lg", name="lg")
                for bi, si in enumerate(ent):
                    nc.tensor.matmul(lg[:, bi * 128:(bi + 1) * 128],
                                     kTn[:, si * 128:(si + 1) * 128],
                                     qh[g][:, t0:t0 + 128],
                                     start=True, stop=True)
                Ps[g] = softmax_head(tb, g, lg, nv)
                if prev is not None:
                    outproj_dc(prev[0], prev[1], prev[2], g, False)
                if g > 1:
                    flush_qkv(g - 2)
            flush_qkv(G - 2)
            flush_qkv(G - 1)
            prev = (tb, qkvh, ob)
        for dc in range(4):
            outproj_dc(prev[0], prev[1], prev[2], dc, True)

    nc.finalize()
    return nc


_CACHE = {}


def kernel(x, segment_ids, Wq, Wk, Wv, Wo, q_scale, k_scale):
    global LAST_RESULTS
    import os

    x = np.asarray(x, np.float32)
    seg = np.asarray(segment_ids)
    Wq = np.asarray(Wq, np.float32)
    Wk = np.asarray(Wk, np.float32)
    Wv = np.asarray(Wv, np.float32)
    Wo = np.asarray(Wo, np.float32)
    q_scale = np.asarray(q_scale, np.float32)
    k_scale = np.asarray(k_scale, np.float32)

    plan, masks = _classify([seg[b] for b in range(B)])
    key = repr(plan)
    if key not in _CACHE:
        _CACHE[key] = _build_nc(plan, masks[0].shape[1])
    nc = _CACHE[key]

    half = H // 2
    timescale = ROPE_BASE ** (2.0 * np.arange(half, dtype=np.float64) / H)
    qscA = np.tile(q_scale[:64], 2).astype(np.float64)[:, None]
    qscB = np.tile(q_scale[64:], 2).astype(np.float64)[:, None]
    kvec = k_scale.astype(np.float64)[:, None]
    tabs = []  # per batch: (cqa, sqa, cqb, sqb, ckt, skt)
    for b in range(B):
        pos = _positions(seg[b])
        sinus = pos[:, None] / timescale[None, :]
        sT = np.sin(sinus).T
        cT = np.cos(sinus).T
        c2 = np.vstack([cT, cT])
        s2 = np.vstack([sT, sT])
        tabs.append(tuple(
            np.ascontiguousarray(a, np.float16)
            for a in (c2 * qscA, s2 * qscA, c2 * qscB, s2 * qscB,
                      c2 * kvec, s2 * kvec)))

    tblf = np.zeros((128, 7), np.float32)
    tblf[:, 0] = np.tile(q_scale[:64], 2)
    tblf[:, 1] = np.tile(q_scale[64:], 2)
    tblf[0:64, 2] = k_scale[:64]
    tblf[64:128, 3] = k_scale[64:]
    tblf[:, 4] = H * EPS
    tblf[:, 5] = EPS
    tblf[:, 6] = EXPB
    tblh = np.zeros((128, 194), np.float16)
    tblh[0:64, 0] = 1.0
    tblh[64:128, 64] = 1.0
    tblh[:, 65] = 1.0
    tblh[:, 66:194] = np.eye(128, dtype=np.float16)

    in_maps = []
    for core in range(8):
        b, kv = core // K, core % K
        qcols = []
        for hv in range(2):
            for g4 in range(G):
                base = kv * 512 + g4 * 128 + hv * 64
                qcols.extend(range(base, base + 64))
        qp = np.array(qcols)
        wq_t = np.ascontiguousarray(
            Wq[:, qp].reshape(ND, 128, G, 128).transpose(2, 1, 0, 3)
            .reshape(G, 128, ND * 128), np.float16)
        wk_t = np.ascontiguousarray(
            Wk[:, kv * 128:(kv + 1) * 128].reshape(ND, 128, 128)
            .transpose(1, 0, 2).reshape(128, ND * 128), np.float16)
        wv_t = np.ascontiguousarray(
            Wv[:, kv * 128:(kv + 1) * 128].reshape(ND, 128, 128)
            .transpose(1, 0, 2).reshape(128, ND * 128), np.float16)
        wo_t = np.ascontiguousarray(
            Wo[kv * 512:(kv + 1) * 512].reshape(G, 128, D), np.float16)
        xt_t = np.ascontiguousarray(
            x[b].T.reshape(ND, 128, T), np.float16)
        cqa, sqa, cqb, sqb, ckt, skt = tabs[b]
        in_maps.append({
            "xT": xt_t, "wq": wq_t, "wk": wk_t, "wv": wv_t, "wo": wo_t,
            "cqa": cqa, "sqa": sqa, "cqb": cqb, "sqb": sqb,
            "ckt": ckt, "skt": skt,
            "tblf": tblf, "tblh": tblh, "masks": masks[b],
        })

    do_trace = os.environ.get("BASS_TRACE") == "1"
    res = run_bass_kernel_spmd(
        nc, in_maps, core_ids=list(range(8)), trace=do_trace)
    LAST_RESULTS = res

    out = np.zeros((B, T, D), np.float32)
    for core in range(8):
        out[core // K] += res.results[core]["out"].astype(np.float32)
    return out


# revision 13
# speedup vs baseline: 1.5094x; 1.0135x over previous
"""Trainium2 Bass kernel v2 for segment-causal GQA attention.

Sharding: 8 cores = batch (2) x kv-head (4); host sums the 4 row-parallel
Wo partial outputs per batch.  All device compute in fp16 (1 PE cycle/row
at any moving width, 2-byte DVE fast modes, half the DMA bytes of fp32).

Layout per core (T=1024, D=2048, H=128, G=4 q-heads):
  xt      [128, 16*1024]  x[b]^T d-tiles side by side (4 DMA'd groups)
  qh[g]   [128, T]   rope'd, rstd-scaled q per head (transposed)
  kTn     [128, T]   rope'd k, with SCALE*rstd_k folded in per-column
  V       [128, 8*128]  v in [s,h] layout per 128-s-block (direct proj)
  attention: per 128-wide t-block tb, the <=4 valid s-blocks' logits are
  packed into one PSUM bank [128, nv*128]; one exp (bias=-4 keeps P in
  fp16 range without max-subtraction), one packed mask multiply, per-
  block qkv/den accumulation; the out-projection of each tb (4x4
  matmuls into [128t, 512d] psums) interleaves with the next tb's
  softmax work to keep the PE saturated.
"""

import sys

sys.path.insert(0, "/opt/trn_rl_repo")

import numpy as np

import concourse.bacc as bacc
import concourse.bass as bass  # noqa: F401
import concourse.tile as tile
from concourse import mybir
from concourse.bass_utils import run_bass_kernel_spmd

B, T, D = 2, 1024, 2048
N, K, H = 16, 4, 128
G = N // K
EPS = 1e-6
SCALE = H ** -0.5
ROPE_BASE = 10000.0
NCHUNK = 2
CW = T // NCHUNK        # 512
NTB = T // 128          # 8 t-blocks (and s-blocks)
ND = D // 128           # 16
F32 = mybir.dt.float32
F16 = mybir.dt.float16
MULT = mybir.AluOpType.mult
EXPB = -4.0             # exp bias: keeps P in fp16 range without max-sub

LAST_RESULTS = None


def _positions(seg):
    t = seg.shape[0]
    idx = np.arange(t, dtype=np.int64)
    is_start = np.concatenate([[True], seg[1:] != seg[:-1]])
    seg_start = np.maximum.accumulate(np.where(is_start, idx, 0))
    return (idx - seg_start).astype(np.float64)


def _classify(seg_rows):
    """Union-over-batches 128x128 block plan.

    Returns (plan, masks): plan[tb] = list of valid s-block indices;
    masks[b] = fp16 [128, n_blocks*128] 0/1 pack in plan order.
    """
    idx = np.arange(T)
    valids = []
    for b in range(B):
        seg = seg_rows[b]
        valids.append((seg[:, None] == seg[None, :]) & (idx[:, None] <= idx[None, :]))
    plan = []
    packs = [[] for _ in range(B)]
    for tb in range(NTB):
        t0 = tb * 128
        ent = []
        for si in range(NTB):
            s0 = si * 128
            subs = [v[s0:s0 + 128, t0:t0 + 128] for v in valids]
            if any(s.any() for s in subs):
                ent.append(si)
                for b in range(B):
                    packs[b].append(subs[b])
        plan.append(ent)
    masks = []
    for b in range(B):
        if packs[b]:
            m = np.concatenate([np.tile(p, (1, 4)) for p in packs[b]], axis=1)
        else:
            m = np.zeros((128, 512), bool)
        masks.append(np.ascontiguousarray(m.astype(np.float16)))
    return plan, masks


def _build_nc(plan, n_mask_cols):
    from contextlib import ExitStack

    nc = bacc.Bacc(None, target_bir_lowering=False, debug=False)
    xT_d = nc.dram_tensor("xT", [ND, 128, T], F16, kind="ExternalInput")
    wq_d = nc.dram_tensor("wq", [G, 128, ND * 128], F16, kind="ExternalInput")
    wk_d = nc.dram_tensor("wk", [128, ND * 128], F16, kind="ExternalInput")
    wv_d = nc.dram_tensor("wv", [128, ND * 128], F16, kind="ExternalInput")
    wo_d = nc.dram_tensor("wo", [G, 128, D], F16, kind="ExternalInput")
    # prescaled rope tables: cos/sin x per-partition rms-scale columns
    cqa_d = nc.dram_tensor("cqa", [128, T], F16, kind="ExternalInput")
    sqa_d = nc.dram_tensor("sqa", [128, T], F16, kind="ExternalInput")
    cqb_d = nc.dram_tensor("cqb", [128, T], F16, kind="ExternalInput")
    sqb_d = nc.dram_tensor("sqb", [128, T], F16, kind="ExternalInput")
    ckt_d = nc.dram_tensor("ckt", [128, T], F16, kind="ExternalInput")
    skt_d = nc.dram_tensor("skt", [128, T], F16, kind="ExternalInput")
    tblf_d = nc.dram_tensor("tblf", [128, 7], F32, kind="ExternalInput")
    tblh_d = nc.dram_tensor("tblh", [128, 194], F16, kind="ExternalInput")
    msk_d = nc.dram_tensor("masks", [128, n_mask_cols], F16, kind="ExternalInput")
    out_d = nc.dram_tensor("out", [T, D], F16, kind="ExternalOutput")

    es = ExitStack()
    with es:
        es.enter_context(nc.allow_low_precision("fp16 kernel"))
        tc = es.enter_context(tile.TileContext(nc))
        pool = lambda *a, **k: es.enter_context(tc.tile_pool(*a, **k))
        pp = pool(name="persist", bufs=1)

        # ---------------- persistent tiles ----------------
        xt = pp.tile([128, ND * T], F16, tag="xt", name="xt")  # 4MB
        qhp = pp.tile([128, NTB * G * 128], F16, tag="qhp", name="qhp")
        kTn = pp.tile([128, T], F16, tag="kTn", name="kTn")
        V = pp.tile([128, NTB * 128], F16, tag="V", name="V")
        wqs = [pp.tile([128, ND * 128], F16, tag=f"wq{g}", name=f"wq{g}")
               for g in range(G)]
        wk_sb = pp.tile([128, ND * 128], F16, tag="wk", name="wk")
        wv_sb = pp.tile([128, ND * 128], F16, tag="wv", name="wv")
        wo_sb = [pp.tile([128, D], F16, tag=f"wo{g}", name=f"wo{g}")
                 for g in range(G)]
        cqa = pp.tile([128, T], F16, tag="cqa", name="cqa")
        sqa = pp.tile([128, T], F16, tag="sqa", name="sqa")
        cqb = pp.tile([128, T], F16, tag="cqb", name="cqb")
        sqb = pp.tile([128, T], F16, tag="sqb", name="sqb")
        ckt = pp.tile([128, T], F16, tag="ckt", name="ckt")
        skt = pp.tile([128, T], F16, tag="skt", name="skt")
        tblf = pp.tile([128, 7], F32, tag="tblf", name="tblf")
        tblh = pp.tile([128, 194], F16, tag="tblh", name="tblh")
        msk = pp.tile([128, n_mask_cols], F16, tag="msk", name="msk")

        qsc = tblf[:, 0:2]       # f32 per-partition scalars
        ksc = tblf[:, 2:4]
        biasc = tblf[:, 4:6]     # [:,0]=H*EPS  [:,1]=EPS
        expb = tblf[:, 6:7]      # exp bias column (EXPB)
        sel65 = tblh[:, 0:65]    # half-selector cols at 0 and 64
        ones1 = tblh[:, 65:66]
        iden = tblh[:, 66:194]   # fp16 identity

        # ---------------- DMA issue (consume order) ----------------
        def xt_ap(d):
            return xt[:, d * T:(d + 1) * T]

        xtv = xt[:].rearrange("p (a t) -> p a t", a=ND)
        # startup splits: first 2 d-tiles of x and first 2 d-cols of wqA
        nc.sync.dma_start(wqs[0][:, 0:512], wq_d[0][:, 0:512])
        nc.sync.dma_start(wqs[2][:, 0:512], wq_d[2][:, 0:512])
        nc.sync.dma_start(xtv[:, 0:2, :], xT_d[0:2].transpose([1, 0, 2]))
        nc.sync.dma_start(tblf[:], tblf_d[:])
        nc.sync.dma_start(tblh[:], tblh_d[:])
        nc.sync.dma_start(xtv[:, 2:4, :], xT_d[2:4].transpose([1, 0, 2]))
        nc.sync.dma_start(wqs[0][:, 512:2048], wq_d[0][:, 512:2048])
        nc.sync.dma_start(wqs[2][:, 512:2048], wq_d[2][:, 512:2048])
        for i in range(2, 8):
            nc.sync.dma_start(xtv[:, 2 * i:2 * i + 2, :],
                              xT_d[2 * i:2 * i + 2].transpose([1, 0, 2]))
        nc.sync.dma_start(wqs[1][:], wq_d[1])
        nc.sync.dma_start(wqs[3][:], wq_d[3])
        nc.sync.dma_start(cqa[:], cqa_d[:])
        nc.sync.dma_start(sqa[:], sqa_d[:])
        nc.sync.dma_start(cqb[:], cqb_d[:])
        nc.sync.dma_start(sqb[:], sqb_d[:])
        nc.sync.dma_start(wk_sb[:], wk_d[:])
        nc.sync.dma_start(ckt[:], ckt_d[:])
        nc.sync.dma_start(skt[:], skt_d[:])
        nc.sync.dma_start(wv_sb[:], wv_d[:])
        nc.sync.dma_start(msk[:], msk_d[:])
        for g in range(G):
            nc.sync.dma_start(wo_sb[g][:], wo_d[g])

        # ---------------- phase-1 pools ----------------
        es1 = ExitStack()
        pool1 = lambda *a, **k: es1.enter_context(tc.tile_pool(*a, **k))
        sbs = pool1(name="sb_stream", bufs=3)
        rsp = pool1(name="ropes", bufs=3)
        vtp = pool1(name="vtp", bufs=1)
        qhtp = pool1(name="qhtmp", bufs=1)
        qh = [qhtp.tile([128, T], F16, tag=f"qh{g}", name=f"qh{g}")
              for g in range(G)]
        ps1 = ExitStack()
        psproj = ps1.enter_context(tc.tile_pool(name="ps_proj", bufs=4, space="PSUM"))
        ps_ss = ps1.enter_context(tc.tile_pool(name="ps_ss", bufs=1, space="PSUM"))
        ps_v = ps1.enter_context(tc.tile_pool(name="ps_v", bufs=2, space="PSUM"))

        def project4(wa, wb):
            """d-outer accumulation: psums[(fi, c)] = [128, CW] f32."""
            pss = {(fi, c): psproj.tile([128, CW], F32, tag="proj", name="proj")
                   for fi in range(2) for c in range(NCHUNK)}
            for d_i in range(ND):
                for fi, w in enumerate((wa, wb)):
                    for c in range(NCHUNK):
                        nc.tensor.matmul(
                            pss[(fi, c)][:],
                            w[:, d_i * 128:(d_i + 1) * 128],
                            xt_ap(d_i)[:, c * CW:(c + 1) * CW],
                            start=(d_i == 0), stop=(d_i == ND - 1))
            return pss

        def rope(psa, psb, out_a, out_b, cs):
            m1 = sbs.tile([128, CW], F16, tag="m1", name="m1")
            m2 = sbs.tile([128, CW], F16, tag="m2", name="m2")
            nc.vector.tensor_mul(m1[:], psa, cqa[:, cs])
            nc.vector.tensor_mul(m2[:], psb, sqb[:, cs])
            nc.vector.tensor_sub(out_a, m1[:], m2[:])
            nc.vector.tensor_mul(m1[:], psb, cqb[:, cs])
            nc.vector.tensor_mul(m2[:], psa, sqa[:, cs])
            nc.vector.tensor_add(out_b, m1[:], m2[:])

        # warm the Exp activation table early so the load is off the
        # attention critical path
        warm = sbs.tile([1, 2], F16, tag="warm", name="warm")

        # ---------------- q pairs ----------------
        for pi in range(2):
            wa, wb = (wqs[0], wqs[2]) if pi == 0 else (wqs[1], wqs[3])
            ga, gb = (0, 1) if pi == 0 else (2, 3)
            pss = project4(wa, wb)
            pcs = {}
            for c in range(NCHUNK):
                pca = sbs.tile([128, CW], F16, tag="pca", name="pca")
                pcb = sbs.tile([128, CW], F16, tag="pcb", name="pcb")
                nc.scalar.copy(pca[:], pss[(0, c)][:])
                nc.vector.tensor_copy(pcb[:], pss[(1, c)][:])
                pcs[c] = (pca, pcb)
            for c in range(NCHUNK):
                cs = slice(c * CW, (c + 1) * CW)
                pca, pcb = pcs[c]
                ssq = ps_ss.tile([65, CW], F32, tag="ss", name="ss")
                for i, pc in enumerate([pca, pcb]):
                    sq = sbs.tile([128, CW], F16, tag="sq", name="sq")
                    nc.vector.tensor_mul(sq[:], pc[:], pc[:])
                    nc.tensor.matmul(ssq[:], sel65, sq[:], start=(i == 0), stop=(i == 1))
                ra = rsp.tile([128, CW], F16, tag="ra", name="ra")
                rb = rsp.tile([128, CW], F16, tag="rb", name="rb")
                rope(pca[:], pcb[:], ra[:], rb[:], cs)
                stmp0 = sbs.tile([1, CW], F32, tag="stmp0", name="stmp0")
                stmp1 = sbs.tile([1, CW], F32, tag="stmp1", name="stmp1")
                nc.scalar.activation(stmp0[:], ssq[0:1, :],
                                     mybir.ActivationFunctionType.Sqrt,
                                     bias=biasc[0:1, 1:2], scale=float(1.0 / H))
                nc.scalar.activation(stmp1[:], ssq[64:65, :],
                                     mybir.ActivationFunctionType.Sqrt,
                                     bias=biasc[0:1, 1:2], scale=float(1.0 / H))
                rstd0 = sbs.tile([1, CW], F16, tag="rstd0", name="rstd0")
                rstd1 = sbs.tile([1, CW], F16, tag="rstd1", name="rstd1")
                nc.vector.reciprocal(rstd0[:], stmp0[:])
                nc.vector.reciprocal(rstd1[:], stmp1[:])
                bca = sbs.tile([128, CW], F16, tag="bca", name="bca")
                bcb = sbs.tile([128, CW], F16, tag="bcb", name="bcb")
                nc.gpsimd.partition_broadcast(bca[:], rstd0[:], channels=128)
                nc.gpsimd.partition_broadcast(bcb[:], rstd1[:], channels=128)
                nc.vector.tensor_mul(qh[ga][0:64, cs], ra[0:64, :], bca[0:64, :])
                nc.vector.tensor_mul(qh[gb][0:64, cs], ra[64:128, :], bcb[64:128, :])
                nc.vector.tensor_mul(qh[ga][64:128, cs], rb[0:64, :], bca[0:64, :])
                nc.vector.tensor_mul(qh[gb][64:128, cs], rb[64:128, :], bcb[64:128, :])
                for g_, tile_ in ((ga, None), (gb, None)):
                    for a_ in range(4 * c, 4 * c + 4):
                        eng = nc.vector.tensor_copy if (g_ + a_) % 2 else nc.scalar.copy
                        eng(qhp[:, a_ * 512 + g_ * 128:a_ * 512 + (g_ + 1) * 128],
                            qh[g_][:, a_ * 128:(a_ + 1) * 128])

        # ---------------- k ----------------
        for c in range(NCHUNK):
            cs = slice(c * CW, (c + 1) * CW)
            psk = psproj.tile([128, CW], F32, tag="proj", name="proj")
            for d_i in range(ND):
                nc.tensor.matmul(psk[:], wk_sb[:, d_i * 128:(d_i + 1) * 128],
                                 xt_ap(d_i)[:, cs],
                                 start=(d_i == 0), stop=(d_i == ND - 1))
            pck = sbs.tile([128, CW], F16, tag="pck", name="pck")
            nc.scalar.copy(pck[:], psk[:])
            sqk = sbs.tile([128, CW], F16, tag="sqk", name="sqk")
            nc.vector.tensor_mul(sqk[:], pck[:], pck[:])
            # row-form sumsq -> sexp = 1/sqrt(sumsq + H*eps) = SCALE*rstd_k
            kssr = ps_ss.tile([1, CW], F32, tag="kssr", name="kssr")
            nc.tensor.matmul(kssr[:], ones1, sqk[:], start=True, stop=True)
            ktmp = sbs.tile([1, CW], F32, tag="ktmp", name="ktmp")
            nc.scalar.activation(ktmp[:], kssr[:],
                                 mybir.ActivationFunctionType.Sqrt,
                                 bias=biasc[0:1, 0:1], scale=1.0)
            krst = sbs.tile([1, CW], F16, tag="krst", name="krst")
            nc.vector.reciprocal(krst[:], ktmp[:])
            last_ktmp = ktmp
            m1 = sbs.tile([128, CW], F16, tag="m1", name="m1")
            m2 = sbs.tile([128, CW], F16, tag="m2", name="m2")
            k0, k1 = pck[0:64, :], pck[64:128, :]
            nc.vector.tensor_mul(m1[0:64, :], k0, ckt[0:64, cs])
            nc.vector.tensor_mul(m2[0:64, :], k1, skt[64:128, cs])
            nc.vector.tensor_sub(kTn[0:64, cs], m1[0:64, :], m2[0:64, :])
            nc.vector.tensor_mul(m1[0:64, :], k1, ckt[64:128, cs])
            nc.vector.tensor_mul(m2[0:64, :], k0, skt[0:64, cs])
            nc.vector.tensor_add(kTn[64:128, cs], m1[0:64, :], m2[0:64, :])
            # fold SCALE*rstd_k into this chunk of kTn
            kbcc = sbs.tile([128, CW], F16, tag="kbcc", name="kbcc")
            nc.gpsimd.partition_broadcast(kbcc[:], krst[:], channels=128)
            nc.vector.tensor_mul(kTn[:, cs], kTn[:, cs], kbcc[:])

        # switch the Act table to the exp set now, off the attention
        # critical path; reading the last sqrt output forces the scheduler
        # to place this after every sqrt-set activation
        nc.scalar.activation(warm[:], last_ktmp[0:1, 0:2],
                             mybir.ActivationFunctionType.Exp,
                             bias=expb[0:1, 0:1], scale=-1.0)

        # ---------------- v: VT projection + PE transposes ----------------
        vts = {}
        for c in range(NCHUNK):
            cs = slice(c * CW, (c + 1) * CW)
            psv = psproj.tile([128, CW], F32, tag="proj", name="proj")
            for d_i in range(ND):
                nc.tensor.matmul(psv[:], wv_sb[:, d_i * 128:(d_i + 1) * 128],
                                 xt_ap(d_i)[:, cs],
                                 start=(d_i == 0), stop=(d_i == ND - 1))
            vt_c = vtp.tile([128, CW], F16, tag=f"vt{c}", name=f"vt{c}")
            nc.scalar.copy(vt_c[:], psv[:])
            vts[c] = vt_c
            for jj in range(4):
                j = 4 * c + jj
                vp = ps_v.tile([128, 128], F16, tag="pv", name="pv")
                nc.tensor.transpose(vp[:], vt_c[:, jj * 128:(jj + 1) * 128], iden)
                if j % 2 == 0:
                    nc.scalar.copy(V[:, j * 128:(j + 1) * 128], vp[:])
                else:
                    nc.vector.tensor_copy(V[:, j * 128:(j + 1) * 128], vp[:])

        es1.close()
        ps1.close()

        # ---------------- attention + out-projection per t-block ----------------
        sbP = pool(name="sbP", bufs=5)
        sbD = pool(name="sbD", bufs=4)
        sbx = pool(name="sbx", bufs=3)
        osp = pool(name="outs", bufs=3)
        ps_lg = pool(name="ps_lg", bufs=4, space="PSUM")
        ps_qkv = pool(name="ps_qkv", bufs=2, space="PSUM")
        ps_op = pool(name="ps_op", bufs=2, space="PSUM")

        moff = []
        off = 0
        for tb in range(NTB):
            moff.append(off)
            off += len(plan[tb])

        from concourse import bass_isa

        def outproj_dc(tb, qkvh, ob, dc, flush):
            t0 = tb * 128
            op = ps_op.tile([128, CW], F32, tag="op", name="op")
            for g in range(G):
                nc.tensor.matmul(op[:],
                                 qkvh[:, g * 128:(g + 1) * 128],
                                 wo_sb[g][:, dc * CW:(dc + 1) * CW],
                                 start=(g == 0), stop=(g == G - 1))
            if dc % 2 == 0:
                nc.vector.tensor_copy(ob[:, dc * CW:(dc + 1) * CW], op[:])
            else:
                nc.scalar.copy(ob[:, dc * CW:(dc + 1) * CW], op[:])
            if flush:
                nc.sync.dma_start(
                    out_d[t0:t0 + 128, dc * CW:(dc + 1) * CW],
                    ob[:, dc * CW:(dc + 1) * CW])
            elif dc == 3:
                nc.sync.dma_start(out_d[t0:t0 + 128, :], ob[:])

        tb_order = sorted(range(NTB), key=lambda t: -len(plan[t]))
        prev = None
        for tb in tb_order:
            ent = plan[tb]
            nv = len(ent)
            t0 = tb * 128
            qkv_ps = ps_qkv.tile([128, 512], F32, tag="qkv", name="qkv")
            qkvh = sbx.tile([128, 512], F16, tag="qkvh", name="qkvh")
            ob = osp.tile([128, D], F16, tag="ob", name="ob")
            acc = sbD.tile([128, 512], F16, tag="dfold", name="dfold")

            Ps = {}
            dbcs = {}
            ndc = 0

            def softmax_si(i, si):
                """one 512-wide logits matmul for all 4 heads, exp, mask,
                and the Pool denominator reduction for s-block si"""
                lg = ps_lg.tile([128, 512], F32, tag="lg", name="lg")
                nc.tensor.matmul(lg[:], kTn[:, si * 128:(si + 1) * 128],
                                 qhp[:, tb * 512:(tb + 1) * 512],
                                 start=True, stop=True)
                P = sbP.tile([128, 512], F16, tag="P", name="P")
                nc.scalar.activation(P[:], lg[:],
                                     mybir.ActivationFunctionType.Exp,
                                     bias=expb[:, 0:1], scale=1.0)
                nc.vector.tensor_mul(P[:], P[:],
                                     msk[:, (moff[tb] + i) * 512:
                                          (moff[tb] + i + 1) * 512])
                dbc = sbD.tile([128, 512], F16, tag="dbc", name="dbc")
                nc.gpsimd.partition_all_reduce(dbc[:], P[:], channels=128,
                                               reduce_op=bass_isa.ReduceOp.add)
                dbcs[i] = dbc
                if i == 1:
                    nc.vector.tensor_add(acc[:], dbcs[0][:], dbc[:])
                elif i > 1:
                    nc.vector.tensor_add(acc[:], acc[:], dbc[:])
                return P

            for i, si in enumerate(ent):
                Ps[i] = softmax_si(i, si)
                if prev is not None and ndc < 4:
                    outproj_dc(prev[0], prev[1], prev[2], ndc, False)
                    ndc += 1
            if prev is not None:
                while ndc < 4:
                    outproj_dc(prev[0], prev[1], prev[2], ndc, False)
                    ndc += 1
            # per-head qkv with contiguous PSUM accumulation groups
            for g in range(G):
                gs = slice(g * 128, (g + 1) * 128)
                for i, si in enumerate(ent):
                    nc.tensor.matmul(qkv_ps[:, gs],
                                     V[:, si * 128:(si + 1) * 128],
                                     Ps[i][:, gs],
                                     start=(i == 0), stop=(i == nv - 1))
            Ps.clear()

            rec = sbD.tile([128, 512], F16, tag="recg", name="recg")
            nc.vector.reciprocal(rec[:], acc[:] if nv > 1 else dbcs[0][:])
            nc.vector.tensor_mul(qkvh[:], qkv_ps[:], rec[:])
            prev = (tb, qkvh, ob)
        for dc in range(4):
            outproj_dc(prev[0], prev[1], prev[2], dc, True)

    nc.finalize()
    return nc


_CACHE = {}


def kernel(x, segment_ids, Wq, Wk, Wv, Wo, q_scale, k_scale):
    global LAST_RESULTS
    import os

    x = np.asarray(x, np.float32)
    seg = np.asarray(segment_ids)
    Wq = np.asarray(Wq, np.float32)
    Wk = np.asarray(Wk, np.float32)
    Wv = np.asarray(Wv, np.float32)
    Wo = np.asarray(Wo, np.float32)
    q_scale = np.asarray(q_scale, np.float32)
    k_scale = np.asarray(k_scale, np.float32)

    plan, masks = _classify([seg[b] for b in range(B)])
    key = repr(plan)
    if key not in _CACHE:
        _CACHE[key] = _build_nc(plan, masks[0].shape[1])
    nc = _CACHE[key]

    half = H // 2
    timescale = ROPE_BASE ** (2.0 * np.arange(half, dtype=np.float64) / H)
    qscA = np.tile(q_scale[:64], 2).astype(np.float64)[:, None]
    qscB = np.tile(q_scale[64:], 2).astype(np.float64)[:, None]
    kvec = k_scale.astype(np.float64)[:, None]
    tabs = []  # per batch: (cqa, sqa, cqb, sqb, ckt, skt)
    for b in range(B):
        pos = _positions(seg[b])
        sinus = pos[:, None] / timescale[None, :]
        sT = np.sin(sinus).T
        cT = np.cos(sinus).T
        c2 = np.vstack([cT, cT])
        s2 = np.vstack([sT, sT])
        tabs.append(tuple(
            np.ascontiguousarray(a, np.float16)
            for a in (c2 * qscA, s2 * qscA, c2 * qscB, s2 * qscB,
                      c2 * kvec, s2 * kvec)))

    tblf = np.zeros((128, 7), np.float32)
    tblf[:, 0] = np.tile(q_scale[:64], 2)
    tblf[:, 1] = np.tile(q_scale[64:], 2)
    tblf[0:64, 2] = k_scale[:64]
    tblf[64:128, 3] = k_scale[64:]
    tblf[:, 4] = H * EPS
    tblf[:, 5] = EPS
    tblf[:, 6] = EXPB
    tblh = np.zeros((128, 194), np.float16)
    tblh[0:64, 0] = 1.0
    tblh[64:128, 64] = 1.0
    tblh[:, 65] = 1.0
    tblh[:, 66:194] = np.eye(128, dtype=np.float16)

    in_maps = []
    for core in range(8):
        b, kv = core // K, core % K
        qcols = []
        for hv in range(2):
            for g4 in range(G):
                base = kv * 512 + g4 * 128 + hv * 64
                qcols.extend(range(base, base + 64))
        qp = np.array(qcols)
        wq_t = np.ascontiguousarray(
            Wq[:, qp].reshape(ND, 128, G, 128).transpose(2, 1, 0, 3)
            .reshape(G, 128, ND * 128), np.float16)
        wk_t = np.ascontiguousarray(
            Wk[:, kv * 128:(kv + 1) * 128].reshape(ND, 128, 128)
            .transpose(1, 0, 2).reshape(128, ND * 128), np.float16)
        wv_t = np.ascontiguousarray(
            Wv[:, kv * 128:(kv + 1) * 128].reshape(ND, 128, 128)
            .transpose(1, 0, 2).reshape(128, ND * 128), np.float16)
        wo_t = np.ascontiguousarray(
            Wo[kv * 512:(kv + 1) * 512].reshape(G, 128, D), np.float16)
        xt_t = np.ascontiguousarray(
            x[b].T.reshape(ND, 128, T), np.float16)
        cqa, sqa, cqb, sqb, ckt, skt = tabs[b]
        in_maps.append({
            "xT": xt_t, "wq": wq_t, "wk": wk_t, "wv": wv_t, "wo": wo_t,
            "cqa": cqa, "sqa": sqa, "cqb": cqb, "sqb": sqb,
            "ckt": ckt, "skt": skt,
            "tblf": tblf, "tblh": tblh, "masks": masks[b],
        })

    do_trace = os.environ.get("BASS_TRACE") == "1"
    res = run_bass_kernel_spmd(
        nc, in_maps, core_ids=list(range(8)), trace=do_trace)
    LAST_RESULTS = res

    out = np.zeros((B, T, D), np.float32)
    for core in range(8):
        out[core // K] += res.results[core]["out"].astype(np.float32)
    return out


# revision 14
# speedup vs baseline: 1.5215x; 1.0080x over previous
"""Trainium2 Bass kernel v2 for segment-causal GQA attention.

Sharding: 8 cores = batch (2) x kv-head (4); host sums the 4 row-parallel
Wo partial outputs per batch.  All device compute in fp16 (1 PE cycle/row
at any moving width, 2-byte DVE fast modes, half the DMA bytes of fp32).

Layout per core (T=1024, D=2048, H=128, G=4 q-heads):
  xt      [128, 16*1024]  x[b]^T d-tiles side by side (4 DMA'd groups)
  qh[g]   [128, T]   rope'd, rstd-scaled q per head (transposed)
  kTn     [128, T]   rope'd k, with SCALE*rstd_k folded in per-column
  V       [128, 8*128]  v in [s,h] layout per 128-s-block (direct proj)
  attention: per 128-wide t-block tb, the <=4 valid s-blocks' logits are
  packed into one PSUM bank [128, nv*128]; one exp (bias=-4 keeps P in
  fp16 range without max-subtraction), one packed mask multiply, per-
  block qkv/den accumulation; the out-projection of each tb (4x4
  matmuls into [128t, 512d] psums) interleaves with the next tb's
  softmax work to keep the PE saturated.
"""

import sys

sys.path.insert(0, "/opt/trn_rl_repo")

import numpy as np

import concourse.bacc as bacc
import concourse.bass as bass  # noqa: F401
import concourse.tile as tile
from concourse import mybir
from concourse.bass_utils import run_bass_kernel_spmd

B, T, D = 2, 1024, 2048
N, K, H = 16, 4, 128
G = N // K
EPS = 1e-6
SCALE = H ** -0.5
ROPE_BASE = 10000.0
NCHUNK = 2
CW = T // NCHUNK        # 512
NTB = T // 128          # 8 t-blocks (and s-blocks)
ND = D // 128           # 16
F32 = mybir.dt.float32
F16 = mybir.dt.float16
MULT = mybir.AluOpType.mult
EXPB = -4.0             # exp bias: keeps P in fp16 range without max-sub

LAST_RESULTS = None


def _positions(seg):
    t = seg.shape[0]
    idx = np.arange(t, dtype=np.int64)
    is_start = np.concatenate([[True], seg[1:] != seg[:-1]])
    seg_start = np.maximum.accumulate(np.where(is_start, idx, 0))
    return (idx - seg_start).astype(np.float64)


def _classify(seg_rows):
    """Union-over-batches 128x128 block plan.

    Returns (plan, masks): plan[tb] = list of valid s-block indices;
    masks[b] = fp16 [128, n_blocks*128] 0/1 pack in plan order.
    """
    idx = np.arange(T)
    valids = []
    for b in range(B):
        seg = seg_rows[b]
        valids.append((seg[:, None] == seg[None, :]) & (idx[:, None] <= idx[None, :]))
    plan = []
    packs = [[] for _ in range(B)]
    for tb in range(NTB):
        t0 = tb * 128
        ent = []
        for si in range(NTB):
            s0 = si * 128
            subs = [v[s0:s0 + 128, t0:t0 + 128] for v in valids]
            if any(s.any() for s in subs):
                ent.append(si)
                for b in range(B):
                    packs[b].append(subs[b])
        plan.append(ent)
    masks = []
    for b in range(B):
        if packs[b]:
            m = np.concatenate([np.tile(p, (1, 4)) for p in packs[b]], axis=1)
        else:
            m = np.zeros((128, 512), bool)
        masks.append(np.ascontiguousarray(m.astype(np.float16)))
    return plan, masks


def _build_nc(plan, n_mask_cols):
    from contextlib import ExitStack

    nc = bacc.Bacc(None, target_bir_lowering=False, debug=False)
    xT_d = nc.dram_tensor("xT", [ND, 128, T], F16, kind="ExternalInput")
    wq_d = nc.dram_tensor("wq", [G, 128, ND * 128], F16, kind="ExternalInput")
    wk_d = nc.dram_tensor("wk", [128, ND * 128], F16, kind="ExternalInput")
    wv_d = nc.dram_tensor("wv", [128, ND * 128], F16, kind="ExternalInput")
    wo_d = nc.dram_tensor("wo", [G, 128, D], F16, kind="ExternalInput")
    # prescaled rope tables: cos/sin x per-partition rms-scale columns
    cqa_d = nc.dram_tensor("cqa", [128, T], F16, kind="ExternalInput")
    sqa_d = nc.dram_tensor("sqa", [128, T], F16, kind="ExternalInput")
    cqb_d = nc.dram_tensor("cqb", [128, T], F16, kind="ExternalInput")
    sqb_d = nc.dram_tensor("sqb", [128, T], F16, kind="ExternalInput")
    ckt_d = nc.dram_tensor("ckt", [128, T], F16, kind="ExternalInput")
    skt_d = nc.dram_tensor("skt", [128, T], F16, kind="ExternalInput")
    tblf_d = nc.dram_tensor("tblf", [128, 7], F32, kind="ExternalInput")
    tblh_d = nc.dram_tensor("tblh", [128, 194], F16, kind="ExternalInput")
    msk_d = nc.dram_tensor("masks", [128, n_mask_cols], F16, kind="ExternalInput")
    out_d = nc.dram_tensor("out", [T, D], F16, kind="ExternalOutput")

    es = ExitStack()
    with es:
        es.enter_context(nc.allow_low_precision("fp16 kernel"))
        tc = es.enter_context(tile.TileContext(nc))
        pool = lambda *a, **k: es.enter_context(tc.tile_pool(*a, **k))
        pp = pool(name="persist", bufs=1)

        # ---------------- persistent tiles ----------------
        xt = pp.tile([128, ND * T], F16, tag="xt", name="xt")  # 4MB
        qhp = pp.tile([128, NTB * G * 128], F16, tag="qhp", name="qhp")
        qhv = qhp[:].rearrange("p (a g t) -> p a g t", a=NTB, g=G)
        kTn = pp.tile([128, T], F16, tag="kTn", name="kTn")
        V = pp.tile([128, NTB * 128], F16, tag="V", name="V")
        wqs = [pp.tile([128, ND * 128], F16, tag=f"wq{g}", name=f"wq{g}")
               for g in range(G)]
        wk_sb = pp.tile([128, ND * 128], F16, tag="wk", name="wk")
        wv_sb = pp.tile([128, ND * 128], F16, tag="wv", name="wv")
        wo_sb = [pp.tile([128, D], F16, tag=f"wo{g}", name=f"wo{g}")
                 for g in range(G)]
        cqa = pp.tile([128, T], F16, tag="cqa", name="cqa")
        sqa = pp.tile([128, T], F16, tag="sqa", name="sqa")
        cqb = pp.tile([128, T], F16, tag="cqb", name="cqb")
        sqb = pp.tile([128, T], F16, tag="sqb", name="sqb")
        ckt = pp.tile([128, T], F16, tag="ckt", name="ckt")
        skt = pp.tile([128, T], F16, tag="skt", name="skt")
        tblf = pp.tile([128, 7], F32, tag="tblf", name="tblf")
        tblh = pp.tile([128, 194], F16, tag="tblh", name="tblh")
        msk = pp.tile([128, n_mask_cols], F16, tag="msk", name="msk")

        qsc = tblf[:, 0:2]       # f32 per-partition scalars
        ksc = tblf[:, 2:4]
        biasc = tblf[:, 4:6]     # [:,0]=H*EPS  [:,1]=EPS
        expb = tblf[:, 6:7]      # exp bias column (EXPB)
        sel65 = tblh[:, 0:65]    # half-selector cols at 0 and 64
        ones1 = tblh[:, 65:66]
        iden = tblh[:, 66:194]   # fp16 identity

        # ---------------- DMA issue (consume order) ----------------
        def xt_ap(d):
            return xt[:, d * T:(d + 1) * T]

        xtv = xt[:].rearrange("p (a t) -> p a t", a=ND)
        # startup splits: first 2 d-tiles of x and first 2 d-cols of wqA
        nc.sync.dma_start(wqs[0][:, 0:512], wq_d[0][:, 0:512])
        nc.sync.dma_start(wqs[2][:, 0:512], wq_d[2][:, 0:512])
        nc.sync.dma_start(xtv[:, 0:2, :], xT_d[0:2].transpose([1, 0, 2]))
        nc.sync.dma_start(tblf[:], tblf_d[:])
        nc.sync.dma_start(tblh[:], tblh_d[:])
        nc.sync.dma_start(xtv[:, 2:4, :], xT_d[2:4].transpose([1, 0, 2]))
        nc.sync.dma_start(wqs[0][:, 512:2048], wq_d[0][:, 512:2048])
        nc.sync.dma_start(wqs[2][:, 512:2048], wq_d[2][:, 512:2048])
        for i in range(2, 8):
            nc.sync.dma_start(xtv[:, 2 * i:2 * i + 2, :],
                              xT_d[2 * i:2 * i + 2].transpose([1, 0, 2]))
        nc.sync.dma_start(wqs[1][:], wq_d[1])
        nc.sync.dma_start(wqs[3][:], wq_d[3])
        nc.sync.dma_start(cqa[:], cqa_d[:])
        nc.sync.dma_start(sqa[:], sqa_d[:])
        nc.sync.dma_start(cqb[:], cqb_d[:])
        nc.sync.dma_start(sqb[:], sqb_d[:])
        nc.sync.dma_start(wk_sb[:], wk_d[:])
        nc.sync.dma_start(ckt[:], ckt_d[:])
        nc.sync.dma_start(skt[:], skt_d[:])
        nc.sync.dma_start(wv_sb[:], wv_d[:])
        nc.sync.dma_start(msk[:], msk_d[:])
        for g in range(G):
            nc.sync.dma_start(wo_sb[g][:], wo_d[g])

        # ---------------- phase-1 pools ----------------
        es1 = ExitStack()
        pool1 = lambda *a, **k: es1.enter_context(tc.tile_pool(*a, **k))
        sbs = pool1(name="sb_stream", bufs=4)
        rsp = pool1(name="ropes", bufs=3)
        vtp = pool1(name="vtp", bufs=1)
        ps1 = ExitStack()
        psproj = ps1.enter_context(tc.tile_pool(name="ps_proj", bufs=4, space="PSUM"))
        ps_ss = ps1.enter_context(tc.tile_pool(name="ps_ss", bufs=1, space="PSUM"))
        ps_v = ps1.enter_context(tc.tile_pool(name="ps_v", bufs=2, space="PSUM"))

        def project4(wa, wb):
            """d-outer accumulation: psums[(fi, c)] = [128, CW] f32."""
            pss = {(fi, c): psproj.tile([128, CW], F32, tag="proj", name="proj")
                   for fi in range(2) for c in range(NCHUNK)}
            for d_i in range(ND):
                for fi, w in enumerate((wa, wb)):
                    for c in range(NCHUNK):
                        nc.tensor.matmul(
                            pss[(fi, c)][:],
                            w[:, d_i * 128:(d_i + 1) * 128],
                            xt_ap(d_i)[:, c * CW:(c + 1) * CW],
                            start=(d_i == 0), stop=(d_i == ND - 1))
            return pss

        def rope(psa, psb, out_a, out_b, cs):
            m1 = sbs.tile([128, CW], F16, tag="m1", name="m1")
            m2 = sbs.tile([128, CW], F16, tag="m2", name="m2")
            nc.vector.tensor_mul(m1[:], psa, cqa[:, cs])
            nc.vector.tensor_mul(m2[:], psb, sqb[:, cs])
            nc.vector.tensor_sub(out_a, m1[:], m2[:])
            nc.vector.tensor_mul(m1[:], psb, cqb[:, cs])
            nc.vector.tensor_mul(m2[:], psa, sqa[:, cs])
            nc.vector.tensor_add(out_b, m1[:], m2[:])

        # warm the Exp activation table early so the load is off the
        # attention critical path
        warm = sbs.tile([1, 2], F16, tag="warm", name="warm")

        # ---------------- q pairs ----------------
        for pi in range(2):
            wa, wb = (wqs[0], wqs[2]) if pi == 0 else (wqs[1], wqs[3])
            ga, gb = (0, 1) if pi == 0 else (2, 3)
            pss = project4(wa, wb)
            pcs = {}
            for c in range(NCHUNK):
                pca = sbs.tile([128, CW], F16, tag="pca", name="pca")
                pcb = sbs.tile([128, CW], F16, tag="pcb", name="pcb")
                nc.scalar.copy(pca[:], pss[(0, c)][:])
                nc.vector.tensor_copy(pcb[:], pss[(1, c)][:])
                pcs[c] = (pca, pcb)
            for c in range(NCHUNK):
                cs = slice(c * CW, (c + 1) * CW)
                pca, pcb = pcs[c]
                ssq = ps_ss.tile([65, CW], F32, tag="ss", name="ss")
                for i, pc in enumerate([pca, pcb]):
                    sq = sbs.tile([128, CW], F16, tag="sq", name="sq")
                    nc.vector.tensor_mul(sq[:], pc[:], pc[:])
                    nc.tensor.matmul(ssq[:], sel65, sq[:], start=(i == 0), stop=(i == 1))
                ra = rsp.tile([128, CW], F16, tag="ra", name="ra")
                rb = rsp.tile([128, CW], F16, tag="rb", name="rb")
                rope(pca[:], pcb[:], ra[:], rb[:], cs)
                stmp0 = sbs.tile([1, CW], F32, tag="stmp0", name="stmp0")
                stmp1 = sbs.tile([1, CW], F32, tag="stmp1", name="stmp1")
                nc.scalar.activation(stmp0[:], ssq[0:1, :],
                                     mybir.ActivationFunctionType.Sqrt,
                                     bias=biasc[0:1, 1:2], scale=float(1.0 / H))
                nc.scalar.activation(stmp1[:], ssq[64:65, :],
                                     mybir.ActivationFunctionType.Sqrt,
                                     bias=biasc[0:1, 1:2], scale=float(1.0 / H))
                rstd0 = sbs.tile([1, CW], F16, tag="rstd0", name="rstd0")
                rstd1 = sbs.tile([1, CW], F16, tag="rstd1", name="rstd1")
                nc.vector.reciprocal(rstd0[:], stmp0[:])
                nc.vector.reciprocal(rstd1[:], stmp1[:])
                bca = sbs.tile([128, CW], F16, tag="bca", name="bca")
                bcb = sbs.tile([128, CW], F16, tag="bcb", name="bcb")
                nc.gpsimd.partition_broadcast(bca[:], rstd0[:], channels=128)
                nc.gpsimd.partition_broadcast(bcb[:], rstd1[:], channels=128)
                tbs = slice(4 * c, 4 * c + 4)
                r3 = lambda ap: ap.rearrange("p (a t) -> p a t", a=4)
                nc.vector.tensor_mul(qhv[0:64, tbs, ga, :], r3(ra[0:64, :]),
                                     r3(bca[0:64, :]))
                nc.vector.tensor_mul(qhv[0:64, tbs, gb, :], r3(ra[64:128, :]),
                                     r3(bcb[64:128, :]))
                nc.vector.tensor_mul(qhv[64:128, tbs, ga, :], r3(rb[0:64, :]),
                                     r3(bca[0:64, :]))
                nc.vector.tensor_mul(qhv[64:128, tbs, gb, :], r3(rb[64:128, :]),
                                     r3(bcb[64:128, :]))

        # ---------------- k ----------------
        for c in range(NCHUNK):
            cs = slice(c * CW, (c + 1) * CW)
            psk = psproj.tile([128, CW], F32, tag="proj", name="proj")
            for d_i in range(ND):
                nc.tensor.matmul(psk[:], wk_sb[:, d_i * 128:(d_i + 1) * 128],
                                 xt_ap(d_i)[:, cs],
                                 start=(d_i == 0), stop=(d_i == ND - 1))
            pck = sbs.tile([128, CW], F16, tag="pck", name="pck")
            nc.scalar.copy(pck[:], psk[:])
            sqk = sbs.tile([128, CW], F16, tag="sqk", name="sqk")
            nc.vector.tensor_mul(sqk[:], pck[:], pck[:])
            # row-form sumsq -> sexp = 1/sqrt(sumsq + H*eps) = SCALE*rstd_k
            kssr = ps_ss.tile([1, CW], F32, tag="kssr", name="kssr")
            nc.tensor.matmul(kssr[:], ones1, sqk[:], start=True, stop=True)
            ktmp = sbs.tile([1, CW], F32, tag="ktmp", name="ktmp")
            nc.scalar.activation(ktmp[:], kssr[:],
                                 mybir.ActivationFunctionType.Sqrt,
                                 bias=biasc[0:1, 0:1], scale=1.0)
            krst = sbs.tile([1, CW], F16, tag="krst", name="krst")
            nc.vector.reciprocal(krst[:], ktmp[:])
            last_ktmp = ktmp
            m1 = sbs.tile([128, CW], F16, tag="m1", name="m1")
            m2 = sbs.tile([128, CW], F16, tag="m2", name="m2")
            k0, k1 = pck[0:64, :], pck[64:128, :]
            nc.vector.tensor_mul(m1[0:64, :], k0, ckt[0:64, cs])
            nc.vector.tensor_mul(m2[0:64, :], k1, skt[64:128, cs])
            nc.vector.tensor_sub(kTn[0:64, cs], m1[0:64, :], m2[0:64, :])
            nc.vector.tensor_mul(m1[0:64, :], k1, ckt[64:128, cs])
            nc.vector.tensor_mul(m2[0:64, :], k0, skt[0:64, cs])
            nc.vector.tensor_add(kTn[64:128, cs], m1[0:64, :], m2[0:64, :])
            # fold SCALE*rstd_k into this chunk of kTn
            kbcc = sbs.tile([128, CW], F16, tag="kbcc", name="kbcc")
            nc.gpsimd.partition_broadcast(kbcc[:], krst[:], channels=128)
            nc.vector.tensor_mul(kTn[:, cs], kTn[:, cs], kbcc[:])

        # switch the Act table to the exp set now, off the attention
        # critical path; reading the last sqrt output forces the scheduler
        # to place this after every sqrt-set activation
        nc.scalar.activation(warm[:], last_ktmp[0:1, 0:2],
                             mybir.ActivationFunctionType.Exp,
                             bias=expb[0:1, 0:1], scale=-1.0)

        # ---------------- v: VT projection + PE transposes ----------------
        vts = {}
        for c in range(NCHUNK):
            cs = slice(c * CW, (c + 1) * CW)
            psv = psproj.tile([128, CW], F32, tag="proj", name="proj")
            for d_i in range(ND):
                nc.tensor.matmul(psv[:], wv_sb[:, d_i * 128:(d_i + 1) * 128],
                                 xt_ap(d_i)[:, cs],
                                 start=(d_i == 0), stop=(d_i == ND - 1))
            vt_c = vtp.tile([128, CW], F16, tag=f"vt{c}", name=f"vt{c}")
            nc.scalar.copy(vt_c[:], psv[:])
            vts[c] = vt_c
            for jj in range(4):
                j = 4 * c + jj
                vp = ps_v.tile([128, 128], F16, tag="pv", name="pv")
                nc.tensor.transpose(vp[:], vt_c[:, jj * 128:(jj + 1) * 128], iden)
                if j % 2 == 0:
                    nc.scalar.copy(V[:, j * 128:(j + 1) * 128], vp[:])
                else:
                    nc.vector.tensor_copy(V[:, j * 128:(j + 1) * 128], vp[:])

        es1.close()
        ps1.close()

        # ---------------- attention + out-projection per t-block ----------------
        sbP = pool(name="sbP", bufs=5)
        sbD = pool(name="sbD", bufs=4)
        sbx = pool(name="sbx", bufs=3)
        osp = pool(name="outs", bufs=3)
        ps_lg = pool(name="ps_lg", bufs=4, space="PSUM")
        ps_qkv = pool(name="ps_qkv", bufs=2, space="PSUM")
        ps_op = pool(name="ps_op", bufs=2, space="PSUM")

        moff = []
        off = 0
        for tb in range(NTB):
            moff.append(off)
            off += len(plan[tb])

        from concourse import bass_isa

        def outproj_dc(tb, qkvh, ob, dc, flush):
            t0 = tb * 128
            op = ps_op.tile([128, CW], F32, tag="op", name="op")
            for g in range(G):
                nc.tensor.matmul(op[:],
                                 qkvh[:, g * 128:(g + 1) * 128],
                                 wo_sb[g][:, dc * CW:(dc + 1) * CW],
                                 start=(g == 0), stop=(g == G - 1))
            if dc % 2 == 0:
                nc.vector.tensor_copy(ob[:, dc * CW:(dc + 1) * CW], op[:])
            else:
                nc.scalar.copy(ob[:, dc * CW:(dc + 1) * CW], op[:])
            if flush:
                nc.sync.dma_start(
                    out_d[t0:t0 + 128, dc * CW:(dc + 1) * CW],
                    ob[:, dc * CW:(dc + 1) * CW])
            elif dc == 3:
                nc.sync.dma_start(out_d[t0:t0 + 128, :], ob[:])

        tb_order = sorted(range(NTB), key=lambda t: -len(plan[t]))
        prev = None
        for tb in tb_order:
            ent = plan[tb]
            nv = len(ent)
            t0 = tb * 128
            qkv_ps = ps_qkv.tile([128, 512], F32, tag="qkv", name="qkv")
            qkvh = sbx.tile([128, 512], F16, tag="qkvh", name="qkvh")
            ob = osp.tile([128, D], F16, tag="ob", name="ob")
            acc = sbD.tile([128, 512], F16, tag="dfold", name="dfold")

            Ps = {}
            dbcs = {}
            ndc = 0

            def softmax_si(i, si):
                """one 512-wide logits matmul for all 4 heads, exp, mask,
                and the Pool denominator reduction for s-block si"""
                lg = ps_lg.tile([128, 512], F32, tag="lg", name="lg")
                nc.tensor.matmul(lg[:], kTn[:, si * 128:(si + 1) * 128],
                                 qhp[:, tb * 512:(tb + 1) * 512],
                                 start=True, stop=True)
                P = sbP.tile([128, 512], F16, tag="P", name="P")
                nc.scalar.activation(P[:], lg[:],
                                     mybir.ActivationFunctionType.Exp,
                                     bias=expb[:, 0:1], scale=1.0)
                nc.vector.tensor_mul(P[:], P[:],
                                     msk[:, (moff[tb] + i) * 512:
                                          (moff[tb] + i + 1) * 512])
                dbc = sbD.tile([128, 512], F16, tag="dbc", name="dbc")
                nc.gpsimd.partition_all_reduce(dbc[:], P[:], channels=128,
                                               reduce_op=bass_isa.ReduceOp.add)
                dbcs[i] = dbc
                if i == 1:
                    nc.vector.tensor_add(acc[:], dbcs[0][:], dbc[:])
                elif i > 1:
                    nc.vector.tensor_add(acc[:], acc[:], dbc[:])
                return P

            for i, si in enumerate(ent):
                Ps[i] = softmax_si(i, si)
                if prev is not None and ndc < 4:
                    outproj_dc(prev[0], prev[1], prev[2], ndc, False)
                    ndc += 1
            if prev is not None:
                while ndc < 4:
                    outproj_dc(prev[0], prev[1], prev[2], ndc, False)
                    ndc += 1
            # per-head qkv with contiguous PSUM accumulation groups
            for g in range(G):
                gs = slice(g * 128, (g + 1) * 128)
                for i, si in enumerate(ent):
                    nc.tensor.matmul(qkv_ps[:, gs],
                                     V[:, si * 128:(si + 1) * 128],
                                     Ps[i][:, gs],
                                     start=(i == 0), stop=(i == nv - 1))
            Ps.clear()

            rec = sbD.tile([128, 512], F16, tag="recg", name="recg")
            nc.vector.reciprocal(rec[:], acc[:] if nv > 1 else dbcs[0][:])
            nc.vector.tensor_mul(qkvh[:], qkv_ps[:], rec[:])
            prev = (tb, qkvh, ob)
        for dc in range(4):
            outproj_dc(prev[0], prev[1], prev[2], dc, True)

    nc.finalize()
    return nc


_CACHE = {}


def kernel(x, segment_ids, Wq, Wk, Wv, Wo, q_scale, k_scale):
    global LAST_RESULTS
    import os

    x = np.asarray(x, np.float32)
    seg = np.asarray(segment_ids)
    Wq = np.asarray(Wq, np.float32)
    Wk = np.asarray(Wk, np.float32)
    Wv = np.asarray(Wv, np.float32)
    Wo = np.asarray(Wo, np.float32)
    q_scale = np.asarray(q_scale, np.float32)
    k_scale = np.asarray(k_scale, np.float32)

    plan, masks = _classify([seg[b] for b in range(B)])
    key = repr(plan)
    if key not in _CACHE:
        _CACHE[key] = _build_nc(plan, masks[0].shape[1])
    nc = _CACHE[key]

    half = H // 2
    timescale = ROPE_BASE ** (2.0 * np.arange(half, dtype=np.float64) / H)
    qscA = np.tile(q_scale[:64], 2).astype(np.float64)[:, None]
    qscB = np.tile(q_scale[64:], 2).astype(np.float64)[:, None]
    kvec = k_scale.astype(np.float64)[:, None]
    tabs = []  # per batch: (cqa, sqa, cqb, sqb, ckt, skt)
    for b in range(B):
        pos = _positions(seg[b])
        sinus = pos[:, None] / timescale[None, :]
        sT = np.sin(sinus).T
        cT = np.cos(sinus).T
        c2 = np.vstack([cT, cT])
        s2 = np.vstack([sT, sT])
        tabs.append(tuple(
            np.ascontiguousarray(a, np.float16)
            for a in (c2 * qscA, s2 * qscA, c2 * qscB, s2 * qscB,
                      c2 * kvec, s2 * kvec)))

    tblf = np.zeros((128, 7), np.float32)
    tblf[:, 0] = np.tile(q_scale[:64], 2)
    tblf[:, 1] = np.tile(q_scale[64:], 2)
    tblf[0:64, 2] = k_scale[:64]
    tblf[64:128, 3] = k_scale[64:]
    tblf[:, 4] = H * EPS
    tblf[:, 5] = EPS
    tblf[:, 6] = EXPB
    tblh = np.zeros((128, 194), np.float16)
    tblh[0:64, 0] = 1.0
    tblh[64:128, 64] = 1.0
    tblh[:, 65] = 1.0
    tblh[:, 66:194] = np.eye(128, dtype=np.float16)

    in_maps = []
    for core in range(8):
        b, kv = core // K, core % K
        qcols = []
        for hv in range(2):
            for g4 in range(G):
                base = kv * 512 + g4 * 128 + hv * 64
                qcols.extend(range(base, base + 64))
        qp = np.array(qcols)
        wq_t = np.ascontiguousarray(
            Wq[:, qp].reshape(ND, 128, G, 128).transpose(2, 1, 0, 3)
            .reshape(G, 128, ND * 128), np.float16)
        wk_t = np.ascontiguousarray(
            Wk[:, kv * 128:(kv + 1) * 128].reshape(ND, 128, 128)
            .transpose(1, 0, 2).reshape(128, ND * 128), np.float16)
        wv_t = np.ascontiguousarray(
            Wv[:, kv * 128:(kv + 1) * 128].reshape(ND, 128, 128)
            .transpose(1, 0, 2).reshape(128, ND * 128), np.float16)
        wo_t = np.ascontiguousarray(
            Wo[kv * 512:(kv + 1) * 512].reshape(G, 128, D), np.float16)
        xt_t = np.ascontiguousarray(
            x[b].T.reshape(ND, 128, T), np.float16)
        cqa, sqa, cqb, sqb, ckt, skt = tabs[b]
        in_maps.append({
            "xT": xt_t, "wq": wq_t, "wk": wk_t, "wv": wv_t, "wo": wo_t,
            "cqa": cqa, "sqa": sqa, "cqb": cqb, "sqb": sqb,
            "ckt": ckt, "skt": skt,
            "tblf": tblf, "tblh": tblh, "masks": masks[b],
        })

    do_trace = os.environ.get("BASS_TRACE") == "1"
    res = run_bass_kernel_spmd(
        nc, in_maps, core_ids=list(range(8)), trace=do_trace)
    LAST_RESULTS = res

    out = np.zeros((B, T, D), np.float32)
    for core in range(8):
        out[core // K] += res.results[core]["out"].astype(np.float32)
    return out


# revision 16
# speedup vs baseline: 1.5704x; 1.0321x over previous
"""Trainium2 Bass kernel v2 for segment-causal GQA attention.

Sharding: 8 cores = batch (2) x kv-head (4); host sums the 4 row-parallel
Wo partial outputs per batch.  All device compute in fp16 (1 PE cycle/row
at any moving width, 2-byte DVE fast modes, half the DMA bytes of fp32).

Layout per core (T=1024, D=2048, H=128, G=4 q-heads):
  xt      [128, 16*1024]  x[b]^T d-tiles side by side (4 DMA'd groups)
  qh[g]   [128, T]   rope'd, rstd-scaled q per head (transposed)
  kTn     [128, T]   rope'd k, with SCALE*rstd_k folded in per-column
  V       [128, 8*128]  v in [s,h] layout per 128-s-block (direct proj)
  attention: per 128-wide t-block tb, the <=4 valid s-blocks' logits are
  packed into one PSUM bank [128, nv*128]; one exp (bias=-4 keeps P in
  fp16 range without max-subtraction), one packed mask multiply, per-
  block qkv/den accumulation; the out-projection of each tb (4x4
  matmuls into [128t, 512d] psums) interleaves with the next tb's
  softmax work to keep the PE saturated.
"""

import sys

sys.path.insert(0, "/opt/trn_rl_repo")

import numpy as np

import concourse.bacc as bacc
import concourse.bass as bass  # noqa: F401
import concourse.tile as tile
from concourse import mybir
from concourse.bass_utils import run_bass_kernel_spmd

B, T, D = 2, 1024, 2048
N, K, H = 16, 4, 128
G = N // K
EPS = 1e-6
SCALE = H ** -0.5
ROPE_BASE = 10000.0
NCHUNK = 2
CW = T // NCHUNK        # 512
NTB = T // 128          # 8 t-blocks (and s-blocks)
ND = D // 128           # 16
F32 = mybir.dt.float32
F16 = mybir.dt.float16
MULT = mybir.AluOpType.mult
EXPB = -4.0             # exp bias: keeps P in fp16 range without max-sub

LAST_RESULTS = None


def _positions(seg):
    t = seg.shape[0]
    idx = np.arange(t, dtype=np.int64)
    is_start = np.concatenate([[True], seg[1:] != seg[:-1]])
    seg_start = np.maximum.accumulate(np.where(is_start, idx, 0))
    return (idx - seg_start).astype(np.float64)


def _classify(seg_rows):
    """Union-over-batches 128x128 block plan.

    Returns (plan, masks): plan[tb] = list of valid s-block indices;
    masks[b] = fp16 [128, n_blocks*128] 0/1 pack in plan order.
    """
    idx = np.arange(T)
    valids = []
    for b in range(B):
        seg = seg_rows[b]
        valids.append((seg[:, None] == seg[None, :]) & (idx[:, None] <= idx[None, :]))
    plan = []
    packs = [[] for _ in range(B)]
    for tb in range(NTB):
        t0 = tb * 128
        ent = []
        for si in range(NTB):
            s0 = si * 128
            subs = [v[s0:s0 + 128, t0:t0 + 128] for v in valids]
            if any(s.any() for s in subs):
                ent.append(si)
                for b in range(B):
                    packs[b].append(subs[b])
        plan.append(ent)
    masks = []
    for b in range(B):
        if packs[b]:
            m = np.concatenate([np.tile(p, (1, 4)) for p in packs[b]], axis=1)
        else:
            m = np.zeros((128, 512), bool)
        masks.append(np.ascontiguousarray(m.astype(np.float16)))
    return plan, masks


def _build_nc(plan, n_mask_cols):
    from contextlib import ExitStack

    nc = bacc.Bacc(None, target_bir_lowering=False, debug=False)
    xT_d = nc.dram_tensor("xT", [ND, 128, T], F16, kind="ExternalInput")
    wq_d = nc.dram_tensor("wq", [G, 128, ND * 128], F16, kind="ExternalInput")
    wk_d = nc.dram_tensor("wk", [128, ND * 128], F16, kind="ExternalInput")
    wv_d = nc.dram_tensor("wv", [128, ND * 128], F16, kind="ExternalInput")
    wo_d = nc.dram_tensor("wo", [G, 128, D], F16, kind="ExternalInput")
    # prescaled rope tables: cos/sin x per-partition rms-scale columns
    cqa_d = nc.dram_tensor("cqa", [128, T], F16, kind="ExternalInput")
    sqa_d = nc.dram_tensor("sqa", [128, T], F16, kind="ExternalInput")
    cqb_d = nc.dram_tensor("cqb", [128, T], F16, kind="ExternalInput")
    sqb_d = nc.dram_tensor("sqb", [128, T], F16, kind="ExternalInput")
    ckt_d = nc.dram_tensor("ckt", [128, T], F16, kind="ExternalInput")
    skt_d = nc.dram_tensor("skt", [128, T], F16, kind="ExternalInput")
    tblf_d = nc.dram_tensor("tblf", [128, 7], F32, kind="ExternalInput")
    tblh_d = nc.dram_tensor("tblh", [128, 194], F16, kind="ExternalInput")
    msk_d = nc.dram_tensor("masks", [128, n_mask_cols], F16, kind="ExternalInput")
    out_d = nc.dram_tensor("out", [T, D], F16, kind="ExternalOutput")

    es = ExitStack()
    with es:
        es.enter_context(nc.allow_low_precision("fp16 kernel"))
        tc = es.enter_context(tile.TileContext(nc))
        pool = lambda *a, **k: es.enter_context(tc.tile_pool(*a, **k))
        pp = pool(name="persist", bufs=1)

        # ---------------- persistent tiles ----------------
        xt = pp.tile([128, ND * T], F16, tag="xt", name="xt")  # 4MB
        qhp = pp.tile([128, NTB * G * 128], F16, tag="qhp", name="qhp")
        qhv = qhp[:].rearrange("p (a g t) -> p a g t", a=NTB, g=G)
        kTn = pp.tile([128, T], F16, tag="kTn", name="kTn")
        V = pp.tile([128, NTB * 128], F16, tag="V", name="V")
        wqs = [pp.tile([128, ND * 128], F16, tag=f"wq{g}", name=f"wq{g}")
               for g in range(G)]
        wk_sb = pp.tile([128, ND * 128], F16, tag="wk", name="wk")
        wv_sb = pp.tile([128, ND * 128], F16, tag="wv", name="wv")
        wo_sb = [pp.tile([128, D], F16, tag=f"wo{g}", name=f"wo{g}")
                 for g in range(G)]
        cqa = pp.tile([128, T], F16, tag="cqa", name="cqa")
        sqa = pp.tile([128, T], F16, tag="sqa", name="sqa")
        cqb = pp.tile([128, T], F16, tag="cqb", name="cqb")
        sqb = pp.tile([128, T], F16, tag="sqb", name="sqb")
        ckt = pp.tile([128, T], F16, tag="ckt", name="ckt")
        skt = pp.tile([128, T], F16, tag="skt", name="skt")
        tblf = pp.tile([128, 7], F32, tag="tblf", name="tblf")
        tblh = pp.tile([128, 194], F16, tag="tblh", name="tblh")
        msk = pp.tile([128, n_mask_cols], F16, tag="msk", name="msk")

        qsc = tblf[:, 0:2]       # f32 per-partition scalars
        ksc = tblf[:, 2:4]
        biasc = tblf[:, 4:6]     # [:,0]=H*EPS  [:,1]=EPS
        expb = tblf[:, 6:7]      # exp bias column (EXPB)
        sel65 = tblh[:, 0:65]    # half-selector cols at 0 and 64
        ones1 = tblh[:, 65:66]
        iden = tblh[:, 66:194]   # fp16 identity

        # ---------------- DMA issue (consume order) ----------------
        def xt_ap(d):
            return xt[:, d * T:(d + 1) * T]

        xtv = xt[:].rearrange("p (a t) -> p a t", a=ND)
        # startup splits: first 2 d-tiles of x and first 2 d-cols of wqA
        nc.sync.dma_start(wqs[0][:, 0:768], wq_d[0][:, 0:768])
        nc.sync.dma_start(wqs[2][:, 0:768], wq_d[2][:, 0:768])
        nc.sync.dma_start(xtv[:, 0:2, :], xT_d[0:2].transpose([1, 0, 2]))
        nc.sync.dma_start(xtv[:, 2:4, :], xT_d[2:4].transpose([1, 0, 2]))
        nc.sync.dma_start(tblf[:], tblf_d[:])
        nc.sync.dma_start(tblh[:], tblh_d[:])
        nc.sync.dma_start(xtv[:, 4:6, :], xT_d[4:6].transpose([1, 0, 2]))
        nc.sync.dma_start(wqs[0][:, 768:2048], wq_d[0][:, 768:2048])
        nc.sync.dma_start(wqs[2][:, 768:2048], wq_d[2][:, 768:2048])
        for i in range(3, 8):
            nc.sync.dma_start(xtv[:, 2 * i:2 * i + 2, :],
                              xT_d[2 * i:2 * i + 2].transpose([1, 0, 2]))
        nc.sync.dma_start(wqs[1][:], wq_d[1])
        nc.sync.dma_start(wqs[3][:], wq_d[3])
        nc.sync.dma_start(cqa[:], cqa_d[:])
        nc.sync.dma_start(sqa[:], sqa_d[:])
        nc.sync.dma_start(cqb[:], cqb_d[:])
        nc.sync.dma_start(sqb[:], sqb_d[:])
        nc.sync.dma_start(wk_sb[:], wk_d[:])
        nc.sync.dma_start(ckt[:], ckt_d[:])
        nc.sync.dma_start(skt[:], skt_d[:])
        nc.sync.dma_start(wv_sb[:], wv_d[:])
        nc.sync.dma_start(msk[:], msk_d[:])
        for g in range(G):
            nc.sync.dma_start(wo_sb[g][:], wo_d[g])

        # ---------------- phase-1 pools ----------------
        es1 = ExitStack()
        pool1 = lambda *a, **k: es1.enter_context(tc.tile_pool(*a, **k))
        sbs = pool1(name="sb_stream", bufs=4)
        rsp = pool1(name="ropes", bufs=3)
        vtp = pool1(name="vtp", bufs=1)
        ps1 = ExitStack()
        psproj = ps1.enter_context(tc.tile_pool(name="ps_proj", bufs=4, space="PSUM"))
        ps_ss = ps1.enter_context(tc.tile_pool(name="ps_ss", bufs=1, space="PSUM"))
        ps_v = ps1.enter_context(tc.tile_pool(name="ps_v", bufs=2, space="PSUM"))

        def project4(wa, wb):
            """d-outer accumulation: psums[(fi, c)] = [128, CW] f32."""
            pss = {(fi, c): psproj.tile([128, CW], F32, tag="proj", name="proj")
                   for fi in range(2) for c in range(NCHUNK)}
            for d_i in range(ND):
                for fi, w in enumerate((wa, wb)):
                    for c in range(NCHUNK):
                        nc.tensor.matmul(
                            pss[(fi, c)][:],
                            w[:, d_i * 128:(d_i + 1) * 128],
                            xt_ap(d_i)[:, c * CW:(c + 1) * CW],
                            start=(d_i == 0), stop=(d_i == ND - 1))
            return pss

        def rope(psa, psb, out_a, out_b, cs):
            m1 = sbs.tile([128, CW], F16, tag="m1", name="m1")
            m2 = sbs.tile([128, CW], F16, tag="m2", name="m2")
            nc.vector.tensor_mul(m1[:], psa, cqa[:, cs])
            nc.vector.tensor_mul(m2[:], psb, sqb[:, cs])
            nc.vector.tensor_sub(out_a, m1[:], m2[:])
            nc.vector.tensor_mul(m1[:], psb, cqb[:, cs])
            nc.vector.tensor_mul(m2[:], psa, sqa[:, cs])
            nc.vector.tensor_add(out_b, m1[:], m2[:])

        # warm the Exp activation table early so the load is off the
        # attention critical path
        warm = sbs.tile([1, 2], F16, tag="warm", name="warm")

        # ---------------- q pairs ----------------
        for pi in range(2):
            wa, wb = (wqs[0], wqs[2]) if pi == 0 else (wqs[1], wqs[3])
            ga, gb = (0, 1) if pi == 0 else (2, 3)
            pss = project4(wa, wb)
            pcs = {}
            for c in range(NCHUNK):
                pca = sbs.tile([128, CW], F16, tag="pca", name="pca")
                pcb = sbs.tile([128, CW], F16, tag="pcb", name="pcb")
                nc.scalar.copy(pca[:], pss[(0, c)][:])
                nc.vector.tensor_copy(pcb[:], pss[(1, c)][:])
                pcs[c] = (pca, pcb)
            for c in range(NCHUNK):
                cs = slice(c * CW, (c + 1) * CW)
                pca, pcb = pcs[c]
                ssq = ps_ss.tile([65, CW], F32, tag="ss", name="ss")
                for i, pc in enumerate([pca, pcb]):
                    sq = sbs.tile([128, CW], F16, tag="sq", name="sq")
                    nc.vector.tensor_mul(sq[:], pc[:], pc[:])
                    nc.tensor.matmul(ssq[:], sel65, sq[:], start=(i == 0), stop=(i == 1))
                ra = rsp.tile([128, CW], F16, tag="ra", name="ra")
                rb = rsp.tile([128, CW], F16, tag="rb", name="rb")
                rope(pca[:], pcb[:], ra[:], rb[:], cs)
                stmp0 = sbs.tile([1, CW], F32, tag="stmp0", name="stmp0")
                stmp1 = sbs.tile([1, CW], F32, tag="stmp1", name="stmp1")
                nc.scalar.activation(stmp0[:], ssq[0:1, :],
                                     mybir.ActivationFunctionType.Sqrt,
                                     bias=biasc[0:1, 1:2], scale=float(1.0 / H))
                nc.scalar.activation(stmp1[:], ssq[64:65, :],
                                     mybir.ActivationFunctionType.Sqrt,
                                     bias=biasc[0:1, 1:2], scale=float(1.0 / H))
                rstd0 = sbs.tile([1, CW], F16, tag="rstd0", name="rstd0")
                rstd1 = sbs.tile([1, CW], F16, tag="rstd1", name="rstd1")
                nc.vector.reciprocal(rstd0[:], stmp0[:])
                nc.vector.reciprocal(rstd1[:], stmp1[:])
                bca = sbs.tile([128, CW], F16, tag="bca", name="bca")
                bcb = sbs.tile([128, CW], F16, tag="bcb", name="bcb")
                nc.gpsimd.partition_broadcast(bca[:], rstd0[:], channels=128)
                nc.gpsimd.partition_broadcast(bcb[:], rstd1[:], channels=128)
                tbs = slice(4 * c, 4 * c + 4)
                r3 = lambda ap: ap.rearrange("p (a t) -> p a t", a=4)
                nc.vector.tensor_mul(qhv[0:64, tbs, ga, :], r3(ra[0:64, :]),
                                     r3(bca[0:64, :]))
                nc.vector.tensor_mul(qhv[0:64, tbs, gb, :], r3(ra[64:128, :]),
                                     r3(bcb[64:128, :]))
                nc.vector.tensor_mul(qhv[64:128, tbs, ga, :], r3(rb[0:64, :]),
                                     r3(bca[0:64, :]))
                nc.vector.tensor_mul(qhv[64:128, tbs, gb, :], r3(rb[64:128, :]),
                                     r3(bcb[64:128, :]))

        # ---------------- k ----------------
        for c in range(NCHUNK):
            cs = slice(c * CW, (c + 1) * CW)
            psk = psproj.tile([128, CW], F32, tag="proj", name="proj")
            for d_i in range(ND):
                nc.tensor.matmul(psk[:], wk_sb[:, d_i * 128:(d_i + 1) * 128],
                                 xt_ap(d_i)[:, cs],
                                 start=(d_i == 0), stop=(d_i == ND - 1))
            pck = sbs.tile([128, CW], F16, tag="pck", name="pck")
            nc.scalar.copy(pck[:], psk[:])
            sqk = sbs.tile([128, CW], F16, tag="sqk", name="sqk")
            nc.vector.tensor_mul(sqk[:], pck[:], pck[:])
            # row-form sumsq -> sexp = 1/sqrt(sumsq + H*eps) = SCALE*rstd_k
            kssr = ps_ss.tile([1, CW], F32, tag="kssr", name="kssr")
            nc.tensor.matmul(kssr[:], ones1, sqk[:], start=True, stop=True)
            ktmp = sbs.tile([1, CW], F32, tag="ktmp", name="ktmp")
            nc.scalar.activation(ktmp[:], kssr[:],
                                 mybir.ActivationFunctionType.Sqrt,
                                 bias=biasc[0:1, 0:1], scale=1.0)
            krst = sbs.tile([1, CW], F16, tag="krst", name="krst")
            nc.vector.reciprocal(krst[:], ktmp[:])
            last_ktmp = ktmp
            m1 = sbs.tile([128, CW], F16, tag="m1", name="m1")
            m2 = sbs.tile([128, CW], F16, tag="m2", name="m2")
            k0, k1 = pck[0:64, :], pck[64:128, :]
            nc.vector.tensor_mul(m1[0:64, :], k0, ckt[0:64, cs])
            nc.vector.tensor_mul(m2[0:64, :], k1, skt[64:128, cs])
            nc.vector.tensor_sub(kTn[0:64, cs], m1[0:64, :], m2[0:64, :])
            nc.vector.tensor_mul(m1[0:64, :], k1, ckt[64:128, cs])
            nc.vector.tensor_mul(m2[0:64, :], k0, skt[0:64, cs])
            nc.vector.tensor_add(kTn[64:128, cs], m1[0:64, :], m2[0:64, :])
            # fold SCALE*rstd_k into this chunk of kTn
            kbcc = sbs.tile([128, CW], F16, tag="kbcc", name="kbcc")
            nc.gpsimd.partition_broadcast(kbcc[:], krst[:], channels=128)
            nc.vector.tensor_mul(kTn[:, cs], kTn[:, cs], kbcc[:])

        # switch the Act table to the exp set now, off the attention
        # critical path; reading the last sqrt output forces the scheduler
        # to place this after every sqrt-set activation
        nc.scalar.activation(warm[:], last_ktmp[0:1, 0:2],
                             mybir.ActivationFunctionType.Exp,
                             bias=expb[0:1, 0:1], scale=-1.0)

        # ---------------- v: VT projection + PE transposes ----------------
        vts = {}
        for c in range(NCHUNK):
            cs = slice(c * CW, (c + 1) * CW)
            psv = psproj.tile([128, CW], F32, tag="proj", name="proj")
            for d_i in range(ND):
                nc.tensor.matmul(psv[:], wv_sb[:, d_i * 128:(d_i + 1) * 128],
                                 xt_ap(d_i)[:, cs],
                                 start=(d_i == 0), stop=(d_i == ND - 1))
            vt_c = vtp.tile([128, CW], F16, tag=f"vt{c}", name=f"vt{c}")
            nc.scalar.copy(vt_c[:], psv[:])
            vts[c] = vt_c
            for jj in range(4):
                j = 4 * c + jj
                vp = ps_v.tile([128, 128], F16, tag="pv", name="pv")
                nc.tensor.transpose(vp[:], vt_c[:, jj * 128:(jj + 1) * 128], iden)
                if j % 2 == 0:
                    nc.scalar.copy(V[:, j * 128:(j + 1) * 128], vp[:])
                else:
                    nc.vector.tensor_copy(V[:, j * 128:(j + 1) * 128], vp[:])

        es1.close()
        ps1.close()

        # ---------------- attention + out-projection per t-block ----------------
        sbP = pool(name="sbP", bufs=5)
        sbD = pool(name="sbD", bufs=4)
        sbx = pool(name="sbx", bufs=3)
        osp = pool(name="outs", bufs=3)
        ps_lg = pool(name="ps_lg", bufs=4, space="PSUM")
        ps_qkv = pool(name="ps_qkv", bufs=1, space="PSUM")
        ps_op = pool(name="ps_op", bufs=2, space="PSUM")

        moff = []
        off = 0
        for tb in range(NTB):
            moff.append(off)
            off += len(plan[tb])

        from concourse import bass_isa

        def outproj_dc(tb, qkvh, ob, dc, flush):
            t0 = tb * 128
            op = ps_op.tile([128, CW], F32, tag="op", name="op")
            for g in range(G):
                nc.tensor.matmul(op[:],
                                 qkvh[:, g * 128:(g + 1) * 128],
                                 wo_sb[g][:, dc * CW:(dc + 1) * CW],
                                 start=(g == 0), stop=(g == G - 1))
            if dc % 2 == 0:
                nc.vector.tensor_copy(ob[:, dc * CW:(dc + 1) * CW], op[:])
            else:
                nc.scalar.copy(ob[:, dc * CW:(dc + 1) * CW], op[:])
            if flush:
                nc.sync.dma_start(
                    out_d[t0:t0 + 128, dc * CW:(dc + 1) * CW],
                    ob[:, dc * CW:(dc + 1) * CW])
            elif dc == 3:
                nc.sync.dma_start(out_d[t0:t0 + 128, :], ob[:])

        tb_order = sorted(range(NTB), key=lambda t: -len(plan[t]))
        prev = None
        for tb in tb_order:
            ent = plan[tb]
            nv = len(ent)
            t0 = tb * 128
            qkv_a = ps_qkv.tile([128, 256], F32, tag="qkva", name="qkva")
            qkv_b = ps_qkv.tile([128, 256], F32, tag="qkvb", name="qkvb")
            qkvh = sbx.tile([128, 512], F16, tag="qkvh", name="qkvh")
            ob = osp.tile([128, D], F16, tag="ob", name="ob")
            acc = sbD.tile([128, 512], F16, tag="dfold", name="dfold")

            Ps = {}
            dbcs = {}
            ndc = 0

            def softmax_si(i, si):
                """one 512-wide logits matmul for all 4 heads, exp, mask,
                and the Pool denominator reduction for s-block si"""
                lg = ps_lg.tile([128, 512], F32, tag="lg", name="lg")
                nc.tensor.matmul(lg[:], kTn[:, si * 128:(si + 1) * 128],
                                 qhp[:, tb * 512:(tb + 1) * 512],
                                 start=True, stop=True)
                P = sbP.tile([128, 512], F16, tag="P", name="P")
                nc.scalar.activation(P[:], lg[:],
                                     mybir.ActivationFunctionType.Exp,
                                     bias=expb[:, 0:1], scale=1.0)
                nc.vector.tensor_mul(P[:], P[:],
                                     msk[:, (moff[tb] + i) * 512:
                                          (moff[tb] + i + 1) * 512])
                dbc = sbD.tile([128, 512], F16, tag="dbc", name="dbc")
                nc.gpsimd.partition_all_reduce(dbc[:], P[:], channels=128,
                                               reduce_op=bass_isa.ReduceOp.add)
                dbcs[i] = dbc
                if i == 1:
                    nc.vector.tensor_add(acc[:], dbcs[0][:], dbc[:])
                elif i > 1:
                    nc.vector.tensor_add(acc[:], acc[:], dbc[:])
                return P

            for i, si in enumerate(ent):
                Ps[i] = softmax_si(i, si)
                if prev is not None and ndc < 4:
                    outproj_dc(prev[0], prev[1], prev[2], ndc, False)
                    ndc += 1
                # heads 0 and 2 accumulate si-pipelined, each contiguous
                # within its own PSUM bank
                nc.tensor.matmul(qkv_a[:, 0:128],
                                 V[:, si * 128:(si + 1) * 128],
                                 Ps[i][:, 0:128],
                                 start=(i == 0), stop=(i == nv - 1))
                nc.tensor.matmul(qkv_b[:, 0:128],
                                 V[:, si * 128:(si + 1) * 128],
                                 Ps[i][:, 256:384],
                                 start=(i == 0), stop=(i == nv - 1))
            if prev is not None:
                while ndc < 4:
                    outproj_dc(prev[0], prev[1], prev[2], ndc, False)
                    ndc += 1
            for i, si in enumerate(ent):
                nc.tensor.matmul(qkv_a[:, 128:256],
                                 V[:, si * 128:(si + 1) * 128],
                                 Ps[i][:, 128:256],
                                 start=(i == 0), stop=(i == nv - 1))
            for i, si in enumerate(ent):
                nc.tensor.matmul(qkv_b[:, 128:256],
                                 V[:, si * 128:(si + 1) * 128],
                                 Ps[i][:, 384:512],
                                 start=(i == 0), stop=(i == nv - 1))
            Ps.clear()

            rec = sbD.tile([128, 512], F16, tag="recg", name="recg")
            nc.vector.reciprocal(rec[:], acc[:] if nv > 1 else dbcs[0][:])
            nc.vector.tensor_mul(qkvh[:, 0:128], qkv_a[:, 0:128], rec[:, 0:128])
            nc.vector.tensor_mul(qkvh[:, 128:256], qkv_a[:, 128:256], rec[:, 128:256])
            nc.vector.tensor_mul(qkvh[:, 256:384], qkv_b[:, 0:128], rec[:, 256:384])
            nc.vector.tensor_mul(qkvh[:, 384:512], qkv_b[:, 128:256], rec[:, 384:512])
            prev = (tb, qkvh, ob)
        for dc in range(4):
            outproj_dc(prev[0], prev[1], prev[2], dc, True)

    nc.finalize()
    return nc


_CACHE = {}


def kernel(x, segment_ids, Wq, Wk, Wv, Wo, q_scale, k_scale):
    global LAST_RESULTS
    import os

    x = np.asarray(x, np.float32)
    seg = np.asarray(segment_ids)
    Wq = np.asarray(Wq, np.float32)
    Wk = np.asarray(Wk, np.float32)
    Wv = np.asarray(Wv, np.float32)
    Wo = np.asarray(Wo, np.float32)
    q_scale = np.asarray(q_scale, np.float32)
    k_scale = np.asarray(k_scale, np.float32)

    plan, masks = _classify([seg[b] for b in range(B)])
    key = repr(plan)
    if key not in _CACHE:
        _CACHE[key] = _build_nc(plan, masks[0].shape[1])
    nc = _CACHE[key]

    half = H // 2
    timescale = ROPE_BASE ** (2.0 * np.arange(half, dtype=np.float64) / H)
    qscA = np.tile(q_scale[:64], 2).astype(np.float64)[:, None]
    qscB = np.tile(q_scale[64:], 2).astype(np.float64)[:, None]
    kvec = k_scale.astype(np.float64)[:, None]
    tabs = []  # per batch: (cqa, sqa, cqb, sqb, ckt, skt)
    for b in range(B):
        pos = _positions(seg[b])
        sinus = pos[:, None] / timescale[None, :]
        sT = np.sin(sinus).T
        cT = np.cos(sinus).T
        c2 = np.vstack([cT, cT])
        s2 = np.vstack([sT, sT])
        tabs.append(tuple(
            np.ascontiguousarray(a, np.float16)
            for a in (c2 * qscA, s2 * qscA, c2 * qscB, s2 * qscB,
                      c2 * kvec, s2 * kvec)))

    tblf = np.zeros((128, 7), np.float32)
    tblf[:, 0] = np.tile(q_scale[:64], 2)
    tblf[:, 1] = np.tile(q_scale[64:], 2)
    tblf[0:64, 2] = k_scale[:64]
    tblf[64:128, 3] = k_scale[64:]
    tblf[:, 4] = H * EPS
    tblf[:, 5] = EPS
    tblf[:, 6] = EXPB
    tblh = np.zeros((128, 194), np.float16)
    tblh[0:64, 0] = 1.0
    tblh[64:128, 64] = 1.0
    tblh[:, 65] = 1.0
    tblh[:, 66:194] = np.eye(128, dtype=np.float16)

    in_maps = []
    for core in range(8):
        b, kv = core // K, core % K
        qcols = []
        for hv in range(2):
            for g4 in range(G):
                base = kv * 512 + g4 * 128 + hv * 64
                qcols.extend(range(base, base + 64))
        qp = np.array(qcols)
        wq_t = np.ascontiguousarray(
            Wq[:, qp].reshape(ND, 128, G, 128).transpose(2, 1, 0, 3)
            .reshape(G, 128, ND * 128), np.float16)
        wk_t = np.ascontiguousarray(
            Wk[:, kv * 128:(kv + 1) * 128].reshape(ND, 128, 128)
            .transpose(1, 0, 2).reshape(128, ND * 128), np.float16)
        wv_t = np.ascontiguousarray(
            Wv[:, kv * 128:(kv + 1) * 128].reshape(ND, 128, 128)
            .transpose(1, 0, 2).reshape(128, ND * 128), np.float16)
        wo_t = np.ascontiguousarray(
            Wo[kv * 512:(kv + 1) * 512].reshape(G, 128, D), np.float16)
        xt_t = np.ascontiguousarray(
            x[b].T.reshape(ND, 128, T), np.float16)
        cqa, sqa, cqb, sqb, ckt, skt = tabs[b]
        in_maps.append({
            "xT": xt_t, "wq": wq_t, "wk": wk_t, "wv": wv_t, "wo": wo_t,
            "cqa": cqa, "sqa": sqa, "cqb": cqb, "sqb": sqb,
            "ckt": ckt, "skt": skt,
            "tblf": tblf, "tblh": tblh, "masks": masks[b],
        })

    do_trace = os.environ.get("BASS_TRACE") == "1"
    res = run_bass_kernel_spmd(
        nc, in_maps, core_ids=list(range(8)), trace=do_trace)
    LAST_RESULTS = res

    out = np.zeros((B, T, D), np.float32)
    for core in range(8):
        out[core // K] += res.results[core]["out"].astype(np.float32)
    return out


# revision 17
# speedup vs baseline: 1.5858x; 1.0099x over previous
"""Trainium2 Bass kernel v2 for segment-causal GQA attention.

Sharding: 8 cores = batch (2) x kv-head (4); host sums the 4 row-parallel
Wo partial outputs per batch.  All device compute in fp16 (1 PE cycle/row
at any moving width, 2-byte DVE fast modes, half the DMA bytes of fp32).

Layout per core (T=1024, D=2048, H=128, G=4 q-heads):
  xt      [128, 16*1024]  x[b]^T d-tiles side by side (4 DMA'd groups)
  qh[g]   [128, T]   rope'd, rstd-scaled q per head (transposed)
  kTn     [128, T]   rope'd k, with SCALE*rstd_k folded in per-column
  V       [128, 8*128]  v in [s,h] layout per 128-s-block (direct proj)
  attention: per 128-wide t-block tb, the <=4 valid s-blocks' logits are
  packed into one PSUM bank [128, nv*128]; one exp (bias=-4 keeps P in
  fp16 range without max-subtraction), one packed mask multiply, per-
  block qkv/den accumulation; the out-projection of each tb (4x4
  matmuls into [128t, 512d] psums) interleaves with the next tb's
  softmax work to keep the PE saturated.
"""

import sys

sys.path.insert(0, "/opt/trn_rl_repo")

import numpy as np

import concourse.bacc as bacc
import concourse.bass as bass  # noqa: F401
import concourse.tile as tile
from concourse import mybir
from concourse.bass_utils import run_bass_kernel_spmd

B, T, D = 2, 1024, 2048
N, K, H = 16, 4, 128
G = N // K
EPS = 1e-6
SCALE = H ** -0.5
ROPE_BASE = 10000.0
NCHUNK = 2
CW = T // NCHUNK        # 512
NTB = T // 128          # 8 t-blocks (and s-blocks)
ND = D // 128           # 16
F32 = mybir.dt.float32
F16 = mybir.dt.float16
MULT = mybir.AluOpType.mult
EXPB = -4.0             # exp bias: keeps P in fp16 range without max-sub

LAST_RESULTS = None


def _positions(seg):
    t = seg.shape[0]
    idx = np.arange(t, dtype=np.int64)
    is_start = np.concatenate([[True], seg[1:] != seg[:-1]])
    seg_start = np.maximum.accumulate(np.where(is_start, idx, 0))
    return (idx - seg_start).astype(np.float64)


def _classify(seg_rows):
    """Union-over-batches 128x128 block plan.

    Returns (plan, masks): plan[tb] = list of valid s-block indices;
    masks[b] = fp16 [128, n_blocks*128] 0/1 pack in plan order.
    """
    idx = np.arange(T)
    valids = []
    for b in range(B):
        seg = seg_rows[b]
        valids.append((seg[:, None] == seg[None, :]) & (idx[:, None] <= idx[None, :]))
    plan = []
    packs = [[] for _ in range(B)]
    for tb in range(NTB):
        t0 = tb * 128
        ent = []
        for si in range(NTB):
            s0 = si * 128
            subs = [v[s0:s0 + 128, t0:t0 + 128] for v in valids]
            if any(s.any() for s in subs):
                ent.append(si)
                for b in range(B):
                    packs[b].append(subs[b])
        plan.append(ent)
    masks = []
    for b in range(B):
        if packs[b]:
            m = np.concatenate([np.tile(p, (1, 4)) for p in packs[b]], axis=1)
        else:
            m = np.zeros((128, 512), bool)
        masks.append(np.ascontiguousarray(m.astype(np.float16)))
    return plan, masks


def _build_nc(plan, n_mask_cols):
    from contextlib import ExitStack

    nc = bacc.Bacc(None, target_bir_lowering=False, debug=False)
    xT_d = nc.dram_tensor("xT", [ND, 128, T], F16, kind="ExternalInput")
    wq_d = nc.dram_tensor("wq", [G, 128, ND * 128], F16, kind="ExternalInput")
    wk_d = nc.dram_tensor("wk", [128, ND * 128], F16, kind="ExternalInput")
    wv_d = nc.dram_tensor("wv", [128, ND * 128], F16, kind="ExternalInput")
    wo_d = nc.dram_tensor("wo", [G, 128, D], F16, kind="ExternalInput")
    # prescaled rope tables: cos/sin x per-partition rms-scale columns
    cqa_d = nc.dram_tensor("cqa", [128, T], F16, kind="ExternalInput")
    sqa_d = nc.dram_tensor("sqa", [128, T], F16, kind="ExternalInput")
    cqb_d = nc.dram_tensor("cqb", [128, T], F16, kind="ExternalInput")
    sqb_d = nc.dram_tensor("sqb", [128, T], F16, kind="ExternalInput")
    ckt_d = nc.dram_tensor("ckt", [128, T], F16, kind="ExternalInput")
    skt_d = nc.dram_tensor("skt", [128, T], F16, kind="ExternalInput")
    tblf_d = nc.dram_tensor("tblf", [128, 7], F32, kind="ExternalInput")
    tblh_d = nc.dram_tensor("tblh", [128, 194], F16, kind="ExternalInput")
    msk_d = nc.dram_tensor("masks", [128, n_mask_cols], F16, kind="ExternalInput")
    out_d = nc.dram_tensor("out", [T, D], F16, kind="ExternalOutput")

    es = ExitStack()
    with es:
        es.enter_context(nc.allow_low_precision("fp16 kernel"))
        tc = es.enter_context(tile.TileContext(nc))
        pool = lambda *a, **k: es.enter_context(tc.tile_pool(*a, **k))
        pp = pool(name="persist", bufs=1)

        # ---------------- persistent tiles ----------------
        xt = pp.tile([128, ND * T], F16, tag="xt", name="xt")  # 4MB
        qhp = pp.tile([128, NTB * G * 128], F16, tag="qhp", name="qhp")
        qhv = qhp[:].rearrange("p (a g t) -> p a g t", a=NTB, g=G)
        kTn = pp.tile([128, T], F16, tag="kTn", name="kTn")
        V = pp.tile([128, NTB * 128], F16, tag="V", name="V")
        wqs = [pp.tile([128, ND * 128], F16, tag=f"wq{g}", name=f"wq{g}")
               for g in range(G)]
        wk_sb = pp.tile([128, ND * 128], F16, tag="wk", name="wk")
        wv_sb = pp.tile([128, ND * 128], F16, tag="wv", name="wv")
        wo_sb = [pp.tile([128, D], F16, tag=f"wo{g}", name=f"wo{g}")
                 for g in range(G)]
        cqa = pp.tile([128, T], F16, tag="cqa", name="cqa")
        sqa = pp.tile([128, T], F16, tag="sqa", name="sqa")
        cqb = pp.tile([128, T], F16, tag="cqb", name="cqb")
        sqb = pp.tile([128, T], F16, tag="sqb", name="sqb")
        ckt = pp.tile([128, T], F16, tag="ckt", name="ckt")
        skt = pp.tile([128, T], F16, tag="skt", name="skt")
        tblf = pp.tile([128, 7], F32, tag="tblf", name="tblf")
        tblh = pp.tile([128, 194], F16, tag="tblh", name="tblh")
        msk = pp.tile([128, n_mask_cols], F16, tag="msk", name="msk")

        qsc = tblf[:, 0:2]       # f32 per-partition scalars
        ksc = tblf[:, 2:4]
        biasc = tblf[:, 4:6]     # [:,0]=H*EPS  [:,1]=EPS
        expb = tblf[:, 6:7]      # exp bias column (EXPB)
        sel65 = tblh[:, 0:65]    # half-selector cols at 0 and 64
        ones1 = tblh[:, 65:66]
        iden = tblh[:, 66:194]   # fp16 identity

        # ---------------- DMA issue (consume order) ----------------
        def xt_ap(d):
            return xt[:, d * T:(d + 1) * T]

        xtv = xt[:].rearrange("p (a t) -> p a t", a=ND)
        # startup splits: first 2 d-tiles of x and first 2 d-cols of wqA
        nc.sync.dma_start(wqs[0][:, 0:768], wq_d[0][:, 0:768])
        nc.sync.dma_start(wqs[2][:, 0:768], wq_d[2][:, 0:768])
        nc.sync.dma_start(xtv[:, 0:2, :], xT_d[0:2].transpose([1, 0, 2]))
        nc.sync.dma_start(xtv[:, 2:4, :], xT_d[2:4].transpose([1, 0, 2]))
        nc.sync.dma_start(tblf[:], tblf_d[:])
        nc.sync.dma_start(tblh[:], tblh_d[:])
        nc.sync.dma_start(xtv[:, 4:6, :], xT_d[4:6].transpose([1, 0, 2]))
        wq_lo = 768
        for i in range(3, 8):
            wq_hi = min(2048, wq_lo + 256)
            if wq_lo < 2048:
                nc.sync.dma_start(wqs[0][:, wq_lo:wq_hi], wq_d[0][:, wq_lo:wq_hi])
                nc.sync.dma_start(wqs[2][:, wq_lo:wq_hi], wq_d[2][:, wq_lo:wq_hi])
                wq_lo = wq_hi
            nc.sync.dma_start(xtv[:, 2 * i:2 * i + 2, :],
                              xT_d[2 * i:2 * i + 2].transpose([1, 0, 2]))
        nc.sync.dma_start(wqs[1][:], wq_d[1])
        nc.sync.dma_start(wqs[3][:], wq_d[3])
        nc.sync.dma_start(cqa[:], cqa_d[:])
        nc.sync.dma_start(sqa[:], sqa_d[:])
        nc.sync.dma_start(cqb[:], cqb_d[:])
        nc.sync.dma_start(sqb[:], sqb_d[:])
        nc.sync.dma_start(wk_sb[:], wk_d[:])
        nc.sync.dma_start(ckt[:], ckt_d[:])
        nc.sync.dma_start(skt[:], skt_d[:])
        nc.sync.dma_start(wv_sb[:], wv_d[:])
        nc.sync.dma_start(msk[:], msk_d[:])
        for g in range(G):
            nc.sync.dma_start(wo_sb[g][:], wo_d[g])

        # ---------------- phase-1 pools ----------------
        es1 = ExitStack()
        pool1 = lambda *a, **k: es1.enter_context(tc.tile_pool(*a, **k))
        sbs = pool1(name="sb_stream", bufs=4)
        rsp = pool1(name="ropes", bufs=3)
        vtp = pool1(name="vtp", bufs=1)
        ps1 = ExitStack()
        psproj = ps1.enter_context(tc.tile_pool(name="ps_proj", bufs=4, space="PSUM"))
        ps_ss = ps1.enter_context(tc.tile_pool(name="ps_ss", bufs=1, space="PSUM"))
        ps_v = ps1.enter_context(tc.tile_pool(name="ps_v", bufs=2, space="PSUM"))

        def project4(wa, wb):
            """d-outer accumulation: psums[(fi, c)] = [128, CW] f32."""
            pss = {(fi, c): psproj.tile([128, CW], F32, tag="proj", name="proj")
                   for fi in range(2) for c in range(NCHUNK)}
            for d_i in range(ND):
                for fi, w in enumerate((wa, wb)):
                    for c in range(NCHUNK):
                        nc.tensor.matmul(
                            pss[(fi, c)][:],
                            w[:, d_i * 128:(d_i + 1) * 128],
                            xt_ap(d_i)[:, c * CW:(c + 1) * CW],
                            start=(d_i == 0), stop=(d_i == ND - 1))
            return pss

        def rope(psa, psb, out_a, out_b, cs):
            m1 = sbs.tile([128, CW], F16, tag="m1", name="m1")
            m2 = sbs.tile([128, CW], F16, tag="m2", name="m2")
            nc.vector.tensor_mul(m1[:], psa, cqa[:, cs])
            nc.vector.tensor_mul(m2[:], psb, sqb[:, cs])
            nc.vector.tensor_sub(out_a, m1[:], m2[:])
            nc.vector.tensor_mul(m1[:], psb, cqb[:, cs])
            nc.vector.tensor_mul(m2[:], psa, sqa[:, cs])
            nc.vector.tensor_add(out_b, m1[:], m2[:])

        # warm the Exp activation table early so the load is off the
        # attention critical path
        warm = sbs.tile([1, 2], F16, tag="warm", name="warm")

        # ---------------- q pairs ----------------
        for pi in range(2):
            wa, wb = (wqs[0], wqs[2]) if pi == 0 else (wqs[1], wqs[3])
            ga, gb = (0, 1) if pi == 0 else (2, 3)
            pss = project4(wa, wb)
            pcs = {}
            for c in range(NCHUNK):
                pca = sbs.tile([128, CW], F16, tag="pca", name="pca")
                pcb = sbs.tile([128, CW], F16, tag="pcb", name="pcb")
                nc.scalar.copy(pca[:], pss[(0, c)][:])
                nc.vector.tensor_copy(pcb[:], pss[(1, c)][:])
                pcs[c] = (pca, pcb)
            for c in range(NCHUNK):
                cs = slice(c * CW, (c + 1) * CW)
                pca, pcb = pcs[c]
                ssq = ps_ss.tile([65, CW], F32, tag="ss", name="ss")
                for i, pc in enumerate([pca, pcb]):
                    sq = sbs.tile([128, CW], F16, tag="sq", name="sq")
                    nc.vector.tensor_mul(sq[:], pc[:], pc[:])
                    nc.tensor.matmul(ssq[:], sel65, sq[:], start=(i == 0), stop=(i == 1))
                ra = rsp.tile([128, CW], F16, tag="ra", name="ra")
                rb = rsp.tile([128, CW], F16, tag="rb", name="rb")
                rope(pca[:], pcb[:], ra[:], rb[:], cs)
                stmp0 = sbs.tile([1, CW], F32, tag="stmp0", name="stmp0")
                stmp1 = sbs.tile([1, CW], F32, tag="stmp1", name="stmp1")
                nc.scalar.activation(stmp0[:], ssq[0:1, :],
                                     mybir.ActivationFunctionType.Sqrt,
                                     bias=biasc[0:1, 1:2], scale=float(1.0 / H))
                nc.scalar.activation(stmp1[:], ssq[64:65, :],
                                     mybir.ActivationFunctionType.Sqrt,
                                     bias=biasc[0:1, 1:2], scale=float(1.0 / H))
                rstd0 = sbs.tile([1, CW], F16, tag="rstd0", name="rstd0")
                rstd1 = sbs.tile([1, CW], F16, tag="rstd1", name="rstd1")
                nc.vector.reciprocal(rstd0[:], stmp0[:])
                nc.vector.reciprocal(rstd1[:], stmp1[:])
                bca = sbs.tile([128, CW], F16, tag="bca", name="bca")
                bcb = sbs.tile([128, CW], F16, tag="bcb", name="bcb")
                nc.gpsimd.partition_broadcast(bca[:], rstd0[:], channels=128)
                nc.gpsimd.partition_broadcast(bcb[:], rstd1[:], channels=128)
                tbs = slice(4 * c, 4 * c + 4)
                r3 = lambda ap: ap.rearrange("p (a t) -> p a t", a=4)
                nc.vector.tensor_mul(qhv[0:64, tbs, ga, :], r3(ra[0:64, :]),
                                     r3(bca[0:64, :]))
                nc.vector.tensor_mul(qhv[0:64, tbs, gb, :], r3(ra[64:128, :]),
                                     r3(bcb[64:128, :]))
                nc.vector.tensor_mul(qhv[64:128, tbs, ga, :], r3(rb[0:64, :]),
                                     r3(bca[0:64, :]))
                nc.vector.tensor_mul(qhv[64:128, tbs, gb, :], r3(rb[64:128, :]),
                                     r3(bcb[64:128, :]))

        # ---------------- k ----------------
        for c in range(NCHUNK):
            cs = slice(c * CW, (c + 1) * CW)
            psk = psproj.tile([128, CW], F32, tag="proj", name="proj")
            for d_i in range(ND):
                nc.tensor.matmul(psk[:], wk_sb[:, d_i * 128:(d_i + 1) * 128],
                                 xt_ap(d_i)[:, cs],
                                 start=(d_i == 0), stop=(d_i == ND - 1))
            pck = sbs.tile([128, CW], F16, tag="pck", name="pck")
            nc.scalar.copy(pck[:], psk[:])
            sqk = sbs.tile([128, CW], F16, tag="sqk", name="sqk")
            nc.vector.tensor_mul(sqk[:], pck[:], pck[:])
            # row-form sumsq -> sexp = 1/sqrt(sumsq + H*eps) = SCALE*rstd_k
            kssr = ps_ss.tile([1, CW], F32, tag="kssr", name="kssr")
            nc.tensor.matmul(kssr[:], ones1, sqk[:], start=True, stop=True)
            ktmp = sbs.tile([1, CW], F32, tag="ktmp", name="ktmp")
            nc.scalar.activation(ktmp[:], kssr[:],
                                 mybir.ActivationFunctionType.Sqrt,
                                 bias=biasc[0:1, 0:1], scale=1.0)
            krst = sbs.tile([1, CW], F16, tag="krst", name="krst")
            nc.vector.reciprocal(krst[:], ktmp[:])
            last_ktmp = ktmp
            m1 = sbs.tile([128, CW], F16, tag="m1", name="m1")
            m2 = sbs.tile([128, CW], F16, tag="m2", name="m2")
            k0, k1 = pck[0:64, :], pck[64:128, :]
            nc.vector.tensor_mul(m1[0:64, :], k0, ckt[0:64, cs])
            nc.vector.tensor_mul(m2[0:64, :], k1, skt[64:128, cs])
            nc.vector.tensor_sub(kTn[0:64, cs], m1[0:64, :], m2[0:64, :])
            nc.vector.tensor_mul(m1[0:64, :], k1, ckt[64:128, cs])
            nc.vector.tensor_mul(m2[0:64, :], k0, skt[0:64, cs])
            nc.vector.tensor_add(kTn[64:128, cs], m1[0:64, :], m2[0:64, :])
            # fold SCALE*rstd_k into this chunk of kTn
            kbcc = sbs.tile([128, CW], F16, tag="kbcc", name="kbcc")
            nc.gpsimd.partition_broadcast(kbcc[:], krst[:], channels=128)
            nc.vector.tensor_mul(kTn[:, cs], kTn[:, cs], kbcc[:])

        # switch the Act table to the exp set now, off the attention
        # critical path; reading the last sqrt output forces the scheduler
        # to place this after every sqrt-set activation
        nc.scalar.activation(warm[:], last_ktmp[0:1, 0:2],
                             mybir.ActivationFunctionType.Exp,
                             bias=expb[0:1, 0:1], scale=-1.0)

        # ---------------- v: VT projection + PE transposes ----------------
        vts = {}
        for c in range(NCHUNK):
            cs = slice(c * CW, (c + 1) * CW)
            psv = psproj.tile([128, CW], F32, tag="proj", name="proj")
            for d_i in range(ND):
                nc.tensor.matmul(psv[:], wv_sb[:, d_i * 128:(d_i + 1) * 128],
                                 xt_ap(d_i)[:, cs],
                                 start=(d_i == 0), stop=(d_i == ND - 1))
            vt_c = vtp.tile([128, CW], F16, tag=f"vt{c}", name=f"vt{c}")
            nc.scalar.copy(vt_c[:], psv[:])
            vts[c] = vt_c
            for jj in range(4):
                j = 4 * c + jj
                vp = ps_v.tile([128, 128], F16, tag="pv", name="pv")
                nc.tensor.transpose(vp[:], vt_c[:, jj * 128:(jj + 1) * 128], iden)
                if j % 2 == 0:
                    nc.scalar.copy(V[:, j * 128:(j + 1) * 128], vp[:])
                else:
                    nc.vector.tensor_copy(V[:, j * 128:(j + 1) * 128], vp[:])

        es1.close()
        ps1.close()

        # ---------------- attention + out-projection per t-block ----------------
        sbP = pool(name="sbP", bufs=5)
        sbD = pool(name="sbD", bufs=4)
        sbx = pool(name="sbx", bufs=3)
        osp = pool(name="outs", bufs=3)
        ps_lg = pool(name="ps_lg", bufs=4, space="PSUM")
        ps_qkv = pool(name="ps_qkv", bufs=1, space="PSUM")
        ps_op = pool(name="ps_op", bufs=2, space="PSUM")

        moff = []
        off = 0
        for tb in range(NTB):
            moff.append(off)
            off += len(plan[tb])

        from concourse import bass_isa

        def outproj_dc(tb, qkvh, ob, dc, flush):
            t0 = tb * 128
            op = ps_op.tile([128, CW], F32, tag="op", name="op")
            for g in range(G):
                nc.tensor.matmul(op[:],
                                 qkvh[:, g * 128:(g + 1) * 128],
                                 wo_sb[g][:, dc * CW:(dc + 1) * CW],
                                 start=(g == 0), stop=(g == G - 1))
            if dc % 2 == 0:
                nc.vector.tensor_copy(ob[:, dc * CW:(dc + 1) * CW], op[:])
            else:
                nc.scalar.copy(ob[:, dc * CW:(dc + 1) * CW], op[:])
            if flush:
                nc.sync.dma_start(
                    out_d[t0:t0 + 128, dc * CW:(dc + 1) * CW],
                    ob[:, dc * CW:(dc + 1) * CW])
            elif dc == 3:
                nc.sync.dma_start(out_d[t0:t0 + 128, :], ob[:])

        tb_order = sorted(range(NTB), key=lambda t: -len(plan[t]))
        prev = None
        for tb in tb_order:
            ent = plan[tb]
            nv = len(ent)
            t0 = tb * 128
            qkv_a = ps_qkv.tile([128, 256], F32, tag="qkva", name="qkva")
            qkv_b = ps_qkv.tile([128, 256], F32, tag="qkvb", name="qkvb")
            qkvh = sbx.tile([128, 512], F16, tag="qkvh", name="qkvh")
            ob = osp.tile([128, D], F16, tag="ob", name="ob")
            acc = sbD.tile([128, 512], F16, tag="dfold", name="dfold")

            Ps = {}
            dbcs = {}
            ndc = 0

            def softmax_si(i, si):
                """one 512-wide logits matmul for all 4 heads, exp, mask,
                and the Pool denominator reduction for s-block si"""
                lg = ps_lg.tile([128, 512], F32, tag="lg", name="lg")
                nc.tensor.matmul(lg[:], kTn[:, si * 128:(si + 1) * 128],
                                 qhp[:, tb * 512:(tb + 1) * 512],
                                 start=True, stop=True)
                P = sbP.tile([128, 512], F16, tag="P", name="P")
                nc.scalar.activation(P[:], lg[:],
                                     mybir.ActivationFunctionType.Exp,
                                     bias=expb[:, 0:1], scale=1.0)
                nc.vector.tensor_mul(P[:], P[:],
                                     msk[:, (moff[tb] + i) * 512:
                                          (moff[tb] + i + 1) * 512])
                dbc = sbD.tile([128, 512], F16, tag="dbc", name="dbc")
                nc.gpsimd.partition_all_reduce(dbc[:], P[:], channels=128,
                                               reduce_op=bass_isa.ReduceOp.add)
                dbcs[i] = dbc
                if i == 1:
                    nc.vector.tensor_add(acc[:], dbcs[0][:], dbc[:])
                elif i > 1:
                    nc.vector.tensor_add(acc[:], acc[:], dbc[:])
                return P

            for i, si in enumerate(ent):
                Ps[i] = softmax_si(i, si)
                if prev is not None and ndc < 4:
                    outproj_dc(prev[0], prev[1], prev[2], ndc, False)
                    ndc += 1
                # heads 0 and 2 accumulate si-pipelined, each contiguous
                # within its own PSUM bank
                nc.tensor.matmul(qkv_a[:, 0:128],
                                 V[:, si * 128:(si + 1) * 128],
                                 Ps[i][:, 0:128],
                                 start=(i == 0), stop=(i == nv - 1))
                nc.tensor.matmul(qkv_b[:, 0:128],
                                 V[:, si * 128:(si + 1) * 128],
                                 Ps[i][:, 256:384],
                                 start=(i == 0), stop=(i == nv - 1))
            if prev is not None:
                while ndc < 4:
                    outproj_dc(prev[0], prev[1], prev[2], ndc, False)
                    ndc += 1
            for i, si in enumerate(ent):
                nc.tensor.matmul(qkv_a[:, 128:256],
                                 V[:, si * 128:(si + 1) * 128],
                                 Ps[i][:, 128:256],
                                 start=(i == 0), stop=(i == nv - 1))
            for i, si in enumerate(ent):
                nc.tensor.matmul(qkv_b[:, 128:256],
                                 V[:, si * 128:(si + 1) * 128],
                                 Ps[i][:, 384:512],
                                 start=(i == 0), stop=(i == nv - 1))
            Ps.clear()

            rec = sbD.tile([128, 512], F16, tag="recg", name="recg")
            nc.vector.reciprocal(rec[:], acc[:] if nv > 1 else dbcs[0][:])
            nc.vector.tensor_mul(qkvh[:, 0:128], qkv_a[:, 0:128], rec[:, 0:128])
            nc.vector.tensor_mul(qkvh[:, 128:256], qkv_a[:, 128:256], rec[:, 128:256])
            nc.vector.tensor_mul(qkvh[:, 256:384], qkv_b[:, 0:128], rec[:, 256:384])
            nc.vector.tensor_mul(qkvh[:, 384:512], qkv_b[:, 128:256], rec[:, 384:512])
            prev = (tb, qkvh, ob)
        for dc in range(4):
            outproj_dc(prev[0], prev[1], prev[2], dc, True)

    nc.finalize()
    return nc


_CACHE = {}


def kernel(x, segment_ids, Wq, Wk, Wv, Wo, q_scale, k_scale):
    global LAST_RESULTS
    import os

    x = np.asarray(x, np.float32)
    seg = np.asarray(segment_ids)
    Wq = np.asarray(Wq, np.float32)
    Wk = np.asarray(Wk, np.float32)
    Wv = np.asarray(Wv, np.float32)
    Wo = np.asarray(Wo, np.float32)
    q_scale = np.asarray(q_scale, np.float32)
    k_scale = np.asarray(k_scale, np.float32)

    plan, masks = _classify([seg[b] for b in range(B)])
    key = repr(plan)
    if key not in _CACHE:
        _CACHE[key] = _build_nc(plan, masks[0].shape[1])
    nc = _CACHE[key]

    half = H // 2
    timescale = ROPE_BASE ** (2.0 * np.arange(half, dtype=np.float64) / H)
    qscA = np.tile(q_scale[:64], 2).astype(np.float64)[:, None]
    qscB = np.tile(q_scale[64:], 2).astype(np.float64)[:, None]
    kvec = k_scale.astype(np.float64)[:, None]
    tabs = []  # per batch: (cqa, sqa, cqb, sqb, ckt, skt)
    for b in range(B):
        pos = _positions(seg[b])
        sinus = pos[:, None] / timescale[None, :]
        sT = np.sin(sinus).T
        cT = np.cos(sinus).T
        c2 = np.vstack([cT, cT])
        s2 = np.vstack([sT, sT])
        tabs.append(tuple(
            np.ascontiguousarray(a, np.float16)
            for a in (c2 * qscA, s2 * qscA, c2 * qscB, s2 * qscB,
                      c2 * kvec, s2 * kvec)))

    tblf = np.zeros((128, 7), np.float32)
    tblf[:, 0] = np.tile(q_scale[:64], 2)
    tblf[:, 1] = np.tile(q_scale[64:], 2)
    tblf[0:64, 2] = k_scale[:64]
    tblf[64:128, 3] = k_scale[64:]
    tblf[:, 4] = H * EPS
    tblf[:, 5] = EPS
    tblf[:, 6] = EXPB
    tblh = np.zeros((128, 194), np.float16)
    tblh[0:64, 0] = 1.0
    tblh[64:128, 64] = 1.0
    tblh[:, 65] = 1.0
    tblh[:, 66:194] = np.eye(128, dtype=np.float16)

    in_maps = []
    for core in range(8):
        b, kv = core // K, core % K
        qcols = []
        for hv in range(2):
            for g4 in range(G):
                base = kv * 512 + g4 * 128 + hv * 64
                qcols.extend(range(base, base + 64))
        qp = np.array(qcols)
        wq_t = np.ascontiguousarray(
            Wq[:, qp].reshape(ND, 128, G, 128).transpose(2, 1, 0, 3)
            .reshape(G, 128, ND * 128), np.float16)
        wk_t = np.ascontiguousarray(
            Wk[:, kv * 128:(kv + 1) * 128].reshape(ND, 128, 128)
            .transpose(1, 0, 2).reshape(128, ND * 128), np.float16)
        wv_t = np.ascontiguousarray(
            Wv[:, kv * 128:(kv + 1) * 128].reshape(ND, 128, 128)
            .transpose(1, 0, 2).reshape(128, ND * 128), np.float16)
        wo_t = np.ascontiguousarray(
            Wo[kv * 512:(kv + 1) * 512].reshape(G, 128, D), np.float16)
        xt_t = np.ascontiguousarray(
            x[b].T.reshape(ND, 128, T), np.float16)
        cqa, sqa, cqb, sqb, ckt, skt = tabs[b]
        in_maps.append({
            "xT": xt_t, "wq": wq_t, "wk": wk_t, "wv": wv_t, "wo": wo_t,
            "cqa": cqa, "sqa": sqa, "cqb": cqb, "sqb": sqb,
            "ckt": ckt, "skt": skt,
            "tblf": tblf, "tblh": tblh, "masks": masks[b],
        })

    do_trace = os.environ.get("BASS_TRACE") == "1"
    res = run_bass_kernel_spmd(
        nc, in_maps, core_ids=list(range(8)), trace=do_trace)
    LAST_RESULTS = res

    out = np.zeros((B, T, D), np.float32)
    for core in range(8):
        out[core // K] += res.results[core]["out"].astype(np.float32)
    return out


# revision 18
# speedup vs baseline: 1.5859x; 1.0001x over previous
"""Trainium2 Bass kernel v2 for segment-causal GQA attention.

Sharding: 8 cores = batch (2) x kv-head (4); host sums the 4 row-parallel
Wo partial outputs per batch.  All device compute in fp16 (1 PE cycle/row
at any moving width, 2-byte DVE fast modes, half the DMA bytes of fp32).

Layout per core (T=1024, D=2048, H=128, G=4 q-heads):
  xt      [128, 16*1024]  x[b]^T d-tiles side by side (4 DMA'd groups)
  qh[g]   [128, T]   rope'd, rstd-scaled q per head (transposed)
  kTn     [128, T]   rope'd k, with SCALE*rstd_k folded in per-column
  V       [128, 8*128]  v in [s,h] layout per 128-s-block (direct proj)
  attention: per 128-wide t-block tb, the <=4 valid s-blocks' logits are
  packed into one PSUM bank [128, nv*128]; one exp (bias=-4 keeps P in
  fp16 range without max-subtraction), one packed mask multiply, per-
  block qkv/den accumulation; the out-projection of each tb (4x4
  matmuls into [128t, 512d] psums) interleaves with the next tb's
  softmax work to keep the PE saturated.
"""

import sys

sys.path.insert(0, "/opt/trn_rl_repo")

import numpy as np

import concourse.bacc as bacc
import concourse.bass as bass  # noqa: F401
import concourse.tile as tile
from concourse import mybir
from concourse.bass_utils import run_bass_kernel_spmd

B, T, D = 2, 1024, 2048
N, K, H = 16, 4, 128
G = N // K
EPS = 1e-6
SCALE = H ** -0.5
ROPE_BASE = 10000.0
NCHUNK = 2
CW = T // NCHUNK        # 512
NTB = T // 128          # 8 t-blocks (and s-blocks)
ND = D // 128           # 16
F32 = mybir.dt.float32
F16 = mybir.dt.float16
MULT = mybir.AluOpType.mult
EXPB = -4.0             # exp bias: keeps P in fp16 range without max-sub

LAST_RESULTS = None


def _positions(seg):
    t = seg.shape[0]
    idx = np.arange(t, dtype=np.int64)
    is_start = np.concatenate([[True], seg[1:] != seg[:-1]])
    seg_start = np.maximum.accumulate(np.where(is_start, idx, 0))
    return (idx - seg_start).astype(np.float64)


def _classify(seg_rows):
    """Union-over-batches 128x128 block plan.

    Returns (plan, masks): plan[tb] = list of valid s-block indices;
    masks[b] = fp16 [128, n_blocks*128] 0/1 pack in plan order.
    """
    idx = np.arange(T)
    valids = []
    for b in range(B):
        seg = seg_rows[b]
        valids.append((seg[:, None] == seg[None, :]) & (idx[:, None] <= idx[None, :]))
    plan = []
    packs = [[] for _ in range(B)]
    for tb in range(NTB):
        t0 = tb * 128
        ent = []
        for si in range(NTB):
            s0 = si * 128
            subs = [v[s0:s0 + 128, t0:t0 + 128] for v in valids]
            if any(s.any() for s in subs):
                ent.append(si)
                for b in range(B):
                    packs[b].append(subs[b])
        plan.append(ent)
    masks = []
    for b in range(B):
        if packs[b]:
            m = np.concatenate([np.tile(p, (1, 4)) for p in packs[b]], axis=1)
        else:
            m = np.zeros((128, 512), bool)
        masks.append(np.ascontiguousarray(m.astype(np.float16)))
    return plan, masks


def _build_nc(plan, n_mask_cols):
    from contextlib import ExitStack

    nc = bacc.Bacc(None, target_bir_lowering=False, debug=False)
    xT_d = nc.dram_tensor("xT", [ND, 128, T], F16, kind="ExternalInput")
    wq_d = nc.dram_tensor("wq", [G, 128, ND * 128], F16, kind="ExternalInput")
    wk_d = nc.dram_tensor("wk", [128, ND * 128], F16, kind="ExternalInput")
    wv_d = nc.dram_tensor("wv", [128, ND * 128], F16, kind="ExternalInput")
    wo_d = nc.dram_tensor("wo", [G, 128, D], F16, kind="ExternalInput")
    # prescaled rope tables: cos/sin x per-partition rms-scale columns
    cqa_d = nc.dram_tensor("cqa", [128, T], F16, kind="ExternalInput")
    sqa_d = nc.dram_tensor("sqa", [128, T], F16, kind="ExternalInput")
    cqb_d = nc.dram_tensor("cqb", [128, T], F16, kind="ExternalInput")
    sqb_d = nc.dram_tensor("sqb", [128, T], F16, kind="ExternalInput")
    ckt_d = nc.dram_tensor("ckt", [128, T], F16, kind="ExternalInput")
    skt_d = nc.dram_tensor("skt", [128, T], F16, kind="ExternalInput")
    tblf_d = nc.dram_tensor("tblf", [128, 7], F32, kind="ExternalInput")
    tblh_d = nc.dram_tensor("tblh", [128, 194], F16, kind="ExternalInput")
    msk_d = nc.dram_tensor("masks", [128, n_mask_cols], F16, kind="ExternalInput")
    out_d = nc.dram_tensor("out", [T, D], F16, kind="ExternalOutput")

    es = ExitStack()
    with es:
        es.enter_context(nc.allow_low_precision("fp16 kernel"))
        tc = es.enter_context(tile.TileContext(nc))
        pool = lambda *a, **k: es.enter_context(tc.tile_pool(*a, **k))
        pp = pool(name="persist", bufs=1)

        # ---------------- persistent tiles ----------------
        xt = pp.tile([128, ND * T], F16, tag="xt", name="xt")  # 4MB
        qhp = pp.tile([128, NTB * G * 128], F16, tag="qhp", name="qhp")
        qhv = qhp[:].rearrange("p (a g t) -> p a g t", a=NTB, g=G)
        kTn = pp.tile([128, T], F16, tag="kTn", name="kTn")
        V = pp.tile([128, NTB * 128], F16, tag="V", name="V")
        wqs = [pp.tile([128, ND * 128], F16, tag=f"wq{g}", name=f"wq{g}")
               for g in range(G)]
        wk_sb = pp.tile([128, ND * 128], F16, tag="wk", name="wk")
        wv_sb = pp.tile([128, ND * 128], F16, tag="wv", name="wv")
        wo_sb = [pp.tile([128, D], F16, tag=f"wo{g}", name=f"wo{g}")
                 for g in range(G)]
        cqa = pp.tile([128, T], F16, tag="cqa", name="cqa")
        sqa = pp.tile([128, T], F16, tag="sqa", name="sqa")
        cqb = pp.tile([128, T], F16, tag="cqb", name="cqb")
        sqb = pp.tile([128, T], F16, tag="sqb", name="sqb")
        ckt = pp.tile([128, T], F16, tag="ckt", name="ckt")
        skt = pp.tile([128, T], F16, tag="skt", name="skt")
        tblf = pp.tile([128, 7], F32, tag="tblf", name="tblf")
        tblh = pp.tile([128, 194], F16, tag="tblh", name="tblh")
        msk = pp.tile([128, n_mask_cols], F16, tag="msk", name="msk")

        qsc = tblf[:, 0:2]       # f32 per-partition scalars
        ksc = tblf[:, 2:4]
        biasc = tblf[:, 4:6]     # [:,0]=H*EPS  [:,1]=EPS
        expb = tblf[:, 6:7]      # exp bias column (EXPB)
        sel65 = tblh[:, 0:65]    # half-selector cols at 0 and 64
        ones1 = tblh[:, 65:66]
        iden = tblh[:, 66:194]   # fp16 identity

        # ---------------- DMA issue (consume order) ----------------
        def xt_ap(d):
            return xt[:, d * T:(d + 1) * T]

        xtv = xt[:].rearrange("p (a t) -> p a t", a=ND)
        # startup splits: first 2 d-tiles of x and first 2 d-cols of wqA
        nc.sync.dma_start(wqs[0][:, 0:768], wq_d[0][:, 0:768])
        nc.sync.dma_start(xtv[:, 0:2, :], xT_d[0:2].transpose([1, 0, 2]))
        nc.sync.dma_start(wqs[2][:, 0:768], wq_d[2][:, 0:768])
        nc.sync.dma_start(xtv[:, 2:4, :], xT_d[2:4].transpose([1, 0, 2]))
        nc.sync.dma_start(tblf[:], tblf_d[:])
        nc.sync.dma_start(tblh[:], tblh_d[:])
        nc.sync.dma_start(xtv[:, 4:6, :], xT_d[4:6].transpose([1, 0, 2]))
        wq_lo = 768
        for i in range(3, 8):
            wq_hi = min(2048, wq_lo + 256)
            if wq_lo < 2048:
                nc.sync.dma_start(wqs[0][:, wq_lo:wq_hi], wq_d[0][:, wq_lo:wq_hi])
                nc.sync.dma_start(wqs[2][:, wq_lo:wq_hi], wq_d[2][:, wq_lo:wq_hi])
                wq_lo = wq_hi
            nc.sync.dma_start(xtv[:, 2 * i:2 * i + 2, :],
                              xT_d[2 * i:2 * i + 2].transpose([1, 0, 2]))
        nc.sync.dma_start(wqs[1][:], wq_d[1])
        nc.sync.dma_start(wqs[3][:], wq_d[3])
        nc.sync.dma_start(cqa[:], cqa_d[:])
        nc.sync.dma_start(sqa[:], sqa_d[:])
        nc.sync.dma_start(cqb[:], cqb_d[:])
        nc.sync.dma_start(sqb[:], sqb_d[:])
        nc.sync.dma_start(wk_sb[:], wk_d[:])
        nc.sync.dma_start(ckt[:], ckt_d[:])
        nc.sync.dma_start(skt[:], skt_d[:])
        nc.sync.dma_start(wv_sb[:], wv_d[:])
        nc.sync.dma_start(msk[:], msk_d[:])
        for g in range(G):
            nc.sync.dma_start(wo_sb[g][:], wo_d[g])

        # ---------------- phase-1 pools ----------------
        es1 = ExitStack()
        pool1 = lambda *a, **k: es1.enter_context(tc.tile_pool(*a, **k))
        sbs = pool1(name="sb_stream", bufs=4)
        rsp = pool1(name="ropes", bufs=3)
        vtp = pool1(name="vtp", bufs=1)
        ps1 = ExitStack()
        psproj = ps1.enter_context(tc.tile_pool(name="ps_proj", bufs=4, space="PSUM"))
        ps_ss = ps1.enter_context(tc.tile_pool(name="ps_ss", bufs=1, space="PSUM"))
        ps_v = ps1.enter_context(tc.tile_pool(name="ps_v", bufs=2, space="PSUM"))

        def project4(wa, wb):
            """d-outer accumulation: psums[(fi, c)] = [128, CW] f32."""
            pss = {(fi, c): psproj.tile([128, CW], F32, tag="proj", name="proj")
                   for fi in range(2) for c in range(NCHUNK)}
            for d_i in range(ND):
                for fi, w in enumerate((wa, wb)):
                    for c in range(NCHUNK):
                        nc.tensor.matmul(
                            pss[(fi, c)][:],
                            w[:, d_i * 128:(d_i + 1) * 128],
                            xt_ap(d_i)[:, c * CW:(c + 1) * CW],
                            start=(d_i == 0), stop=(d_i == ND - 1))
            return pss

        def rope(psa, psb, out_a, out_b, cs):
            m1 = sbs.tile([128, CW], F16, tag="m1", name="m1")
            m2 = sbs.tile([128, CW], F16, tag="m2", name="m2")
            nc.vector.tensor_mul(m1[:], psa, cqa[:, cs])
            nc.vector.tensor_mul(m2[:], psb, sqb[:, cs])
            nc.vector.tensor_sub(out_a, m1[:], m2[:])
            nc.vector.tensor_mul(m1[:], psb, cqb[:, cs])
            nc.vector.tensor_mul(m2[:], psa, sqa[:, cs])
            nc.vector.tensor_add(out_b, m1[:], m2[:])

        # warm the Exp activation table early so the load is off the
        # attention critical path
        warm = sbs.tile([1, 2], F16, tag="warm", name="warm")

        # ---------------- q pairs ----------------
        for pi in range(2):
            wa, wb = (wqs[0], wqs[2]) if pi == 0 else (wqs[1], wqs[3])
            ga, gb = (0, 1) if pi == 0 else (2, 3)
            pss = project4(wa, wb)
            pcs = {}
            for c in range(NCHUNK):
                pca = sbs.tile([128, CW], F16, tag="pca", name="pca")
                pcb = sbs.tile([128, CW], F16, tag="pcb", name="pcb")
                nc.scalar.copy(pca[:], pss[(0, c)][:])
                nc.vector.tensor_copy(pcb[:], pss[(1, c)][:])
                pcs[c] = (pca, pcb)
            for c in range(NCHUNK):
                cs = slice(c * CW, (c + 1) * CW)
                pca, pcb = pcs[c]
                ssq = ps_ss.tile([65, CW], F32, tag="ss", name="ss")
                for i, pc in enumerate([pca, pcb]):
                    sq = sbs.tile([128, CW], F16, tag="sq", name="sq")
                    nc.vector.tensor_mul(sq[:], pc[:], pc[:])
                    nc.tensor.matmul(ssq[:], sel65, sq[:], start=(i == 0), stop=(i == 1))
                ra = rsp.tile([128, CW], F16, tag="ra", name="ra")
                rb = rsp.tile([128, CW], F16, tag="rb", name="rb")
                rope(pca[:], pcb[:], ra[:], rb[:], cs)
                stmp0 = sbs.tile([1, CW], F32, tag="stmp0", name="stmp0")
                stmp1 = sbs.tile([1, CW], F32, tag="stmp1", name="stmp1")
                nc.scalar.activation(stmp0[:], ssq[0:1, :],
                                     mybir.ActivationFunctionType.Sqrt,
                                     bias=biasc[0:1, 1:2], scale=float(1.0 / H))
                nc.scalar.activation(stmp1[:], ssq[64:65, :],
                                     mybir.ActivationFunctionType.Sqrt,
                                     bias=biasc[0:1, 1:2], scale=float(1.0 / H))
                rstd0 = sbs.tile([1, CW], F16, tag="rstd0", name="rstd0")
                rstd1 = sbs.tile([1, CW], F16, tag="rstd1", name="rstd1")
                nc.vector.reciprocal(rstd0[:], stmp0[:])
                nc.vector.reciprocal(rstd1[:], stmp1[:])
                bca = sbs.tile([128, CW], F16, tag="bca", name="bca")
                bcb = sbs.tile([128, CW], F16, tag="bcb", name="bcb")
                nc.gpsimd.partition_broadcast(bca[:], rstd0[:], channels=128)
                nc.gpsimd.partition_broadcast(bcb[:], rstd1[:], channels=128)
                tbs = slice(4 * c, 4 * c + 4)
                r3 = lambda ap: ap.rearrange("p (a t) -> p a t", a=4)
                nc.vector.tensor_mul(qhv[0:64, tbs, ga, :], r3(ra[0:64, :]),
                                     r3(bca[0:64, :]))
                nc.vector.tensor_mul(qhv[0:64, tbs, gb, :], r3(ra[64:128, :]),
                                     r3(bcb[64:128, :]))
                nc.vector.tensor_mul(qhv[64:128, tbs, ga, :], r3(rb[0:64, :]),
                                     r3(bca[0:64, :]))
                nc.vector.tensor_mul(qhv[64:128, tbs, gb, :], r3(rb[64:128, :]),
                                     r3(bcb[64:128, :]))

        # ---------------- k ----------------
        for c in range(NCHUNK):
            cs = slice(c * CW, (c + 1) * CW)
            psk = psproj.tile([128, CW], F32, tag="proj", name="proj")
            for d_i in range(ND):
                nc.tensor.matmul(psk[:], wk_sb[:, d_i * 128:(d_i + 1) * 128],
                                 xt_ap(d_i)[:, cs],
                                 start=(d_i == 0), stop=(d_i == ND - 1))
            pck = sbs.tile([128, CW], F16, tag="pck", name="pck")
            nc.scalar.copy(pck[:], psk[:])
            sqk = sbs.tile([128, CW], F16, tag="sqk", name="sqk")
            nc.vector.tensor_mul(sqk[:], pck[:], pck[:])
            # row-form sumsq -> sexp = 1/sqrt(sumsq + H*eps) = SCALE*rstd_k
            kssr = ps_ss.tile([1, CW], F32, tag="kssr", name="kssr")
            nc.tensor.matmul(kssr[:], ones1, sqk[:], start=True, stop=True)
            ktmp = sbs.tile([1, CW], F32, tag="ktmp", name="ktmp")
            nc.scalar.activation(ktmp[:], kssr[:],
                                 mybir.ActivationFunctionType.Sqrt,
                                 bias=biasc[0:1, 0:1], scale=1.0)
            krst = sbs.tile([1, CW], F16, tag="krst", name="krst")
            nc.vector.reciprocal(krst[:], ktmp[:])
            last_ktmp = ktmp
            m1 = sbs.tile([128, CW], F16, tag="m1", name="m1")
            m2 = sbs.tile([128, CW], F16, tag="m2", name="m2")
            k0, k1 = pck[0:64, :], pck[64:128, :]
            nc.vector.tensor_mul(m1[0:64, :], k0, ckt[0:64, cs])
            nc.vector.tensor_mul(m2[0:64, :], k1, skt[64:128, cs])
            nc.vector.tensor_sub(kTn[0:64, cs], m1[0:64, :], m2[0:64, :])
            nc.vector.tensor_mul(m1[0:64, :], k1, ckt[64:128, cs])
            nc.vector.tensor_mul(m2[0:64, :], k0, skt[0:64, cs])
            nc.vector.tensor_add(kTn[64:128, cs], m1[0:64, :], m2[0:64, :])
            # fold SCALE*rstd_k into this chunk of kTn
            kbcc = sbs.tile([128, CW], F16, tag="kbcc", name="kbcc")
            nc.gpsimd.partition_broadcast(kbcc[:], krst[:], channels=128)
            nc.vector.tensor_mul(kTn[:, cs], kTn[:, cs], kbcc[:])

        # switch the Act table to the exp set now, off the attention
        # critical path; reading the last sqrt output forces the scheduler
        # to place this after every sqrt-set activation
        nc.scalar.activation(warm[:], last_ktmp[0:1, 0:2],
                             mybir.ActivationFunctionType.Exp,
                             bias=expb[0:1, 0:1], scale=-1.0)

        # ---------------- v: VT projection + PE transposes ----------------
        vts = {}
        for c in range(NCHUNK):
            cs = slice(c * CW, (c + 1) * CW)
            psv = psproj.tile([128, CW], F32, tag="proj", name="proj")
            for d_i in range(ND):
                nc.tensor.matmul(psv[:], wv_sb[:, d_i * 128:(d_i + 1) * 128],
                                 xt_ap(d_i)[:, cs],
                                 start=(d_i == 0), stop=(d_i == ND - 1))
            vt_c = vtp.tile([128, CW], F16, tag=f"vt{c}", name=f"vt{c}")
            nc.scalar.copy(vt_c[:], psv[:])
            vts[c] = vt_c
            for jj in range(4):
                j = 4 * c + jj
                vp = ps_v.tile([128, 128], F16, tag="pv", name="pv")
                nc.tensor.transpose(vp[:], vt_c[:, jj * 128:(jj + 1) * 128], iden)
                if j % 2 == 0:
                    nc.scalar.copy(V[:, j * 128:(j + 1) * 128], vp[:])
                else:
                    nc.vector.tensor_copy(V[:, j * 128:(j + 1) * 128], vp[:])

        es1.close()
        ps1.close()

        # ---------------- attention + out-projection per t-block ----------------
        sbP = pool(name="sbP", bufs=5)
        sbD = pool(name="sbD", bufs=4)
        sbx = pool(name="sbx", bufs=3)
        osp = pool(name="outs", bufs=3)
        ps_lg = pool(name="ps_lg", bufs=4, space="PSUM")
        ps_qkv = pool(name="ps_qkv", bufs=1, space="PSUM")
        ps_op = pool(name="ps_op", bufs=2, space="PSUM")

        moff = []
        off = 0
        for tb in range(NTB):
            moff.append(off)
            off += len(plan[tb])

        from concourse import bass_isa

        def outproj_dc(tb, qkvh, ob, dc, flush):
            t0 = tb * 128
            op = ps_op.tile([128, CW], F32, tag="op", name="op")
            for g in range(G):
                nc.tensor.matmul(op[:],
                                 qkvh[:, g * 128:(g + 1) * 128],
                                 wo_sb[g][:, dc * CW:(dc + 1) * CW],
                                 start=(g == 0), stop=(g == G - 1))
            if dc % 2 == 0:
                nc.vector.tensor_copy(ob[:, dc * CW:(dc + 1) * CW], op[:])
            else:
                nc.scalar.copy(ob[:, dc * CW:(dc + 1) * CW], op[:])
            if flush:
                nc.sync.dma_start(
                    out_d[t0:t0 + 128, dc * CW:(dc + 1) * CW],
                    ob[:, dc * CW:(dc + 1) * CW])
            elif dc == 3:
                nc.sync.dma_start(out_d[t0:t0 + 128, :], ob[:])

        tb_order = sorted(range(NTB), key=lambda t: -len(plan[t]))
        prev = None
        for tb in tb_order:
            ent = plan[tb]
            nv = len(ent)
            t0 = tb * 128
            qkv_a = ps_qkv.tile([128, 256], F32, tag="qkva", name="qkva")
            qkv_b = ps_qkv.tile([128, 256], F32, tag="qkvb", name="qkvb")
            qkvh = sbx.tile([128, 512], F16, tag="qkvh", name="qkvh")
            ob = osp.tile([128, D], F16, tag="ob", name="ob")
            acc = sbD.tile([128, 512], F16, tag="dfold", name="dfold")

            Ps = {}
            dbcs = {}
            ndc = 0

            def softmax_si(i, si):
                """one 512-wide logits matmul for all 4 heads, exp, mask,
                and the Pool denominator reduction for s-block si"""
                lg = ps_lg.tile([128, 512], F32, tag="lg", name="lg")
                nc.tensor.matmul(lg[:], kTn[:, si * 128:(si + 1) * 128],
                                 qhp[:, tb * 512:(tb + 1) * 512],
                                 start=True, stop=True)
                P = sbP.tile([128, 512], F16, tag="P", name="P")
                nc.scalar.activation(P[:], lg[:],
                                     mybir.ActivationFunctionType.Exp,
                                     bias=expb[:, 0:1], scale=1.0)
                nc.vector.tensor_mul(P[:], P[:],
                                     msk[:, (moff[tb] + i) * 512:
                                          (moff[tb] + i + 1) * 512])
                dbc = sbD.tile([128, 512], F16, tag="dbc", name="dbc")
                nc.gpsimd.partition_all_reduce(dbc[:], P[:], channels=128,
                                               reduce_op=bass_isa.ReduceOp.add)
                dbcs[i] = dbc
                if i == 1:
                    nc.vector.tensor_add(acc[:], dbcs[0][:], dbc[:])
                elif i > 1:
                    nc.vector.tensor_add(acc[:], acc[:], dbc[:])
                return P

            for i, si in enumerate(ent):
                Ps[i] = softmax_si(i, si)
                if prev is not None and ndc < 4:
                    outproj_dc(prev[0], prev[1], prev[2], ndc, False)
                    ndc += 1
                # heads 0 and 2 accumulate si-pipelined, each contiguous
                # within its own PSUM bank
                nc.tensor.matmul(qkv_a[:, 0:128],
                                 V[:, si * 128:(si + 1) * 128],
                                 Ps[i][:, 0:128],
                                 start=(i == 0), stop=(i == nv - 1))
                nc.tensor.matmul(qkv_b[:, 0:128],
                                 V[:, si * 128:(si + 1) * 128],
                                 Ps[i][:, 256:384],
                                 start=(i == 0), stop=(i == nv - 1))
            if prev is not None:
                while ndc < 4:
                    outproj_dc(prev[0], prev[1], prev[2], ndc, False)
                    ndc += 1
            for i, si in enumerate(ent):
                nc.tensor.matmul(qkv_a[:, 128:256],
                                 V[:, si * 128:(si + 1) * 128],
                                 Ps[i][:, 128:256],
                                 start=(i == 0), stop=(i == nv - 1))
            for i, si in enumerate(ent):
                nc.tensor.matmul(qkv_b[:, 128:256],
                                 V[:, si * 128:(si + 1) * 128],
                                 Ps[i][:, 384:512],
                                 start=(i == 0), stop=(i == nv - 1))
            Ps.clear()

            rec = sbD.tile([128, 512], F16, tag="recg", name="recg")
            nc.vector.reciprocal(rec[:], acc[:] if nv > 1 else dbcs[0][:])
            nc.vector.tensor_mul(qkvh[:, 0:128], qkv_a[:, 0:128], rec[:, 0:128])
            nc.vector.tensor_mul(qkvh[:, 128:256], qkv_a[:, 128:256], rec[:, 128:256])
            nc.vector.tensor_mul(qkvh[:, 256:384], qkv_b[:, 0:128], rec[:, 256:384])
            nc.vector.tensor_mul(qkvh[:, 384:512], qkv_b[:, 128:256], rec[:, 384:512])
            prev = (tb, qkvh, ob)
        for dc in range(4):
            outproj_dc(prev[0], prev[1], prev[2], dc, True)

    nc.finalize()
    return nc


_CACHE = {}


def kernel(x, segment_ids, Wq, Wk, Wv, Wo, q_scale, k_scale):
    global LAST_RESULTS
    import os

    x = np.asarray(x, np.float32)
    seg = np.asarray(segment_ids)
    Wq = np.asarray(Wq, np.float32)
    Wk = np.asarray(Wk, np.float32)
    Wv = np.asarray(Wv, np.float32)
    Wo = np.asarray(Wo, np.float32)
    q_scale = np.asarray(q_scale, np.float32)
    k_scale = np.asarray(k_scale, np.float32)

    plan, masks = _classify([seg[b] for b in range(B)])
    key = repr(plan)
    if key not in _CACHE:
        _CACHE[key] = _build_nc(plan, masks[0].shape[1])
    nc = _CACHE[key]

    half = H // 2
    timescale = ROPE_BASE ** (2.0 * np.arange(half, dtype=np.float64) / H)
    qscA = np.tile(q_scale[:64], 2).astype(np.float64)[:, None]
    qscB = np.tile(q_scale[64:], 2).astype(np.float64)[:, None]
    kvec = k_scale.astype(np.float64)[:, None]
    tabs = []  # per batch: (cqa, sqa, cqb, sqb, ckt, skt)
    for b in range(B):
        pos = _positions(seg[b])
        sinus = pos[:, None] / timescale[None, :]
        sT = np.sin(sinus).T
        cT = np.cos(sinus).T
        c2 = np.vstack([cT, cT])
        s2 = np.vstack([sT, sT])
        tabs.append(tuple(
            np.ascontiguousarray(a, np.float16)
            for a in (c2 * qscA, s2 * qscA, c2 * qscB, s2 * qscB,
                      c2 * kvec, s2 * kvec)))

    tblf = np.zeros((128, 7), np.float32)
    tblf[:, 0] = np.tile(q_scale[:64], 2)
    tblf[:, 1] = np.tile(q_scale[64:], 2)
    tblf[0:64, 2] = k_scale[:64]
    tblf[64:128, 3] = k_scale[64:]
    tblf[:, 4] = H * EPS
    tblf[:, 5] = EPS
    tblf[:, 6] = EXPB
    tblh = np.zeros((128, 194), np.float16)
    tblh[0:64, 0] = 1.0
    tblh[64:128, 64] = 1.0
    tblh[:, 65] = 1.0
    tblh[:, 66:194] = np.eye(128, dtype=np.float16)

    in_maps = []
    for core in range(8):
        b, kv = core // K, core % K
        qcols = []
        for hv in range(2):
            for g4 in range(G):
                base = kv * 512 + g4 * 128 + hv * 64
                qcols.extend(range(base, base + 64))
        qp = np.array(qcols)
        wq_t = np.ascontiguousarray(
            Wq[:, qp].reshape(ND, 128, G, 128).transpose(2, 1, 0, 3)
            .reshape(G, 128, ND * 128), np.float16)
        wk_t = np.ascontiguousarray(
            Wk[:, kv * 128:(kv + 1) * 128].reshape(ND, 128, 128)
            .transpose(1, 0, 2).reshape(128, ND * 128), np.float16)
        wv_t = np.ascontiguousarray(
            Wv[:, kv * 128:(kv + 1) * 128].reshape(ND, 128, 128)
            .transpose(1, 0, 2).reshape(128, ND * 128), np.float16)
        wo_t = np.ascontiguousarray(
            Wo[kv * 512:(kv + 1) * 512].reshape(G, 128, D), np.float16)
        xt_t = np.ascontiguousarray(
            x[b].T.reshape(ND, 128, T), np.float16)
        cqa, sqa, cqb, sqb, ckt, skt = tabs[b]
        in_maps.append({
            "xT": xt_t, "wq": wq_t, "wk": wk_t, "wv": wv_t, "wo": wo_t,
            "cqa": cqa, "sqa": sqa, "cqb": cqb, "sqb": sqb,
            "ckt": ckt, "skt": skt,
            "tblf": tblf, "tblh": tblh, "masks": masks[b],
        })

    do_trace = os.environ.get("BASS_TRACE") == "1"
    res = run_bass_kernel_spmd(
        nc, in_maps, core_ids=list(range(8)), trace=do_trace)
    LAST_RESULTS = res

    out = np.zeros((B, T, D), np.float32)
    for core in range(8):
        out[core // K] += res.results[core]["out"].astype(np.float32)
    return out
